# revision 1
# baseline (speedup 1.0000x reference)
"""Grouped-index Conv1D (moe_routing) on 8 TRN2 NeuronCores.

Math:  out[b,d,t] = sum_c sum_k x[b,c,t+k] * W[gi[b,c],d,k] + count0[b]*bias[d]

Device algorithm (per core, 2 batches, data-parallel over batch):
  1. one-hot M[c,g] = (gi[b,c]==g) built on-chip (iota + tensor_scalar is_equal)
  2. S[g,t] = sum_c M[c,g]*x[c,t]          (PE: one-hot matmul, contraction=256)
  3. out[d,t] = sum_k Wk[g,d]^T S[g,t+k]   (PE: 7 shifted matmuls accumulated
                                            in PSUM, contraction=16)
  4. bias: counts via ones-matmul, count0*bias broadcast via 1-row matmul,
     fused add on DVE during PSUM->SBUF evacuation.
"""

import sys
import numpy as np

sys.path.insert(0, "/opt/trn_rl_repo")

BS, CH, T = 16, 256, 2048
G, D, K = 16, 64, 7
T_OUT = T - K + 1  # 2042
N_CORES = 8
BPC = BS // N_CORES  # batches per core = 2

# config string: "<s_dtype>/<conv_mode>/<conv_dtype>"
#   s_dtype: f32 | f32r      (S-stage matmul input dtype)
#   conv_mode: kaccum | swin (7 contraction-16 matmuls vs DRAM-bounce
#                             replication + 1 contraction-112 matmul)
#   conv_dtype: f32 | f32r
MM_DTYPE = "f32r/swin/f32r"

_COMPILED = {}


def _build(cfg: str):
    from concourse import bacc, tile
    import concourse.mybir as mybir

    s_dt, conv_mode, conv_dt = (cfg.split("/") + ["kaccum", "f32"])[:3] \
        if "/" in cfg else (cfg, "kaccum", cfg)
    f32 = mybir.dt.float32
    f32r = mybir.dt.float32r
    eq = mybir.AluOpType.is_equal
    add = mybir.AluOpType.add
    # f32r matmul inputs must be produced by engine ops (which round) or
    # DMAs whose source data is already f32r-rounded.
    mmdt = f32r if s_dt == "f32r" else f32
    use_r = s_dt == "f32r"
    cdt = f32r if conv_dt == "f32r" else f32

    nc = bacc.Bacc("TRN2", target_bir_lowering=False, debug=False,
                   num_devices=N_CORES)
    x_ext = nc.dram_tensor("x", [BPC, CH, T], f32, kind="ExternalInput").ap()
    gi_ext = nc.dram_tensor("gi", [BPC, 2, 128, 1], f32, kind="ExternalInput").ap()
    wt_ext = nc.dram_tensor("wt", [G, K * D], f32, kind="ExternalInput").ap()
    ws_ext = nc.dram_tensor("ws", [K * G, D], f32, kind="ExternalInput").ap()
    b_ext = nc.dram_tensor("bias", [1, D], f32, kind="ExternalInput").ap()
    io_ext = nc.dram_tensor("iota", [128, G], f32, kind="ExternalInput").ap()
    out_ext = nc.dram_tensor("out", [BPC, D, T_OUT], f32, kind="ExternalOutput").ap()
    swin = conv_mode == "swin"
    if swin:
        s_dram = nc.dram_tensor("s_dram", [BPC, G, T], cdt).ap()

    NCHUNK = T // 512  # 4

    with tile.TileContext(nc) as tc:
        with (
            tc.tile_pool(name="const", bufs=1) as cpool,
            tc.tile_pool(name="work", bufs=2) as wpool,
            tc.tile_pool(name="ps_pool", bufs=2, space="PSUM") as ppool,
            tc.tile_pool(name="psmall", bufs=1, space="PSUM") as spool,
            tc.tile_pool(name="po_pool", bufs=4, space="PSUM") as opool,
        ):
            iota_f = cpool.tile([128, G], f32, name="iota_f")
            nc.sync.dma_start(iota_f[:], io_ext[:])
            if swin:
                # stationary for the 112-contraction conv: ws[k*16+g, d]
                ws_sb = cpool.tile([K * G, D], f32, name="ws_sb")
                nc.sync.dma_start(ws_sb[:], ws_ext[:])
                if cdt == f32r:
                    ws_r = cpool.tile([K * G, D], f32r, name="ws_r")
                    nc.vector.tensor_copy(ws_r[:], ws_sb[:])
                else:
                    ws_r = ws_sb
            else:
                wt_sb = cpool.tile([G, K * D], f32, name="wt_sb")
                nc.sync.dma_start(wt_sb[:], wt_ext[:])
                if cdt == f32r:
                    wt_r = cpool.tile([G, K * D], f32r, name="wt_r")
                    nc.vector.tensor_copy(wt_r[:], wt_sb[:])
                else:
                    wt_r = wt_sb
            brow = cpool.tile([1, D], f32, name="brow")
            nc.sync.dma_start(brow[:], b_ext[:])
            ones_col = cpool.tile([128, 1], f32, name="ones_col")
            nc.vector.memset(ones_col[:], 1.0)

            # Stage-major emission: both batches' loads first, then both S
            # stages, then bounces, then convs — so batch 1 PE work fills
            # batch 0's DMA-bounce latency.

            # --- stage G: tiny control DMAs first so they don't queue
            # behind megabytes of x traffic (everything depends on them) ---
            gi_all = []
            for b in range(BPC):
                gis = []
                for h in range(2):
                    gi_t = wpool.tile([128, 1], f32, name=f"gi{b}{h}",
                                      tag="gi", bufs=4)
                    nc.sync.dma_start(gi_t[:], gi_ext[b, h])
                    gis.append(gi_t)
                gi_all.append(gis)

            # --- stage X: all x DMAs (+ f32r rounding on scalar engine) ---
            xps = [[[None, None], [None, None]] for _ in range(BPC)]
            for b in range(BPC):
                for h in range(2):
                    for p in range(2):
                        t_ = wpool.tile([128, 1024], f32, name=f"xp{b}{h}{p}",
                                        tag="xp", bufs=8)
                        nc.sync.dma_start(
                            t_[:],
                            x_ext[b, 128 * h:128 * (h + 1),
                                  1024 * p:1024 * (p + 1)])
                        if use_r:
                            # split the f32r rounding passes between DVE and
                            # ACT so neither engine serializes the S stage
                            xr = wpool.tile([128, 1024], f32r,
                                            name=f"xr{b}{h}{p}", tag="xr",
                                            bufs=8)
                            if (h + p) % 2 == 0:
                                nc.vector.tensor_copy(xr[:], t_[:])
                            else:
                                nc.scalar.activation(
                                    xr[:], t_[:],
                                    mybir.ActivationFunctionType.Copy)
                            t_ = xr
                        xps[b][h][p] = t_

            # --- stage A: one-hot + counts + per-batch bias vector ---
            ms_all, bcnt_all = [], []
            for b in range(BPC):
                ms, ms_f = [], []
                for h in range(2):
                    gi_t = gi_all[b][h]
                    mf_t = wpool.tile([128, G], f32, name=f"mf{b}{h}",
                                      tag="mf", bufs=4)
                    nc.vector.tensor_scalar(out=mf_t[:], in0=iota_f[:],
                                            scalar1=gi_t[:, 0:1], scalar2=None,
                                            op0=eq)
                    ms_f.append(mf_t)
                    if use_r:
                        m_t = wpool.tile([128, G], f32r, name=f"m{b}{h}",
                                         tag="m", bufs=4)
                        nc.vector.tensor_copy(m_t[:], mf_t[:])
                    else:
                        m_t = mf_t
                    ms.append(m_t)
                ms_all.append(ms)

                pcnt = spool.tile([G, 1], f32, name=f"pcnt{b}", tag="pcnt")
                nc.tensor.matmul(pcnt[:], ms_f[0][:], ones_col[:],
                                 start=True, stop=False)
                nc.tensor.matmul(pcnt[:], ms_f[1][:], ones_col[:],
                                 start=False, stop=True)
                cnt_sb = wpool.tile([G, 1], f32, name=f"cnt{b}", tag="cnt")
                nc.vector.tensor_copy(cnt_sb[:], pcnt[:])
                pbc = spool.tile([D, 1], f32, name=f"pbc{b}", tag="pbc")
                nc.tensor.matmul(pbc[:], brow[:], cnt_sb[0:1, 0:1],
                                 start=True, stop=True)
                bcnt = wpool.tile([D, 1], f32, name=f"bcnt{b}", tag="bcnt")
                nc.vector.tensor_copy(bcnt[:], pbc[:])
                bcnt_all.append(bcnt)

            # --- stage B: S = M^T @ X (+ per-chunk DRAM spill for swin) ---
            s_all = []
            for b in range(BPC):
                xp = xps[b]
                s_sb = wpool.tile([G, T], cdt, name=f"s{b}", tag="s")
                for c in range(NCHUNK):
                    ps = ppool.tile([G, 512], f32, name=f"ps{b}{c}", tag="ps")
                    off = 512 * c
                    p, o = off // 1024, off % 1024
                    nc.tensor.matmul(ps[:], ms_all[b][0][:],
                                     xp[0][p][:, o:o + 512],
                                     start=True, stop=False)
                    nc.tensor.matmul(ps[:], ms_all[b][1][:],
                                     xp[1][p][:, o:o + 512],
                                     start=False, stop=True)
                    nc.vector.tensor_copy(s_sb[:, off:off + 512], ps[:])
                    if swin:
                        nc.sync.dma_start(s_dram[b, :, off:off + 512],
                                          s_sb[:, off:off + 512])
                s_all.append(s_sb)

            # --- stage C: replicate S onto partitions 16k+g with per-k
            # column shifts via DRAM (engines cannot cross partitions) ---
            swin_all = []
            if swin:
                for b in range(BPC):
                    swin_sb = wpool.tile([K * G, T_OUT], cdt,
                                         name=f"swin{b}", tag="swin")
                    # column halves so conv chunks 0-1 start after half the
                    # replication traffic has landed
                    half = 1024
                    for lo, hi in ((0, half), (half, T_OUT)):
                        for k in range(K):
                            nc.sync.dma_start(
                                swin_sb[G * k:G * (k + 1), lo:hi],
                                s_dram[b, :, k + lo:k + hi])
                    swin_all.append(swin_sb)

            # --- stage D: conv + bias-add + store ---
            for b in range(BPC):
                for c in range(NCHUNK):
                    c0 = 512 * c
                    L = min(512, T_OUT - c0)
                    po = opool.tile([D, 512], f32, name=f"po{b}{c}", tag="po")
                    if swin:
                        nc.tensor.matmul(po[:, :L], ws_r[:],
                                         swin_all[b][:, c0:c0 + L],
                                         start=True, stop=True)
                    else:
                        for k in range(K):
                            nc.tensor.matmul(po[:, :L],
                                             wt_r[:, D * k:D * (k + 1)],
                                             s_all[b][:, c0 + k:c0 + k + L],
                                             start=(k == 0), stop=(k == K - 1))
                    osb = wpool.tile([D, 512], f32, name=f"osb{b}{c}",
                                     tag="osb", bufs=4)
                    nc.scalar.activation(osb[:, :L], po[:, :L],
                                         mybir.ActivationFunctionType.Identity,
                                         bias=bcnt_all[b][:, 0:1])
                    nc.sync.dma_start(out_ext[b, :, c0:c0 + L], osb[:, :L])

    nc.compile()
    return nc


def _get_nc(mm_dtype: str):
    if mm_dtype not in _COMPILED:
        _COMPILED[mm_dtype] = _build(mm_dtype)
    return _COMPILED[mm_dtype]


def _run(x, group_idxs, W, bias, mm_dtype=None, trace=False, tmpdir=None):
    from concourse.bass_utils import run_bass_kernel_spmd

    x = np.ascontiguousarray(np.asarray(x, dtype=np.float32))
    gi = np.asarray(group_idxs).astype(np.float32).reshape(BS, 2, 128, 1)
    W = np.asarray(W, dtype=np.float32)
    bias = np.asarray(bias, dtype=np.float32)
    # wt[g, k*64+d] = W[g,d,k];  ws[k*16+g, d] = W[g,d,k]
    wt = np.ascontiguousarray(W.transpose(0, 2, 1).reshape(G, K * D))
    ws = np.ascontiguousarray(W.transpose(2, 0, 1).reshape(K * G, D))
    brow = np.ascontiguousarray(bias.reshape(1, D))
    iota = np.ascontiguousarray(
        np.broadcast_to(np.arange(G, dtype=np.float32), (128, G)))

    nc = _get_nc(mm_dtype or MM_DTYPE)
    in_maps = []
    for i in range(N_CORES):
        sl = slice(i * BPC, (i + 1) * BPC)
        in_maps.append({
            "x": np.ascontiguousarray(x[sl]),
            "gi": np.ascontiguousarray(gi[sl]),
            "wt": wt,
            "ws": ws,
            "bias": brow,
            "iota": iota,
        })
    res = run_bass_kernel_spmd(nc, in_maps, core_ids=list(range(N_CORES)),
                               trace=trace, tmpdir=tmpdir)
    out = np.concatenate([r["out"] for r in res.results], axis=0)
    assert out.shape == (BS, D, T_OUT)
    return out.astype(np.float32), res


def kernel(x, group_idxs, W, bias):
    out, _ = _run(x, group_idxs, W, bias)
    return out



# revision 7
# speedup vs baseline: 1.2470x; 1.2470x over previous
"""Grouped-index Conv1D (moe_routing) on 8 TRN2 NeuronCores.

Math:  out[b,d,t] = sum_c sum_k x[b,c,t+k] * W[gi[b,c],d,k] + count0[b]*bias[d]

v2 device algorithm (per core, 2 batches, data-parallel over batch):
  1. one-hot M[c,g] = (gi[b,c]==g) built on-chip (gpsimd iota + DVE is_equal)
  2. S[g,t] = sum_c M[c,g]*x[c,t]       (PE one-hot matmul, f32r via bitcast,
                                         written into swin block 0)
  3. swin[16r+g, t] = S[g, t+r]         (SBUF->SBUF DMA replication for
                                         r=1..R-1; no DRAM bounce)
  4. out[d,t] = sum_j Wj^T swin[...]    (ceil(7/R) PSUM-accumulated matmuls
                                         per 512-chunk)
  5. bias: count0 via DVE is_equal+reduce on a [1,512] gi row; one tiny
     matmul broadcasts count0*bias to [64,2]; fused add on DVE/ACT during
     PSUM->SBUF evacuation.
DMA queues: x split across qSP/qAct, control+weights on qPool (SWDGE),
replication on qSP, outputs on qAct.  (v1 used one queue: 59 serialized
DMAs ~ 35us; v2 has ~20 spread over 3 queues.)
"""

import sys
import numpy as np

sys.path.insert(0, "/opt/trn_rl_repo")

BS, CH, T = 16, 256, 2048
G, D, K = 16, 64, 7
T_OUT = T - K + 1  # 2042
N_CORES = 8
BPC = BS // N_CORES  # batches per core = 2

# default config: "v2/<R>" with R = partition replicas for the conv
# (R=7: 1 conv matmul/chunk, 6 repl DMAs/batch; R=4: 2 mm, 3 DMAs)
MM_DTYPE = "v2/4"

_COMPILED = {}

NCHUNK = 4  # T / 512


def _build_v2(cfg: str):
    from concourse import bacc, tile
    import concourse.mybir as mybir

    parts = cfg.split("/")
    R = int(parts[1]) if len(parts) > 1 else 4
    assert 1 <= R <= 7
    f32 = mybir.dt.float32
    f32r = mybir.dt.float32r
    eq = mybir.AluOpType.is_equal
    add = mybir.AluOpType.add
    NMM = (K + R - 1) // R  # conv matmuls per chunk
    NP = 16 * R             # swin partitions

    # block b of swin is read by conv matmul j (with blocks 0..len_j-1,
    # column window up to 1536 + j*R + 505); width needed per block:
    blk_w = [0] * R
    for j in range(NMM):
        jR = j * R
        lenj = min(R, K - jR)
        maxcol = 1536 + jR + (T_OUT - 1536) - 1  # 2041 + jR
        for b in range(lenj):
            blk_w[b] = max(blk_w[b], maxcol + 1)
    for b in range(1, R):
        assert blk_w[b] + b <= T, (b, blk_w[b])

    nc = bacc.Bacc("TRN2", target_bir_lowering=False, debug=False,
                   num_devices=N_CORES)
    x_ext = nc.dram_tensor("x", [BPC, CH, T], f32r, kind="ExternalInput").ap()
    git_ext = nc.dram_tensor("git", [128, 2 * BPC], f32, kind="ExternalInput").ap()
    gir_ext = nc.dram_tensor("gir", [1, CH * BPC], f32, kind="ExternalInput").ap()
    ws_ext = nc.dram_tensor("ws", [K * G, D], f32r, kind="ExternalInput").ap()
    b_ext = nc.dram_tensor("bias", [1, D], f32, kind="ExternalInput").ap()
    out_ext = nc.dram_tensor("out", [BPC, D, T_OUT], f32, kind="ExternalOutput").ap()

    with tile.TileContext(nc) as tc:
        with (
            tc.tile_pool(name="const", bufs=1) as cpool,
            tc.tile_pool(name="work", bufs=2) as wpool,
            tc.tile_pool(name="ps_pool", bufs=2, space="PSUM") as ppool,
            tc.tile_pool(name="psmall", bufs=1, space="PSUM") as spool,
            tc.tile_pool(name="po_pool", bufs=4, space="PSUM") as opool,
        ):
            # --- on-chip constants (Pool engine + SWDGE queue) ---
            iota_f = cpool.tile([128, G], f32, name="iota_f")
            nc.gpsimd.iota(iota_f[:], [[1, G]], channel_multiplier=0,
                           allow_small_or_imprecise_dtypes=True)
            # one stationary tile per conv matmul (matmul requires equal
            # base partitions for stationary and moving operands)
            wsj_all = []
            for j in range(NMM):
                jR = j * R
                lenj = min(R, K - jR)
                wsj = cpool.tile([G * lenj, D], f32r, name=f"wsj{j}")
                nc.gpsimd.dma_start(wsj[:], ws_ext[G * jR:G * (jR + lenj), :])
                wsj_all.append(wsj)
            brow = cpool.tile([1, D], f32, name="brow")
            nc.gpsimd.dma_start(brow[:], b_ext[:])
            gir_sb = cpool.tile([1, CH * BPC], f32, name="gir_sb")
            nc.gpsimd.dma_start(gir_sb[:], gir_ext[:])

            # --- qSP: gi (tiny, feeds one-hot) then batch-1 x tiles ---
            git_sb = cpool.tile([128, 2 * BPC], f32, name="git_sb")
            nc.sync.dma_start(git_sb[:], git_ext[:])
            xps = [[[None, None], [None, None]] for _ in range(BPC)]
            for p in range(2):
                for h in range(2):
                    t_ = wpool.tile([128, 1024], f32r, name=f"xp1{h}{p}",
                                    tag="xp", bufs=8)
                    nc.sync.dma_start(
                        t_[:], x_ext[1, 128 * h:128 * (h + 1),
                                     1024 * p:1024 * (p + 1)])
                    xps[1][h][p] = t_
            # --- qAct: batch-0 x tiles ---
            for p in range(2):
                for h in range(2):
                    t_ = wpool.tile([128, 1024], f32r, name=f"xp0{h}{p}",
                                    tag="xp", bufs=8)
                    nc.scalar.dma_start(
                        t_[:], x_ext[0, 128 * h:128 * (h + 1),
                                     1024 * p:1024 * (p + 1)])
                    xps[0][h][p] = t_

            # --- DVE: one-hot + count0 ---
            ms_all = []
            for b in range(BPC):
                ms = []
                for h in range(2):
                    m_t = wpool.tile([128, G], f32r, name=f"m{b}{h}",
                                     tag="m", bufs=4)
                    nc.vector.tensor_scalar(
                        out=m_t[:], in0=iota_f[:],
                        scalar1=git_sb[:, 2 * b + h:2 * b + h + 1],
                        scalar2=None, op0=eq)
                    ms.append(m_t)
                ms_all.append(ms)
            eq0 = wpool.tile([1, CH * BPC], f32, name="eq0", tag="eq0", bufs=1)
            nc.vector.tensor_scalar(out=eq0[:], in0=gir_sb[:], scalar1=0.0,
                                    scalar2=None, op0=eq)
            cnt2 = wpool.tile([1, BPC], f32, name="cnt2", tag="cnt2", bufs=1)
            for b in range(BPC):
                nc.vector.tensor_reduce(
                    out=cnt2[:, b:b + 1], in_=eq0[:, CH * b:CH * (b + 1)],
                    axis=mybir.AxisListType.X, op=add)

            # --- PE: S = M^T @ x, evacuated into swin block 0 (DVE);
            #     then SBUF->SBUF replication DMAs (qSP) ---
            swin_all = []
            for b in range(BPC):
                swin = wpool.tile([NP, T], f32r, name=f"swin{b}", tag="swin")
                for c in range(NCHUNK):
                    off = 512 * c
                    p, o = off // 1024, off % 1024
                    ps = ppool.tile([G, 512], f32, name=f"ps{b}{c}", tag="ps")
                    nc.tensor.matmul(ps[:], ms_all[b][0][:],
                                     xps[b][0][p][:, o:o + 512],
                                     start=True, stop=False)
                    nc.tensor.matmul(ps[:], ms_all[b][1][:],
                                     xps[b][1][p][:, o:o + 512],
                                     start=False, stop=True)
                    nc.vector.tensor_copy(swin[0:G, off:off + 512], ps[:])
                for r in range(1, R):
                    w = blk_w[r]
                    nc.sync.dma_start(swin[G * r:G * (r + 1), 0:w],
                                      swin[0:G, r:r + w])
                swin_all.append(swin)

            # --- PE: count0*bias broadcast to [64, BPC] ---
            pb = spool.tile([D, BPC], f32, name="pb", tag="pb")
            nc.tensor.matmul(pb[:], brow[:], cnt2[:],
                             start=True, stop=True)
            bcnt = wpool.tile([D, BPC], f32, name="bcnt", tag="bcnt", bufs=1)
            nc.vector.tensor_copy(bcnt[:], pb[:])

            # --- conv + fused bias-add + store ---
            for b in range(BPC):
                osb = wpool.tile([D, T_OUT], f32, name=f"osb{b}", tag="osb")
                for c in range(NCHUNK):
                    c0 = 512 * c
                    L = min(512, T_OUT - c0)
                    po = opool.tile([D, 512], f32, name=f"po{b}{c}", tag="po")
                    for j in range(NMM):
                        jR = j * R
                        lenj = min(R, K - jR)
                        nc.tensor.matmul(
                            po[:, :L],
                            wsj_all[j][:],
                            swin_all[b][0:G * lenj,
                                        c0 + jR:c0 + jR + L],
                            start=(j == 0), stop=(j == NMM - 1))
                    # fused bias add during evacuation; alternate DVE/ACT
                    if c % 2 == 0:
                        nc.vector.tensor_scalar(
                            out=osb[:, c0:c0 + L], in0=po[:, :L],
                            scalar1=bcnt[:, b:b + 1], scalar2=None, op0=add)
                    else:
                        nc.scalar.activation(
                            osb[:, c0:c0 + L], po[:, :L],
                            mybir.ActivationFunctionType.Identity,
                            bias=bcnt[:, b:b + 1])
                    if c == 1:
                        nc.scalar.dma_start(out_ext[b, :, 0:1024],
                                            osb[:, 0:1024])
                    elif c == NCHUNK - 1:
                        nc.scalar.dma_start(out_ext[b, :, 1024:T_OUT],
                                            osb[:, 1024:T_OUT])

    nc.compile()
    return nc


# ----------------------------------------------------------------------
# v1 baseline (kept for A/B comparison): config "<s_dtype>/<conv_mode>/<conv_dtype>"
def _build_v1(cfg: str):
    from concourse import bacc, tile
    import concourse.mybir as mybir

    s_dt, conv_mode, conv_dt = (cfg.split("/") + ["kaccum", "f32"])[:3] \
        if "/" in cfg else (cfg, "kaccum", cfg)
    f32 = mybir.dt.float32
    f32r = mybir.dt.float32r
    eq = mybir.AluOpType.is_equal
    mmdt = f32r if s_dt == "f32r" else f32
    use_r = s_dt == "f32r"
    cdt = f32r if conv_dt == "f32r" else f32

    nc = bacc.Bacc("TRN2", target_bir_lowering=False, debug=False,
                   num_devices=N_CORES)
    x_ext = nc.dram_tensor("x", [BPC, CH, T], f32, kind="ExternalInput").ap()
    gi_ext = nc.dram_tensor("gi", [BPC, 2, 128, 1], f32, kind="ExternalInput").ap()
    wt_ext = nc.dram_tensor("wt", [G, K * D], f32, kind="ExternalInput").ap()
    ws_ext = nc.dram_tensor("ws", [K * G, D], f32, kind="ExternalInput").ap()
    b_ext = nc.dram_tensor("bias", [1, D], f32, kind="ExternalInput").ap()
    io_ext = nc.dram_tensor("iota", [128, G], f32, kind="ExternalInput").ap()
    out_ext = nc.dram_tensor("out", [BPC, D, T_OUT], f32, kind="ExternalOutput").ap()
    swin = conv_mode == "swin"
    if swin:
        s_dram = nc.dram_tensor("s_dram", [BPC, G, T], cdt).ap()

    with tile.TileContext(nc) as tc:
        with (
            tc.tile_pool(name="const", bufs=1) as cpool,
            tc.tile_pool(name="work", bufs=2) as wpool,
            tc.tile_pool(name="ps_pool", bufs=2, space="PSUM") as ppool,
            tc.tile_pool(name="psmall", bufs=1, space="PSUM") as spool,
            tc.tile_pool(name="po_pool", bufs=4, space="PSUM") as opool,
        ):
            iota_f = cpool.tile([128, G], f32, name="iota_f")
            nc.sync.dma_start(iota_f[:], io_ext[:])
            if swin:
                ws_sb = cpool.tile([K * G, D], f32, name="ws_sb")
                nc.sync.dma_start(ws_sb[:], ws_ext[:])
                if cdt == f32r:
                    ws_r = cpool.tile([K * G, D], f32r, name="ws_r")
                    nc.vector.tensor_copy(ws_r[:], ws_sb[:])
                else:
                    ws_r = ws_sb
            else:
                wt_sb = cpool.tile([G, K * D], f32, name="wt_sb")
                nc.sync.dma_start(wt_sb[:], wt_ext[:])
                if cdt == f32r:
                    wt_r = cpool.tile([G, K * D], f32r, name="wt_r")
                    nc.vector.tensor_copy(wt_r[:], wt_sb[:])
                else:
                    wt_r = wt_sb
            brow = cpool.tile([1, D], f32, name="brow")
            nc.sync.dma_start(brow[:], b_ext[:])
            ones_col = cpool.tile([128, 1], f32, name="ones_col")
            nc.vector.memset(ones_col[:], 1.0)

            gi_all = []
            for b in range(BPC):
                gis = []
                for h in range(2):
                    gi_t = wpool.tile([128, 1], f32, name=f"gi{b}{h}",
                                      tag="gi", bufs=4)
                    nc.sync.dma_start(gi_t[:], gi_ext[b, h])
                    gis.append(gi_t)
                gi_all.append(gis)

            xps = [[[None, None], [None, None]] for _ in range(BPC)]
            for b in range(BPC):
                for h in range(2):
                    for p in range(2):
                        t_ = wpool.tile([128, 1024], f32, name=f"xp{b}{h}{p}",
                                        tag="xp", bufs=8)
                        nc.sync.dma_start(
                            t_[:],
                            x_ext[b, 128 * h:128 * (h + 1),
                                  1024 * p:1024 * (p + 1)])
                        if use_r:
                            xr = wpool.tile([128, 1024], f32r,
                                            name=f"xr{b}{h}{p}", tag="xr",
                                            bufs=8)
                            if (h + p) % 2 == 0:
                                nc.vector.tensor_copy(xr[:], t_[:])
                            else:
                                nc.scalar.activation(
                                    xr[:], t_[:],
                                    mybir.ActivationFunctionType.Copy)
                            t_ = xr
                        xps[b][h][p] = t_

            ms_all, bcnt_all = [], []
            for b in range(BPC):
                ms, ms_f = [], []
                for h in range(2):
                    gi_t = gi_all[b][h]
                    mf_t = wpool.tile([128, G], f32, name=f"mf{b}{h}",
                                      tag="mf", bufs=4)
                    nc.vector.tensor_scalar(out=mf_t[:], in0=iota_f[:],
                                            scalar1=gi_t[:, 0:1], scalar2=None,
                                            op0=eq)
                    ms_f.append(mf_t)
                    if use_r:
                        m_t = wpool.tile([128, G], f32r, name=f"m{b}{h}",
                                         tag="m", bufs=4)
                        nc.vector.tensor_copy(m_t[:], mf_t[:])
                    else:
                        m_t = mf_t
                    ms.append(m_t)
                ms_all.append(ms)

                pcnt = spool.tile([G, 1], f32, name=f"pcnt{b}", tag="pcnt")
                nc.tensor.matmul(pcnt[:], ms_f[0][:], ones_col[:],
                                 start=True, stop=False)
                nc.tensor.matmul(pcnt[:], ms_f[1][:], ones_col[:],
                                 start=False, stop=True)
                cnt_sb = wpool.tile([G, 1], f32, name=f"cnt{b}", tag="cnt")
                nc.vector.tensor_copy(cnt_sb[:], pcnt[:])
                pbc = spool.tile([D, 1], f32, name=f"pbc{b}", tag="pbc")
                nc.tensor.matmul(pbc[:], brow[:], cnt_sb[0:1, 0:1],
                                 start=True, stop=True)
                bcnt = wpool.tile([D, 1], f32, name=f"bcnt{b}", tag="bcnt")
                nc.vector.tensor_copy(bcnt[:], pbc[:])
                bcnt_all.append(bcnt)

            s_all = []
            for b in range(BPC):
                xp = xps[b]
                s_sb = wpool.tile([G, T], cdt, name=f"s{b}", tag="s")
                for c in range(4):
                    ps = ppool.tile([G, 512], f32, name=f"ps{b}{c}", tag="ps")
                    off = 512 * c
                    p, o = off // 1024, off % 1024
                    nc.tensor.matmul(ps[:], ms_all[b][0][:],
                                     xp[0][p][:, o:o + 512],
                                     start=True, stop=False)
                    nc.tensor.matmul(ps[:], ms_all[b][1][:],
                                     xp[1][p][:, o:o + 512],
                                     start=False, stop=True)
                    nc.vector.tensor_copy(s_sb[:, off:off + 512], ps[:])
                    if swin:
                        nc.sync.dma_start(s_dram[b, :, off:off + 512],
                                          s_sb[:, off:off + 512])
                s_all.append(s_sb)

            swin_all = []
            if swin:
                for b in range(BPC):
                    swin_sb = wpool.tile([K * G, T_OUT], cdt,
                                         name=f"swin{b}", tag="swin")
                    half = 1024
                    for lo, hi in ((0, half), (half, T_OUT)):
                        for k in range(K):
                            nc.sync.dma_start(
                                swin_sb[G * k:G * (k + 1), lo:hi],
                                s_dram[b, :, k + lo:k + hi])
                    swin_all.append(swin_sb)

            for b in range(BPC):
                for c in range(4):
                    c0 = 512 * c
                    L = min(512, T_OUT - c0)
                    po = opool.tile([D, 512], f32, name=f"po{b}{c}", tag="po")
                    if swin:
                        nc.tensor.matmul(po[:, :L], ws_r[:],
                                         swin_all[b][:, c0:c0 + L],
                                         start=True, stop=True)
                    else:
                        for k in range(K):
                            nc.tensor.matmul(po[:, :L],
                                             wt_r[:, D * k:D * (k + 1)],
                                             s_all[b][:, c0 + k:c0 + k + L],
                                             start=(k == 0), stop=(k == K - 1))
                    osb = wpool.tile([D, 512], f32, name=f"osb{b}{c}",
                                     tag="osb", bufs=4)
                    nc.scalar.activation(osb[:, :L], po[:, :L],
                                         mybir.ActivationFunctionType.Identity,
                                         bias=bcnt_all[b][:, 0:1])
                    nc.sync.dma_start(out_ext[b, :, c0:c0 + L], osb[:, :L])

    nc.compile()
    return nc


def _build(cfg: str):
    if cfg.startswith("v2"):
        return _build_v2(cfg)
    return _build_v1(cfg)


def _get_nc(mm_dtype: str):
    if mm_dtype not in _COMPILED:
        _COMPILED[mm_dtype] = _build(mm_dtype)
    return _COMPILED[mm_dtype]


def _run(x, group_idxs, W, bias, mm_dtype=None, trace=False, tmpdir=None):
    from concourse.bass_utils import run_bass_kernel_spmd

    cfg = mm_dtype or MM_DTYPE
    x = np.ascontiguousarray(np.asarray(x, dtype=np.float32))
    gi_f = np.asarray(group_idxs).astype(np.float32)  # [BS, CH]
    W = np.asarray(W, dtype=np.float32)
    bias = np.asarray(bias, dtype=np.float32)
    # ws[k*16+g, d] = W[g,d,k]
    ws = np.ascontiguousarray(W.transpose(2, 0, 1).reshape(K * G, D))
    brow = np.ascontiguousarray(bias.reshape(1, D))
    nc = _get_nc(cfg)

    in_maps = []
    if cfg.startswith("v2"):
        for i in range(N_CORES):
            sl = slice(i * BPC, (i + 1) * BPC)
            gi_c = gi_f[sl]  # [BPC, CH]
            # git[c, 2b+h] = gi[b, h*128+c]
            git = np.ascontiguousarray(
                gi_c.reshape(BPC * 2, 128).T)  # [128, 2*BPC]
            gir = np.ascontiguousarray(gi_c.reshape(1, BPC * CH))
            in_maps.append({
                "x": np.ascontiguousarray(x[sl]),
                "git": git,
                "gir": gir,
                "ws": ws,
                "bias": brow,
            })
    else:
        gi = gi_f.reshape(BS, 2, 128, 1)
        wt = np.ascontiguousarray(W.transpose(0, 2, 1).reshape(G, K * D))
        iota = np.ascontiguousarray(
            np.broadcast_to(np.arange(G, dtype=np.float32), (128, G)))
        for i in range(N_CORES):
            sl = slice(i * BPC, (i + 1) * BPC)
            in_maps.append({
                "x": np.ascontiguousarray(x[sl]),
                "gi": np.ascontiguousarray(gi[sl]),
                "wt": wt,
                "ws": ws,
                "bias": brow,
                "iota": iota,
            })
    res = run_bass_kernel_spmd(nc, in_maps, core_ids=list(range(N_CORES)),
                               trace=trace, tmpdir=tmpdir)
    out = np.concatenate([r["out"] for r in res.results], axis=0)
    assert out.shape == (BS, D, T_OUT)
    return out.astype(np.float32), res


def kernel(x, group_idxs, W, bias):
    out, _ = _run(x, group_idxs, W, bias)
    return out


# revision 8
# speedup vs baseline: 1.5657x; 1.2556x over previous
"""Grouped-index Conv1D (moe_routing) on 8 TRN2 NeuronCores.

Math:  out[b,d,t] = sum_c sum_k x[b,c,t+k] * W[gi[b,c],d,k] + count0[b]*bias[d]

v2 device algorithm (per core, 2 batches, data-parallel over batch):
  1. one-hot M[c,g] = (gi[b,c]==g) built on-chip (gpsimd iota + DVE is_equal)
  2. S[g,t] = sum_c M[c,g]*x[c,t]       (PE one-hot matmul, f32r via bitcast,
                                         written into swin block 0)
  3. swin[16r+g, t] = S[g, t+r]         (SBUF->SBUF DMA replication for
                                         r=1..R-1; no DRAM bounce)
  4. out[d,t] = sum_j Wj^T swin[...]    (ceil(7/R) PSUM-accumulated matmuls
                                         per 512-chunk)
  5. bias: count0 via DVE is_equal+reduce on a [1,512] gi row; one tiny
     matmul broadcasts count0*bias to [64,2]; fused add on DVE/ACT during
     PSUM->SBUF evacuation.
DMA queues: x split across qSP/qAct, control+weights on qPool (SWDGE),
replication on qSP, outputs on qAct.  (v1 used one queue: 59 serialized
DMAs ~ 35us; v2 has ~20 spread over 3 queues.)
"""

import sys
import numpy as np

sys.path.insert(0, "/opt/trn_rl_repo")

BS, CH, T = 16, 256, 2048
G, D, K = 16, 64, 7
T_OUT = T - K + 1  # 2042
N_CORES = 8
BPC = BS // N_CORES  # batches per core = 2

# default config: "v2/<R>" with R = partition replicas for the conv
# (R=7: 1 conv matmul/chunk, 6 repl DMAs/batch; R=4: 2 mm, 3 DMAs)
MM_DTYPE = "v2/4"

_COMPILED = {}

NCHUNK = 4  # T / 512


def _build_v2(cfg: str):
    from concourse import bacc, tile
    import concourse.mybir as mybir

    parts = cfg.split("/")
    R = int(parts[1]) if len(parts) > 1 else 4
    assert 1 <= R <= 7
    f32 = mybir.dt.float32
    f32r = mybir.dt.float32r
    # v2: f32r (raw f32 bits, 2e-2 budget); v3: bf16 (half the HBM/SBUF/DVE
    # bytes for x, swin, weights)
    mdt = mybir.dt.bfloat16 if parts[0] == "v3" else f32r
    eq = mybir.AluOpType.is_equal
    add = mybir.AluOpType.add
    NMM = (K + R - 1) // R  # conv matmuls per chunk
    NP = 16 * R             # swin partitions

    # block b of swin is read by conv matmul j (with blocks 0..len_j-1,
    # column window up to 1536 + j*R + 505); width needed per block:
    blk_w = [0] * R
    for j in range(NMM):
        jR = j * R
        lenj = min(R, K - jR)
        maxcol = 1536 + jR + (T_OUT - 1536) - 1  # 2041 + jR
        for b in range(lenj):
            blk_w[b] = max(blk_w[b], maxcol + 1)
    for b in range(1, R):
        assert blk_w[b] + b <= T, (b, blk_w[b])

    nc = bacc.Bacc("TRN2", target_bir_lowering=False, debug=False,
                   num_devices=N_CORES)
    x_ext = nc.dram_tensor("x", [BPC, CH, T], mdt, kind="ExternalInput").ap()
    git_ext = nc.dram_tensor("git", [128, 2 * BPC], f32, kind="ExternalInput").ap()
    gir_ext = nc.dram_tensor("gir", [1, CH * BPC], f32, kind="ExternalInput").ap()
    ws_ext = nc.dram_tensor("ws", [K * G, D], mdt, kind="ExternalInput").ap()
    b_ext = nc.dram_tensor("bias", [1, D], f32, kind="ExternalInput").ap()
    out_ext = nc.dram_tensor("out", [BPC, D, T_OUT], f32, kind="ExternalOutput").ap()

    with tile.TileContext(nc) as tc:
        with (
            tc.tile_pool(name="const", bufs=1) as cpool,
            tc.tile_pool(name="work", bufs=2) as wpool,
            tc.tile_pool(name="ps_pool", bufs=2, space="PSUM") as ppool,
            tc.tile_pool(name="psmall", bufs=1, space="PSUM") as spool,
            tc.tile_pool(name="po_pool", bufs=4, space="PSUM") as opool,
        ):
            # --- on-chip constants (Pool engine + SWDGE queue) ---
            iota_f = cpool.tile([128, G], f32, name="iota_f")
            nc.gpsimd.iota(iota_f[:], [[1, G]], channel_multiplier=0,
                           allow_small_or_imprecise_dtypes=True)
            # one stationary tile per conv matmul (matmul requires equal
            # base partitions for stationary and moving operands)
            wsj_all = []
            for j in range(NMM):
                jR = j * R
                lenj = min(R, K - jR)
                wsj = cpool.tile([G * lenj, D], mdt, name=f"wsj{j}")
                nc.gpsimd.dma_start(wsj[:], ws_ext[G * jR:G * (jR + lenj), :])
                wsj_all.append(wsj)
            brow = cpool.tile([1, D], f32, name="brow")
            nc.gpsimd.dma_start(brow[:], b_ext[:])
            gir_sb = cpool.tile([1, CH * BPC], f32, name="gir_sb")
            nc.gpsimd.dma_start(gir_sb[:], gir_ext[:])

            # --- qSP: gi (tiny, feeds one-hot) then batch-1 x tiles ---
            git_sb = cpool.tile([128, 2 * BPC], f32, name="git_sb")
            nc.sync.dma_start(git_sb[:], git_ext[:])
            xps = [[[None, None], [None, None]] for _ in range(BPC)]
            for p in range(2):
                for h in range(2):
                    t_ = wpool.tile([128, 1024], mdt, name=f"xp1{h}{p}",
                                    tag="xp", bufs=8)
                    nc.sync.dma_start(
                        t_[:], x_ext[1, 128 * h:128 * (h + 1),
                                     1024 * p:1024 * (p + 1)])
                    xps[1][h][p] = t_
            # --- qAct: batch-0 x tiles ---
            for p in range(2):
                for h in range(2):
                    t_ = wpool.tile([128, 1024], mdt, name=f"xp0{h}{p}",
                                    tag="xp", bufs=8)
                    nc.scalar.dma_start(
                        t_[:], x_ext[0, 128 * h:128 * (h + 1),
                                     1024 * p:1024 * (p + 1)])
                    xps[0][h][p] = t_

            # --- DVE: one-hot + count0 ---
            ms_all = []
            for b in range(BPC):
                ms = []
                for h in range(2):
                    m_t = wpool.tile([128, G], mdt, name=f"m{b}{h}",
                                     tag="m", bufs=4)
                    nc.vector.tensor_scalar(
                        out=m_t[:], in0=iota_f[:],
                        scalar1=git_sb[:, 2 * b + h:2 * b + h + 1],
                        scalar2=None, op0=eq)
                    ms.append(m_t)
                ms_all.append(ms)
            eq0 = wpool.tile([1, CH * BPC], f32, name="eq0", tag="eq0", bufs=1)
            nc.vector.tensor_scalar(out=eq0[:], in0=gir_sb[:], scalar1=0.0,
                                    scalar2=None, op0=eq)
            cnt2 = wpool.tile([1, BPC], f32, name="cnt2", tag="cnt2", bufs=1)
            for b in range(BPC):
                nc.vector.tensor_reduce(
                    out=cnt2[:, b:b + 1], in_=eq0[:, CH * b:CH * (b + 1)],
                    axis=mybir.AxisListType.X, op=add)

            # --- PE: S = M^T @ x, evacuated into swin block 0 (DVE);
            #     then SBUF->SBUF replication DMAs (qSP) ---
            swin_all = []
            for b in range(BPC):
                swin = wpool.tile([NP, T], mdt, name=f"swin{b}", tag="swin")
                for c in range(NCHUNK):
                    off = 512 * c
                    p, o = off // 1024, off % 1024
                    ps = ppool.tile([G, 512], f32, name=f"ps{b}{c}", tag="ps")
                    nc.tensor.matmul(ps[:], ms_all[b][0][:],
                                     xps[b][0][p][:, o:o + 512],
                                     start=True, stop=False)
                    nc.tensor.matmul(ps[:], ms_all[b][1][:],
                                     xps[b][1][p][:, o:o + 512],
                                     start=False, stop=True)
                    nc.vector.tensor_copy(swin[0:G, off:off + 512], ps[:])
                for r in range(1, R):
                    w = blk_w[r]
                    nc.sync.dma_start(swin[G * r:G * (r + 1), 0:w],
                                      swin[0:G, r:r + w])
                swin_all.append(swin)

            # --- PE: count0*bias broadcast to [64, BPC] ---
            pb = spool.tile([D, BPC], f32, name="pb", tag="pb")
            nc.tensor.matmul(pb[:], brow[:], cnt2[:],
                             start=True, stop=True)
            bcnt = wpool.tile([D, BPC], f32, name="bcnt", tag="bcnt", bufs=1)
            nc.vector.tensor_copy(bcnt[:], pb[:])

            # --- conv + fused bias-add + store ---
            for b in range(BPC):
                osb = wpool.tile([D, T_OUT], f32, name=f"osb{b}", tag="osb")
                for c in range(NCHUNK):
                    c0 = 512 * c
                    L = min(512, T_OUT - c0)
                    po = opool.tile([D, 512], f32, name=f"po{b}{c}", tag="po")
                    for j in range(NMM):
                        jR = j * R
                        lenj = min(R, K - jR)
                        nc.tensor.matmul(
                            po[:, :L],
                            wsj_all[j][:],
                            swin_all[b][0:G * lenj,
                                        c0 + jR:c0 + jR + L],
                            start=(j == 0), stop=(j == NMM - 1))
                    # fused bias add during evacuation; alternate DVE/ACT
                    if c % 2 == 0:
                        nc.vector.tensor_scalar(
                            out=osb[:, c0:c0 + L], in0=po[:, :L],
                            scalar1=bcnt[:, b:b + 1], scalar2=None, op0=add)
                    else:
                        nc.scalar.activation(
                            osb[:, c0:c0 + L], po[:, :L],
                            mybir.ActivationFunctionType.Identity,
                            bias=bcnt[:, b:b + 1])
                    if c == 1:
                        nc.scalar.dma_start(out_ext[b, :, 0:1024],
                                            osb[:, 0:1024])
                    elif c == NCHUNK - 1:
                        nc.scalar.dma_start(out_ext[b, :, 1024:T_OUT],
                                            osb[:, 1024:T_OUT])

    nc.compile()
    return nc


# ----------------------------------------------------------------------
# v1 baseline (kept for A/B comparison): config "<s_dtype>/<conv_mode>/<conv_dtype>"
def _build_v1(cfg: str):
    from concourse import bacc, tile
    import concourse.mybir as mybir

    s_dt, conv_mode, conv_dt = (cfg.split("/") + ["kaccum", "f32"])[:3] \
        if "/" in cfg else (cfg, "kaccum", cfg)
    f32 = mybir.dt.float32
    f32r = mybir.dt.float32r
    eq = mybir.AluOpType.is_equal
    mmdt = f32r if s_dt == "f32r" else f32
    use_r = s_dt == "f32r"
    cdt = f32r if conv_dt == "f32r" else f32

    nc = bacc.Bacc("TRN2", target_bir_lowering=False, debug=False,
                   num_devices=N_CORES)
    x_ext = nc.dram_tensor("x", [BPC, CH, T], f32, kind="ExternalInput").ap()
    gi_ext = nc.dram_tensor("gi", [BPC, 2, 128, 1], f32, kind="ExternalInput").ap()
    wt_ext = nc.dram_tensor("wt", [G, K * D], f32, kind="ExternalInput").ap()
    ws_ext = nc.dram_tensor("ws", [K * G, D], f32, kind="ExternalInput").ap()
    b_ext = nc.dram_tensor("bias", [1, D], f32, kind="ExternalInput").ap()
    io_ext = nc.dram_tensor("iota", [128, G], f32, kind="ExternalInput").ap()
    out_ext = nc.dram_tensor("out", [BPC, D, T_OUT], f32, kind="ExternalOutput").ap()
    swin = conv_mode == "swin"
    if swin:
        s_dram = nc.dram_tensor("s_dram", [BPC, G, T], cdt).ap()

    with tile.TileContext(nc) as tc:
        with (
            tc.tile_pool(name="const", bufs=1) as cpool,
            tc.tile_pool(name="work", bufs=2) as wpool,
            tc.tile_pool(name="ps_pool", bufs=2, space="PSUM") as ppool,
            tc.tile_pool(name="psmall", bufs=1, space="PSUM") as spool,
            tc.tile_pool(name="po_pool", bufs=4, space="PSUM") as opool,
        ):
            iota_f = cpool.tile([128, G], f32, name="iota_f")
            nc.sync.dma_start(iota_f[:], io_ext[:])
            if swin:
                ws_sb = cpool.tile([K * G, D], f32, name="ws_sb")
                nc.sync.dma_start(ws_sb[:], ws_ext[:])
                if cdt == f32r:
                    ws_r = cpool.tile([K * G, D], f32r, name="ws_r")
                    nc.vector.tensor_copy(ws_r[:], ws_sb[:])
                else:
                    ws_r = ws_sb
            else:
                wt_sb = cpool.tile([G, K * D], f32, name="wt_sb")
                nc.sync.dma_start(wt_sb[:], wt_ext[:])
                if cdt == f32r:
                    wt_r = cpool.tile([G, K * D], f32r, name="wt_r")
                    nc.vector.tensor_copy(wt_r[:], wt_sb[:])
                else:
                    wt_r = wt_sb
            brow = cpool.tile([1, D], f32, name="brow")
            nc.sync.dma_start(brow[:], b_ext[:])
            ones_col = cpool.tile([128, 1], f32, name="ones_col")
            nc.vector.memset(ones_col[:], 1.0)

            gi_all = []
            for b in range(BPC):
                gis = []
                for h in range(2):
                    gi_t = wpool.tile([128, 1], f32, name=f"gi{b}{h}",
                                      tag="gi", bufs=4)
                    nc.sync.dma_start(gi_t[:], gi_ext[b, h])
                    gis.append(gi_t)
                gi_all.append(gis)

            xps = [[[None, None], [None, None]] for _ in range(BPC)]
            for b in range(BPC):
                for h in range(2):
                    for p in range(2):
                        t_ = wpool.tile([128, 1024], f32, name=f"xp{b}{h}{p}",
                                        tag="xp", bufs=8)
                        nc.sync.dma_start(
                            t_[:],
                            x_ext[b, 128 * h:128 * (h + 1),
                                  1024 * p:1024 * (p + 1)])
                        if use_r:
                            xr = wpool.tile([128, 1024], f32r,
                                            name=f"xr{b}{h}{p}", tag="xr",
                                            bufs=8)
                            if (h + p) % 2 == 0:
                                nc.vector.tensor_copy(xr[:], t_[:])
                            else:
                                nc.scalar.activation(
                                    xr[:], t_[:],
                                    mybir.ActivationFunctionType.Copy)
                            t_ = xr
                        xps[b][h][p] = t_

            ms_all, bcnt_all = [], []
            for b in range(BPC):
                ms, ms_f = [], []
                for h in range(2):
                    gi_t = gi_all[b][h]
                    mf_t = wpool.tile([128, G], f32, name=f"mf{b}{h}",
                                      tag="mf", bufs=4)
                    nc.vector.tensor_scalar(out=mf_t[:], in0=iota_f[:],
                                            scalar1=gi_t[:, 0:1], scalar2=None,
                                            op0=eq)
                    ms_f.append(mf_t)
                    if use_r:
                        m_t = wpool.tile([128, G], mdt, name=f"m{b}{h}",
                                         tag="m", bufs=4)
                        nc.vector.tensor_copy(m_t[:], mf_t[:])
                    else:
                        m_t = mf_t
                    ms.append(m_t)
                ms_all.append(ms)

                pcnt = spool.tile([G, 1], f32, name=f"pcnt{b}", tag="pcnt")
                nc.tensor.matmul(pcnt[:], ms_f[0][:], ones_col[:],
                                 start=True, stop=False)
                nc.tensor.matmul(pcnt[:], ms_f[1][:], ones_col[:],
                                 start=False, stop=True)
                cnt_sb = wpool.tile([G, 1], f32, name=f"cnt{b}", tag="cnt")
                nc.vector.tensor_copy(cnt_sb[:], pcnt[:])
                pbc = spool.tile([D, 1], f32, name=f"pbc{b}", tag="pbc")
                nc.tensor.matmul(pbc[:], brow[:], cnt_sb[0:1, 0:1],
                                 start=True, stop=True)
                bcnt = wpool.tile([D, 1], f32, name=f"bcnt{b}", tag="bcnt")
                nc.vector.tensor_copy(bcnt[:], pbc[:])
                bcnt_all.append(bcnt)

            s_all = []
            for b in range(BPC):
                xp = xps[b]
                s_sb = wpool.tile([G, T], cdt, name=f"s{b}", tag="s")
                for c in range(4):
                    ps = ppool.tile([G, 512], f32, name=f"ps{b}{c}", tag="ps")
                    off = 512 * c
                    p, o = off // 1024, off % 1024
                    nc.tensor.matmul(ps[:], ms_all[b][0][:],
                                     xp[0][p][:, o:o + 512],
                                     start=True, stop=False)
                    nc.tensor.matmul(ps[:], ms_all[b][1][:],
                                     xp[1][p][:, o:o + 512],
                                     start=False, stop=True)
                    nc.vector.tensor_copy(s_sb[:, off:off + 512], ps[:])
                    if swin:
                        nc.sync.dma_start(s_dram[b, :, off:off + 512],
                                          s_sb[:, off:off + 512])
                s_all.append(s_sb)

            swin_all = []
            if swin:
                for b in range(BPC):
                    swin_sb = wpool.tile([K * G, T_OUT], cdt,
                                         name=f"swin{b}", tag="swin")
                    half = 1024
                    for lo, hi in ((0, half), (half, T_OUT)):
                        for k in range(K):
                            nc.sync.dma_start(
                                swin_sb[G * k:G * (k + 1), lo:hi],
                                s_dram[b, :, k + lo:k + hi])
                    swin_all.append(swin_sb)

            for b in range(BPC):
                for c in range(4):
                    c0 = 512 * c
                    L = min(512, T_OUT - c0)
                    po = opool.tile([D, 512], f32, name=f"po{b}{c}", tag="po")
                    if swin:
                        nc.tensor.matmul(po[:, :L], ws_r[:],
                                         swin_all[b][:, c0:c0 + L],
                                         start=True, stop=True)
                    else:
                        for k in range(K):
                            nc.tensor.matmul(po[:, :L],
                                             wt_r[:, D * k:D * (k + 1)],
                                             s_all[b][:, c0 + k:c0 + k + L],
                                             start=(k == 0), stop=(k == K - 1))
                    osb = wpool.tile([D, 512], f32, name=f"osb{b}{c}",
                                     tag="osb", bufs=4)
                    nc.scalar.activation(osb[:, :L], po[:, :L],
                                         mybir.ActivationFunctionType.Identity,
                                         bias=bcnt_all[b][:, 0:1])
                    nc.sync.dma_start(out_ext[b, :, c0:c0 + L], osb[:, :L])

    nc.compile()
    return nc


def _build(cfg: str):
    if cfg.startswith("v2") or cfg.startswith("v3"):
        return _build_v2(cfg)
    return _build_v1(cfg)


def _get_nc(mm_dtype: str):
    if mm_dtype not in _COMPILED:
        _COMPILED[mm_dtype] = _build(mm_dtype)
    return _COMPILED[mm_dtype]


def _run(x, group_idxs, W, bias, mm_dtype=None, trace=False, tmpdir=None):
    from concourse.bass_utils import run_bass_kernel_spmd

    cfg = mm_dtype or MM_DTYPE
    x = np.ascontiguousarray(np.asarray(x, dtype=np.float32))
    gi_f = np.asarray(group_idxs).astype(np.float32)  # [BS, CH]
    W = np.asarray(W, dtype=np.float32)
    bias = np.asarray(bias, dtype=np.float32)
    # ws[k*16+g, d] = W[g,d,k]
    ws = np.ascontiguousarray(W.transpose(2, 0, 1).reshape(K * G, D))
    brow = np.ascontiguousarray(bias.reshape(1, D))
    nc = _get_nc(cfg)

    in_maps = []
    if cfg.startswith("v2") or cfg.startswith("v3"):
        if cfg.startswith("v3"):
            import ml_dtypes
            x_send = x.astype(ml_dtypes.bfloat16)
            ws_send = ws.astype(ml_dtypes.bfloat16)
        else:
            x_send, ws_send = x, ws
        for i in range(N_CORES):
            sl = slice(i * BPC, (i + 1) * BPC)
            gi_c = gi_f[sl]  # [BPC, CH]
            # git[c, 2b+h] = gi[b, h*128+c]
            git = np.ascontiguousarray(
                gi_c.reshape(BPC * 2, 128).T)  # [128, 2*BPC]
            gir = np.ascontiguousarray(gi_c.reshape(1, BPC * CH))
            in_maps.append({
                "x": np.ascontiguousarray(x_send[sl]),
                "git": git,
                "gir": gir,
                "ws": ws_send,
                "bias": brow,
            })
    else:
        gi = gi_f.reshape(BS, 2, 128, 1)
        wt = np.ascontiguousarray(W.transpose(0, 2, 1).reshape(G, K * D))
        iota = np.ascontiguousarray(
            np.broadcast_to(np.arange(G, dtype=np.float32), (128, G)))
        for i in range(N_CORES):
            sl = slice(i * BPC, (i + 1) * BPC)
            in_maps.append({
                "x": np.ascontiguousarray(x[sl]),
                "gi": np.ascontiguousarray(gi[sl]),
                "wt": wt,
                "ws": ws,
                "bias": brow,
                "iota": iota,
            })
    res = run_bass_kernel_spmd(nc, in_maps, core_ids=list(range(N_CORES)),
                               trace=trace, tmpdir=tmpdir)
    out = np.concatenate([r["out"] for r in res.results], axis=0)
    assert out.shape == (BS, D, T_OUT)
    return out.astype(np.float32), res


def kernel(x, group_idxs, W, bias):
    out, _ = _run(x, group_idxs, W, bias)
    return out


# revision 10
# speedup vs baseline: 1.6033x; 1.0240x over previous
"""Grouped-index Conv1D (moe_routing) on 8 TRN2 NeuronCores.

Math:  out[b,d,t] = sum_c sum_k x[b,c,t+k] * W[gi[b,c],d,k] + count0[b]*bias[d]

v2 device algorithm (per core, 2 batches, data-parallel over batch):
  1. one-hot M[c,g] = (gi[b,c]==g) built on-chip (gpsimd iota + DVE is_equal)
  2. S[g,t] = sum_c M[c,g]*x[c,t]       (PE one-hot matmul, f32r via bitcast,
                                         written into swin block 0)
  3. swin[16r+g, t] = S[g, t+r]         (SBUF->SBUF DMA replication for
                                         r=1..R-1; no DRAM bounce)
  4. out[d,t] = sum_j Wj^T swin[...]    (ceil(7/R) PSUM-accumulated matmuls
                                         per 512-chunk)
  5. bias: count0 via DVE is_equal+reduce on a [1,512] gi row; one tiny
     matmul broadcasts count0*bias to [64,2]; fused add on DVE/ACT during
     PSUM->SBUF evacuation.
DMA queues: x split across qSP/qAct, control+weights on qPool (SWDGE),
replication on qSP, outputs on qAct.  (v1 used one queue: 59 serialized
DMAs ~ 35us; v2 has ~20 spread over 3 queues.)
"""

import sys
import numpy as np

sys.path.insert(0, "/opt/trn_rl_repo")

BS, CH, T = 16, 256, 2048
G, D, K = 16, 64, 7
T_OUT = T - K + 1  # 2042
N_CORES = 8
BPC = BS // N_CORES  # batches per core = 2

# default config: "v2/<R>" with R = partition replicas for the conv
# (R=7: 1 conv matmul/chunk, 6 repl DMAs/batch; R=4: 2 mm, 3 DMAs)
MM_DTYPE = "v2/4"

_COMPILED = {}

NCHUNK = 4  # T / 512


def _build_v2(cfg: str):
    from concourse import bacc, tile
    import concourse.mybir as mybir

    parts = cfg.split("/")
    R = int(parts[1]) if len(parts) > 1 else 4
    assert 1 <= R <= 7
    f32 = mybir.dt.float32
    f32r = mybir.dt.float32r
    # v2: f32r (raw f32 bits, 2e-2 budget); v3: bf16 (half the HBM/SBUF/DVE
    # bytes for x, swin, weights)
    mdt = mybir.dt.bfloat16 if parts[0] == "v3" else f32r
    eq = mybir.AluOpType.is_equal
    add = mybir.AluOpType.add
    NMM = (K + R - 1) // R  # conv matmuls per chunk
    NP = 16 * R             # swin partitions

    # block b of swin is read by conv matmul j (with blocks 0..len_j-1,
    # column window up to 1536 + j*R + 505); width needed per block:
    blk_w = [0] * R
    for j in range(NMM):
        jR = j * R
        lenj = min(R, K - jR)
        maxcol = 1536 + jR + (T_OUT - 1536) - 1  # 2041 + jR
        for b in range(lenj):
            blk_w[b] = max(blk_w[b], maxcol + 1)
    for b in range(1, R):
        assert blk_w[b] + b <= T, (b, blk_w[b])

    nc = bacc.Bacc("TRN2", target_bir_lowering=False, debug=False,
                   num_devices=N_CORES)
    x_ext = nc.dram_tensor("x", [BPC, CH, T], mdt, kind="ExternalInput").ap()
    git_ext = nc.dram_tensor("git", [128, 2 * BPC], f32, kind="ExternalInput").ap()
    gir_ext = nc.dram_tensor("gir", [1, CH * BPC], f32, kind="ExternalInput").ap()
    ws_ext = nc.dram_tensor("ws", [K * G, D], mdt, kind="ExternalInput").ap()
    b_ext = nc.dram_tensor("bias", [1, D], f32, kind="ExternalInput").ap()
    out_ext = nc.dram_tensor("out", [BPC, D, T_OUT], f32, kind="ExternalOutput").ap()

    with tile.TileContext(nc) as tc:
        with (
            tc.tile_pool(name="const", bufs=1) as cpool,
            tc.tile_pool(name="work", bufs=2) as wpool,
            tc.tile_pool(name="ps_pool", bufs=2, space="PSUM") as ppool,
            tc.tile_pool(name="psmall", bufs=1, space="PSUM") as spool,
            tc.tile_pool(name="po_pool", bufs=4, space="PSUM") as opool,
        ):
            # --- on-chip constants (Pool engine + SWDGE queue) ---
            iota_f = cpool.tile([128, G], f32, name="iota_f")
            nc.gpsimd.iota(iota_f[:], [[1, G]], channel_multiplier=0,
                           allow_small_or_imprecise_dtypes=True)
            # --- qSP: gi rows (tiny, feed one-hot + count) then batch-1 x ---
            # (SWDGE/gpsimd DMAs have ~5us end-to-end latency; keep every
            # DMA on the two HWDGE queues)
            git_sb = cpool.tile([128, 2 * BPC], f32, name="git_sb")
            nc.sync.dma_start(git_sb[:], git_ext[:])
            gir_sb = cpool.tile([1, CH * BPC], f32, name="gir_sb")
            nc.sync.dma_start(gir_sb[:], gir_ext[:])
            xps = [[[None, None], [None, None]] for _ in range(BPC)]
            for p in range(2):
                for h in range(2):
                    t_ = wpool.tile([128, 1024], mdt, name=f"xp1{h}{p}",
                                    tag="xp", bufs=8)
                    nc.sync.dma_start(
                        t_[:], x_ext[1, 128 * h:128 * (h + 1),
                                     1024 * p:1024 * (p + 1)])
                    xps[1][h][p] = t_
            # --- qAct: batch-0 x tiles ---
            for p in range(2):
                for h in range(2):
                    t_ = wpool.tile([128, 1024], mdt, name=f"xp0{h}{p}",
                                    tag="xp", bufs=8)
                    nc.scalar.dma_start(
                        t_[:], x_ext[0, 128 * h:128 * (h + 1),
                                     1024 * p:1024 * (p + 1)])
                    xps[0][h][p] = t_
            # conv stationaries + bias row: tiny, needed only by conv (~mid
            # kernel); one tile per matmul (matmul requires equal base
            # partitions for stationary and moving operands)
            wsj_all = []
            for j in range(NMM):
                jR = j * R
                lenj = min(R, K - jR)
                wsj = cpool.tile([G * lenj, D], mdt, name=f"wsj{j}")
                nc.scalar.dma_start(wsj[:], ws_ext[G * jR:G * (jR + lenj), :])
                wsj_all.append(wsj)
            brow = cpool.tile([1, D], f32, name="brow")
            nc.scalar.dma_start(brow[:], b_ext[:])

            # --- DVE: one-hot + count0 ---
            ms_all = []
            for b in range(BPC):
                ms = []
                for h in range(2):
                    m_t = wpool.tile([128, G], mdt, name=f"m{b}{h}",
                                     tag="m", bufs=4)
                    nc.vector.tensor_scalar(
                        out=m_t[:], in0=iota_f[:],
                        scalar1=git_sb[:, 2 * b + h:2 * b + h + 1],
                        scalar2=None, op0=eq)
                    ms.append(m_t)
                ms_all.append(ms)
            # --- PE: S = M^T @ x, evacuated into swin block 0 (DVE);
            #     then SBUF->SBUF replication DMAs (qSP) ---
            swin_all = []
            for b in range(BPC):
                swin = wpool.tile([NP, T], mdt, name=f"swin{b}", tag="swin")
                for c in range(NCHUNK):
                    off = 512 * c
                    p, o = off // 1024, off % 1024
                    ps = ppool.tile([G, 512], f32, name=f"ps{b}{c}", tag="ps")
                    nc.tensor.matmul(ps[:], ms_all[b][0][:],
                                     xps[b][0][p][:, o:o + 512],
                                     start=True, stop=False)
                    nc.tensor.matmul(ps[:], ms_all[b][1][:],
                                     xps[b][1][p][:, o:o + 512],
                                     start=False, stop=True)
                    nc.vector.tensor_copy(swin[0:G, off:off + 512], ps[:])
                eng = nc.scalar if b == 0 else nc.sync
                for r in range(1, R):
                    w = blk_w[r]
                    eng.dma_start(swin[G * r:G * (r + 1), 0:w],
                                  swin[0:G, r:r + w])
                swin_all.append(swin)

            # --- count0 (DVE, after S evacs so S never waits on it)
            #     then count0*bias broadcast to [64, BPC] on PE ---
            eq0 = wpool.tile([1, CH * BPC], f32, name="eq0", tag="eq0", bufs=1)
            nc.vector.tensor_scalar(out=eq0[:], in0=gir_sb[:], scalar1=0.0,
                                    scalar2=None, op0=eq)
            cnt2 = wpool.tile([1, BPC], f32, name="cnt2", tag="cnt2", bufs=1)
            for b in range(BPC):
                nc.vector.tensor_reduce(
                    out=cnt2[:, b:b + 1], in_=eq0[:, CH * b:CH * (b + 1)],
                    axis=mybir.AxisListType.X, op=add)
            pb = spool.tile([D, BPC], f32, name="pb", tag="pb")
            nc.tensor.matmul(pb[:], brow[:], cnt2[:],
                             start=True, stop=True)
            bcnt = wpool.tile([D, BPC], f32, name="bcnt", tag="bcnt", bufs=1)
            nc.vector.tensor_copy(bcnt[:], pb[:])

            # --- conv + fused bias-add + store ---
            for b in range(BPC):
                osb = wpool.tile([D, T_OUT], f32, name=f"osb{b}", tag="osb")
                for c in range(NCHUNK):
                    c0 = 512 * c
                    L = min(512, T_OUT - c0)
                    po = opool.tile([D, 512], f32, name=f"po{b}{c}", tag="po")
                    for j in range(NMM):
                        jR = j * R
                        lenj = min(R, K - jR)
                        nc.tensor.matmul(
                            po[:, :L],
                            wsj_all[j][:],
                            swin_all[b][0:G * lenj,
                                        c0 + jR:c0 + jR + L],
                            start=(j == 0), stop=(j == NMM - 1))
                    # fused bias add during evacuation; alternate DVE/ACT
                    if c % 2 == 0:
                        nc.vector.tensor_scalar(
                            out=osb[:, c0:c0 + L], in0=po[:, :L],
                            scalar1=bcnt[:, b:b + 1], scalar2=None, op0=add)
                    else:
                        nc.scalar.activation(
                            osb[:, c0:c0 + L], po[:, :L],
                            mybir.ActivationFunctionType.Identity,
                            bias=bcnt[:, b:b + 1])
                    if c == 1:
                        nc.scalar.dma_start(out_ext[b, :, 0:1024],
                                            osb[:, 0:1024])
                    elif c == NCHUNK - 1:
                        nc.scalar.dma_start(out_ext[b, :, 1024:T_OUT],
                                            osb[:, 1024:T_OUT])

    nc.compile()
    return nc


# ----------------------------------------------------------------------
# v1 baseline (kept for A/B comparison): config "<s_dtype>/<conv_mode>/<conv_dtype>"
def _build_v1(cfg: str):
    from concourse import bacc, tile
    import concourse.mybir as mybir

    s_dt, conv_mode, conv_dt = (cfg.split("/") + ["kaccum", "f32"])[:3] \
        if "/" in cfg else (cfg, "kaccum", cfg)
    f32 = mybir.dt.float32
    f32r = mybir.dt.float32r
    eq = mybir.AluOpType.is_equal
    mmdt = f32r if s_dt == "f32r" else f32
    use_r = s_dt == "f32r"
    cdt = f32r if conv_dt == "f32r" else f32

    nc = bacc.Bacc("TRN2", target_bir_lowering=False, debug=False,
                   num_devices=N_CORES)
    x_ext = nc.dram_tensor("x", [BPC, CH, T], f32, kind="ExternalInput").ap()
    gi_ext = nc.dram_tensor("gi", [BPC, 2, 128, 1], f32, kind="ExternalInput").ap()
    wt_ext = nc.dram_tensor("wt", [G, K * D], f32, kind="ExternalInput").ap()
    ws_ext = nc.dram_tensor("ws", [K * G, D], f32, kind="ExternalInput").ap()
    b_ext = nc.dram_tensor("bias", [1, D], f32, kind="ExternalInput").ap()
    io_ext = nc.dram_tensor("iota", [128, G], f32, kind="ExternalInput").ap()
    out_ext = nc.dram_tensor("out", [BPC, D, T_OUT], f32, kind="ExternalOutput").ap()
    swin = conv_mode == "swin"
    if swin:
        s_dram = nc.dram_tensor("s_dram", [BPC, G, T], cdt).ap()

    with tile.TileContext(nc) as tc:
        with (
            tc.tile_pool(name="const", bufs=1) as cpool,
            tc.tile_pool(name="work", bufs=2) as wpool,
            tc.tile_pool(name="ps_pool", bufs=2, space="PSUM") as ppool,
            tc.tile_pool(name="psmall", bufs=1, space="PSUM") as spool,
            tc.tile_pool(name="po_pool", bufs=4, space="PSUM") as opool,
        ):
            iota_f = cpool.tile([128, G], f32, name="iota_f")
            nc.sync.dma_start(iota_f[:], io_ext[:])
            if swin:
                ws_sb = cpool.tile([K * G, D], f32, name="ws_sb")
                nc.sync.dma_start(ws_sb[:], ws_ext[:])
                if cdt == f32r:
                    ws_r = cpool.tile([K * G, D], f32r, name="ws_r")
                    nc.vector.tensor_copy(ws_r[:], ws_sb[:])
                else:
                    ws_r = ws_sb
            else:
                wt_sb = cpool.tile([G, K * D], f32, name="wt_sb")
                nc.sync.dma_start(wt_sb[:], wt_ext[:])
                if cdt == f32r:
                    wt_r = cpool.tile([G, K * D], f32r, name="wt_r")
                    nc.vector.tensor_copy(wt_r[:], wt_sb[:])
                else:
                    wt_r = wt_sb
            brow = cpool.tile([1, D], f32, name="brow")
            nc.sync.dma_start(brow[:], b_ext[:])
            ones_col = cpool.tile([128, 1], f32, name="ones_col")
            nc.vector.memset(ones_col[:], 1.0)

            gi_all = []
            for b in range(BPC):
                gis = []
                for h in range(2):
                    gi_t = wpool.tile([128, 1], f32, name=f"gi{b}{h}",
                                      tag="gi", bufs=4)
                    nc.sync.dma_start(gi_t[:], gi_ext[b, h])
                    gis.append(gi_t)
                gi_all.append(gis)

            xps = [[[None, None], [None, None]] for _ in range(BPC)]
            for b in range(BPC):
                for h in range(2):
                    for p in range(2):
                        t_ = wpool.tile([128, 1024], f32, name=f"xp{b}{h}{p}",
                                        tag="xp", bufs=8)
                        nc.sync.dma_start(
                            t_[:],
                            x_ext[b, 128 * h:128 * (h + 1),
                                  1024 * p:1024 * (p + 1)])
                        if use_r:
                            xr = wpool.tile([128, 1024], f32r,
                                            name=f"xr{b}{h}{p}", tag="xr",
                                            bufs=8)
                            if (h + p) % 2 == 0:
                                nc.vector.tensor_copy(xr[:], t_[:])
                            else:
                                nc.scalar.activation(
                                    xr[:], t_[:],
                                    mybir.ActivationFunctionType.Copy)
                            t_ = xr
                        xps[b][h][p] = t_

            ms_all, bcnt_all = [], []
            for b in range(BPC):
                ms, ms_f = [], []
                for h in range(2):
                    gi_t = gi_all[b][h]
                    mf_t = wpool.tile([128, G], f32, name=f"mf{b}{h}",
                                      tag="mf", bufs=4)
                    nc.vector.tensor_scalar(out=mf_t[:], in0=iota_f[:],
                                            scalar1=gi_t[:, 0:1], scalar2=None,
                                            op0=eq)
                    ms_f.append(mf_t)
                    if use_r:
                        m_t = wpool.tile([128, G], mdt, name=f"m{b}{h}",
                                         tag="m", bufs=4)
                        nc.vector.tensor_copy(m_t[:], mf_t[:])
                    else:
                        m_t = mf_t
                    ms.append(m_t)
                ms_all.append(ms)

                pcnt = spool.tile([G, 1], f32, name=f"pcnt{b}", tag="pcnt")
                nc.tensor.matmul(pcnt[:], ms_f[0][:], ones_col[:],
                                 start=True, stop=False)
                nc.tensor.matmul(pcnt[:], ms_f[1][:], ones_col[:],
                                 start=False, stop=True)
                cnt_sb = wpool.tile([G, 1], f32, name=f"cnt{b}", tag="cnt")
                nc.vector.tensor_copy(cnt_sb[:], pcnt[:])
                pbc = spool.tile([D, 1], f32, name=f"pbc{b}", tag="pbc")
                nc.tensor.matmul(pbc[:], brow[:], cnt_sb[0:1, 0:1],
                                 start=True, stop=True)
                bcnt = wpool.tile([D, 1], f32, name=f"bcnt{b}", tag="bcnt")
                nc.vector.tensor_copy(bcnt[:], pbc[:])
                bcnt_all.append(bcnt)

            s_all = []
            for b in range(BPC):
                xp = xps[b]
                s_sb = wpool.tile([G, T], cdt, name=f"s{b}", tag="s")
                for c in range(4):
                    ps = ppool.tile([G, 512], f32, name=f"ps{b}{c}", tag="ps")
                    off = 512 * c
                    p, o = off // 1024, off % 1024
                    nc.tensor.matmul(ps[:], ms_all[b][0][:],
                                     xp[0][p][:, o:o + 512],
                                     start=True, stop=False)
                    nc.tensor.matmul(ps[:], ms_all[b][1][:],
                                     xp[1][p][:, o:o + 512],
                                     start=False, stop=True)
                    nc.vector.tensor_copy(s_sb[:, off:off + 512], ps[:])
                    if swin:
                        nc.sync.dma_start(s_dram[b, :, off:off + 512],
                                          s_sb[:, off:off + 512])
                s_all.append(s_sb)

            swin_all = []
            if swin:
                for b in range(BPC):
                    swin_sb = wpool.tile([K * G, T_OUT], cdt,
                                         name=f"swin{b}", tag="swin")
                    half = 1024
                    for lo, hi in ((0, half), (half, T_OUT)):
                        for k in range(K):
                            nc.sync.dma_start(
                                swin_sb[G * k:G * (k + 1), lo:hi],
                                s_dram[b, :, k + lo:k + hi])
                    swin_all.append(swin_sb)

            for b in range(BPC):
                for c in range(4):
                    c0 = 512 * c
                    L = min(512, T_OUT - c0)
                    po = opool.tile([D, 512], f32, name=f"po{b}{c}", tag="po")
                    if swin:
                        nc.tensor.matmul(po[:, :L], ws_r[:],
                                         swin_all[b][:, c0:c0 + L],
                                         start=True, stop=True)
                    else:
                        for k in range(K):
                            nc.tensor.matmul(po[:, :L],
                                             wt_r[:, D * k:D * (k + 1)],
                                             s_all[b][:, c0 + k:c0 + k + L],
                                             start=(k == 0), stop=(k == K - 1))
                    osb = wpool.tile([D, 512], f32, name=f"osb{b}{c}",
                                     tag="osb", bufs=4)
                    nc.scalar.activation(osb[:, :L], po[:, :L],
                                         mybir.ActivationFunctionType.Identity,
                                         bias=bcnt_all[b][:, 0:1])
                    nc.sync.dma_start(out_ext[b, :, c0:c0 + L], osb[:, :L])

    nc.compile()
    return nc


def _build(cfg: str):
    if cfg.startswith("v2") or cfg.startswith("v3"):
        return _build_v2(cfg)
    return _build_v1(cfg)


def _get_nc(mm_dtype: str):
    if mm_dtype not in _COMPILED:
        _COMPILED[mm_dtype] = _build(mm_dtype)
    return _COMPILED[mm_dtype]


def _run(x, group_idxs, W, bias, mm_dtype=None, trace=False, tmpdir=None):
    from concourse.bass_utils import run_bass_kernel_spmd

    cfg = mm_dtype or MM_DTYPE
    x = np.ascontiguousarray(np.asarray(x, dtype=np.float32))
    gi_f = np.asarray(group_idxs).astype(np.float32)  # [BS, CH]
    W = np.asarray(W, dtype=np.float32)
    bias = np.asarray(bias, dtype=np.float32)
    # ws[k*16+g, d] = W[g,d,k]
    ws = np.ascontiguousarray(W.transpose(2, 0, 1).reshape(K * G, D))
    brow = np.ascontiguousarray(bias.reshape(1, D))
    nc = _get_nc(cfg)

    in_maps = []
    if cfg.startswith("v2") or cfg.startswith("v3"):
        if cfg.startswith("v3"):
            import ml_dtypes
            x_send = x.astype(ml_dtypes.bfloat16)
            ws_send = ws.astype(ml_dtypes.bfloat16)
        else:
            x_send, ws_send = x, ws
        for i in range(N_CORES):
            sl = slice(i * BPC, (i + 1) * BPC)
            gi_c = gi_f[sl]  # [BPC, CH]
            # git[c, 2b+h] = gi[b, h*128+c]
            git = np.ascontiguousarray(
                gi_c.reshape(BPC * 2, 128).T)  # [128, 2*BPC]
            gir = np.ascontiguousarray(gi_c.reshape(1, BPC * CH))
            in_maps.append({
                "x": np.ascontiguousarray(x_send[sl]),
                "git": git,
                "gir": gir,
                "ws": ws_send,
                "bias": brow,
            })
    else:
        gi = gi_f.reshape(BS, 2, 128, 1)
        wt = np.ascontiguousarray(W.transpose(0, 2, 1).reshape(G, K * D))
        iota = np.ascontiguousarray(
            np.broadcast_to(np.arange(G, dtype=np.float32), (128, G)))
        for i in range(N_CORES):
            sl = slice(i * BPC, (i + 1) * BPC)
            in_maps.append({
                "x": np.ascontiguousarray(x[sl]),
                "gi": np.ascontiguousarray(gi[sl]),
                "wt": wt,
                "ws": ws,
                "bias": brow,
                "iota": iota,
            })
    res = run_bass_kernel_spmd(nc, in_maps, core_ids=list(range(N_CORES)),
                               trace=trace, tmpdir=tmpdir)
    out = np.concatenate([r["out"] for r in res.results], axis=0)
    assert out.shape == (BS, D, T_OUT)
    return out.astype(np.float32), res


def kernel(x, group_idxs, W, bias):
    out, _ = _run(x, group_idxs, W, bias)
    return out


# revision 11
# speedup vs baseline: 1.6559x; 1.0328x over previous
"""Grouped-index Conv1D (moe_routing) on 8 TRN2 NeuronCores.

Math:  out[b,d,t] = sum_c sum_k x[b,c,t+k] * W[gi[b,c],d,k] + count0[b]*bias[d]

v2 device algorithm (per core, 2 batches, data-parallel over batch):
  1. one-hot M[c,g] = (gi[b,c]==g) built on-chip (gpsimd iota + DVE is_equal)
  2. S[g,t] = sum_c M[c,g]*x[c,t]       (PE one-hot matmul, f32r via bitcast,
                                         written into swin block 0)
  3. swin[16r+g, t] = S[g, t+r]         (SBUF->SBUF DMA replication for
                                         r=1..R-1; no DRAM bounce)
  4. out[d,t] = sum_j Wj^T swin[...]    (ceil(7/R) PSUM-accumulated matmuls
                                         per 512-chunk)
  5. bias: count0 via DVE is_equal+reduce on a [1,512] gi row; one tiny
     matmul broadcasts count0*bias to [64,2]; fused add on DVE/ACT during
     PSUM->SBUF evacuation.
DMA queues: x split across qSP/qAct, control+weights on qPool (SWDGE),
replication on qSP, outputs on qAct.  (v1 used one queue: 59 serialized
DMAs ~ 35us; v2 has ~20 spread over 3 queues.)
"""

import sys
import numpy as np

sys.path.insert(0, "/opt/trn_rl_repo")

BS, CH, T = 16, 256, 2048
G, D, K = 16, 64, 7
T_OUT = T - K + 1  # 2042
N_CORES = 8
BPC = BS // N_CORES  # batches per core = 2

# default config: "v2/<R>" with R = partition replicas for the conv
# (R=7: 1 conv matmul/chunk, 6 repl DMAs/batch; R=4: 2 mm, 3 DMAs)
MM_DTYPE = "v2/4"

_COMPILED = {}

NCHUNK = 4  # T / 512


def _build_v2(cfg: str):
    from concourse import bacc, tile
    import concourse.mybir as mybir

    parts = cfg.split("/")
    R = int(parts[1]) if len(parts) > 1 else 4
    assert 1 <= R <= 7
    f32 = mybir.dt.float32
    f32r = mybir.dt.float32r
    # v2: f32r (raw f32 bits, 2e-2 budget); v3: bf16 (half the HBM/SBUF/DVE
    # bytes for x, swin, weights)
    mdt = mybir.dt.bfloat16 if parts[0] == "v3" else f32r
    eq = mybir.AluOpType.is_equal
    add = mybir.AluOpType.add
    NMM = (K + R - 1) // R  # conv matmuls per chunk
    NP = 16 * R             # swin partitions

    # block b of swin is read by conv matmul j (with blocks 0..len_j-1,
    # column window up to 1536 + j*R + 505); width needed per block:
    blk_w = [0] * R
    for j in range(NMM):
        jR = j * R
        lenj = min(R, K - jR)
        maxcol = 1536 + jR + (T_OUT - 1536) - 1  # 2041 + jR
        for b in range(lenj):
            blk_w[b] = max(blk_w[b], maxcol + 1)
    for b in range(1, R):
        assert blk_w[b] + b <= T, (b, blk_w[b])

    nc = bacc.Bacc("TRN2", target_bir_lowering=False, debug=False,
                   num_devices=N_CORES)
    x_ext = nc.dram_tensor("x", [BPC, CH, T], mdt, kind="ExternalInput").ap()
    git_ext = nc.dram_tensor("git", [128, 2 * BPC], f32, kind="ExternalInput").ap()
    gir_ext = nc.dram_tensor("gir", [1, CH * BPC], f32, kind="ExternalInput").ap()
    ws_ext = nc.dram_tensor("ws", [K * G, D], mdt, kind="ExternalInput").ap()
    b_ext = nc.dram_tensor("bias", [1, D], f32, kind="ExternalInput").ap()
    out_ext = nc.dram_tensor("out", [BPC, D, T_OUT], f32, kind="ExternalOutput").ap()

    with tile.TileContext(nc) as tc:
        with (
            tc.tile_pool(name="const", bufs=1) as cpool,
            tc.tile_pool(name="work", bufs=2) as wpool,
            tc.tile_pool(name="ps_pool", bufs=4, space="PSUM") as ppool,
            tc.tile_pool(name="psmall", bufs=1, space="PSUM") as spool,
            tc.tile_pool(name="po_pool", bufs=3, space="PSUM") as opool,
        ):
            # --- on-chip constants (Pool engine + SWDGE queue) ---
            iota_f = cpool.tile([128, G], f32, name="iota_f")
            nc.gpsimd.iota(iota_f[:], [[1, G]], channel_multiplier=0,
                           allow_small_or_imprecise_dtypes=True)
            # --- qSP: gi rows (tiny, feed one-hot + count) then batch-1 x ---
            # (SWDGE/gpsimd DMAs have ~5us end-to-end latency; keep every
            # DMA on the two HWDGE queues)
            git_sb = cpool.tile([128, 2 * BPC], f32, name="git_sb")
            nc.sync.dma_start(git_sb[:], git_ext[:])
            gir_sb = cpool.tile([1, CH * BPC], f32, name="gir_sb")
            nc.sync.dma_start(gir_sb[:], gir_ext[:])
            xps = [[[None, None], [None, None]] for _ in range(BPC)]
            for p in range(2):
                for h in range(2):
                    t_ = wpool.tile([128, 1024], mdt, name=f"xp1{h}{p}",
                                    tag="xp", bufs=8)
                    nc.sync.dma_start(
                        t_[:], x_ext[1, 128 * h:128 * (h + 1),
                                     1024 * p:1024 * (p + 1)])
                    xps[1][h][p] = t_
            # --- qAct: batch-0 x tiles ---
            for p in range(2):
                for h in range(2):
                    t_ = wpool.tile([128, 1024], mdt, name=f"xp0{h}{p}",
                                    tag="xp", bufs=8)
                    nc.scalar.dma_start(
                        t_[:], x_ext[0, 128 * h:128 * (h + 1),
                                     1024 * p:1024 * (p + 1)])
                    xps[0][h][p] = t_
            # conv stationaries + bias row: tiny, on qSP so the scheduler's
            # hoist-small-DMAs-first policy cannot delay batch-0 x on qAct;
            # one tile per matmul (matmul requires equal base partitions
            # for stationary and moving operands)
            wsj_all = []
            for j in range(NMM):
                jR = j * R
                lenj = min(R, K - jR)
                wsj = cpool.tile([G * lenj, D], mdt, name=f"wsj{j}")
                nc.sync.dma_start(wsj[:], ws_ext[G * jR:G * (jR + lenj), :])
                wsj_all.append(wsj)
            brow = cpool.tile([1, D], f32, name="brow")
            nc.sync.dma_start(brow[:], b_ext[:])

            # --- DVE: one-hot + count0 ---
            ms_all = []
            for b in range(BPC):
                ms = []
                for h in range(2):
                    m_t = wpool.tile([128, G], mdt, name=f"m{b}{h}",
                                     tag="m", bufs=4)
                    nc.vector.tensor_scalar(
                        out=m_t[:], in0=iota_f[:],
                        scalar1=git_sb[:, 2 * b + h:2 * b + h + 1],
                        scalar2=None, op0=eq)
                    ms.append(m_t)
                ms_all.append(ms)
            # --- PE: S = M^T @ x, evacuated into swin block 0 (DVE);
            #     then SBUF->SBUF replication DMAs (qSP) ---
            swin_all = []
            for b in range(BPC):
                swin = wpool.tile([NP, T], mdt, name=f"swin{b}", tag="swin")
                for c in range(NCHUNK):
                    off = 512 * c
                    p, o = off // 1024, off % 1024
                    ps = ppool.tile([G, 512], f32, name=f"ps{b}{c}", tag="ps")
                    nc.tensor.matmul(ps[:], ms_all[b][0][:],
                                     xps[b][0][p][:, o:o + 512],
                                     start=True, stop=False)
                    nc.tensor.matmul(ps[:], ms_all[b][1][:],
                                     xps[b][1][p][:, o:o + 512],
                                     start=False, stop=True)
                    nc.vector.tensor_copy(swin[0:G, off:off + 512], ps[:])
                eng = nc.scalar if b == 0 else nc.sync
                for r in range(1, R):
                    w = blk_w[r]
                    eng.dma_start(swin[G * r:G * (r + 1), 0:w],
                                  swin[0:G, r:r + w])
                swin_all.append(swin)

            # --- count0 (DVE, after S evacs so S never waits on it)
            #     then count0*bias broadcast to [64, BPC] on PE ---
            eq0 = wpool.tile([1, CH * BPC], f32, name="eq0", tag="eq0", bufs=1)
            nc.vector.tensor_scalar(out=eq0[:], in0=gir_sb[:], scalar1=0.0,
                                    scalar2=None, op0=eq)
            cnt2 = wpool.tile([1, BPC], f32, name="cnt2", tag="cnt2", bufs=1)
            for b in range(BPC):
                nc.vector.tensor_reduce(
                    out=cnt2[:, b:b + 1], in_=eq0[:, CH * b:CH * (b + 1)],
                    axis=mybir.AxisListType.X, op=add)
            pb = spool.tile([D, BPC], f32, name="pb", tag="pb")
            nc.tensor.matmul(pb[:], brow[:], cnt2[:],
                             start=True, stop=True)
            bcnt = wpool.tile([D, BPC], f32, name="bcnt", tag="bcnt", bufs=1)
            nc.vector.tensor_copy(bcnt[:], pb[:])

            # --- conv + fused bias-add + store ---
            for b in range(BPC):
                osb = wpool.tile([D, T_OUT], f32, name=f"osb{b}", tag="osb")
                for c in range(NCHUNK):
                    c0 = 512 * c
                    L = min(512, T_OUT - c0)
                    po = opool.tile([D, 512], f32, name=f"po{b}{c}", tag="po")
                    for j in range(NMM):
                        jR = j * R
                        lenj = min(R, K - jR)
                        nc.tensor.matmul(
                            po[:, :L],
                            wsj_all[j][:],
                            swin_all[b][0:G * lenj,
                                        c0 + jR:c0 + jR + L],
                            start=(j == 0), stop=(j == NMM - 1))
                    # fused bias add during evacuation; alternate DVE/ACT
                    if c % 2 == 0:
                        nc.vector.tensor_scalar(
                            out=osb[:, c0:c0 + L], in0=po[:, :L],
                            scalar1=bcnt[:, b:b + 1], scalar2=None, op0=add)
                    else:
                        nc.scalar.activation(
                            osb[:, c0:c0 + L], po[:, :L],
                            mybir.ActivationFunctionType.Identity,
                            bias=bcnt[:, b:b + 1])
                    if c == 1:
                        nc.scalar.dma_start(out_ext[b, :, 0:1024],
                                            osb[:, 0:1024])
                    elif c == NCHUNK - 1:
                        nc.scalar.dma_start(out_ext[b, :, 1024:T_OUT],
                                            osb[:, 1024:T_OUT])

    nc.compile()
    return nc


# ----------------------------------------------------------------------
# v1 baseline (kept for A/B comparison): config "<s_dtype>/<conv_mode>/<conv_dtype>"
def _build_v1(cfg: str):
    from concourse import bacc, tile
    import concourse.mybir as mybir

    s_dt, conv_mode, conv_dt = (cfg.split("/") + ["kaccum", "f32"])[:3] \
        if "/" in cfg else (cfg, "kaccum", cfg)
    f32 = mybir.dt.float32
    f32r = mybir.dt.float32r
    eq = mybir.AluOpType.is_equal
    mmdt = f32r if s_dt == "f32r" else f32
    use_r = s_dt == "f32r"
    cdt = f32r if conv_dt == "f32r" else f32

    nc = bacc.Bacc("TRN2", target_bir_lowering=False, debug=False,
                   num_devices=N_CORES)
    x_ext = nc.dram_tensor("x", [BPC, CH, T], f32, kind="ExternalInput").ap()
    gi_ext = nc.dram_tensor("gi", [BPC, 2, 128, 1], f32, kind="ExternalInput").ap()
    wt_ext = nc.dram_tensor("wt", [G, K * D], f32, kind="ExternalInput").ap()
    ws_ext = nc.dram_tensor("ws", [K * G, D], f32, kind="ExternalInput").ap()
    b_ext = nc.dram_tensor("bias", [1, D], f32, kind="ExternalInput").ap()
    io_ext = nc.dram_tensor("iota", [128, G], f32, kind="ExternalInput").ap()
    out_ext = nc.dram_tensor("out", [BPC, D, T_OUT], f32, kind="ExternalOutput").ap()
    swin = conv_mode == "swin"
    if swin:
        s_dram = nc.dram_tensor("s_dram", [BPC, G, T], cdt).ap()

    with tile.TileContext(nc) as tc:
        with (
            tc.tile_pool(name="const", bufs=1) as cpool,
            tc.tile_pool(name="work", bufs=2) as wpool,
            tc.tile_pool(name="ps_pool", bufs=4, space="PSUM") as ppool,
            tc.tile_pool(name="psmall", bufs=1, space="PSUM") as spool,
            tc.tile_pool(name="po_pool", bufs=3, space="PSUM") as opool,
        ):
            iota_f = cpool.tile([128, G], f32, name="iota_f")
            nc.sync.dma_start(iota_f[:], io_ext[:])
            if swin:
                ws_sb = cpool.tile([K * G, D], f32, name="ws_sb")
                nc.sync.dma_start(ws_sb[:], ws_ext[:])
                if cdt == f32r:
                    ws_r = cpool.tile([K * G, D], f32r, name="ws_r")
                    nc.vector.tensor_copy(ws_r[:], ws_sb[:])
                else:
                    ws_r = ws_sb
            else:
                wt_sb = cpool.tile([G, K * D], f32, name="wt_sb")
                nc.sync.dma_start(wt_sb[:], wt_ext[:])
                if cdt == f32r:
                    wt_r = cpool.tile([G, K * D], f32r, name="wt_r")
                    nc.vector.tensor_copy(wt_r[:], wt_sb[:])
                else:
                    wt_r = wt_sb
            brow = cpool.tile([1, D], f32, name="brow")
            nc.sync.dma_start(brow[:], b_ext[:])
            ones_col = cpool.tile([128, 1], f32, name="ones_col")
            nc.vector.memset(ones_col[:], 1.0)

            gi_all = []
            for b in range(BPC):
                gis = []
                for h in range(2):
                    gi_t = wpool.tile([128, 1], f32, name=f"gi{b}{h}",
                                      tag="gi", bufs=4)
                    nc.sync.dma_start(gi_t[:], gi_ext[b, h])
                    gis.append(gi_t)
                gi_all.append(gis)

            xps = [[[None, None], [None, None]] for _ in range(BPC)]
            for b in range(BPC):
                for h in range(2):
                    for p in range(2):
                        t_ = wpool.tile([128, 1024], f32, name=f"xp{b}{h}{p}",
                                        tag="xp", bufs=8)
                        nc.sync.dma_start(
                            t_[:],
                            x_ext[b, 128 * h:128 * (h + 1),
                                  1024 * p:1024 * (p + 1)])
                        if use_r:
                            xr = wpool.tile([128, 1024], f32r,
                                            name=f"xr{b}{h}{p}", tag="xr",
                                            bufs=8)
                            if (h + p) % 2 == 0:
                                nc.vector.tensor_copy(xr[:], t_[:])
                            else:
                                nc.scalar.activation(
                                    xr[:], t_[:],
                                    mybir.ActivationFunctionType.Copy)
                            t_ = xr
                        xps[b][h][p] = t_

            ms_all, bcnt_all = [], []
            for b in range(BPC):
                ms, ms_f = [], []
                for h in range(2):
                    gi_t = gi_all[b][h]
                    mf_t = wpool.tile([128, G], f32, name=f"mf{b}{h}",
                                      tag="mf", bufs=4)
                    nc.vector.tensor_scalar(out=mf_t[:], in0=iota_f[:],
                                            scalar1=gi_t[:, 0:1], scalar2=None,
                                            op0=eq)
                    ms_f.append(mf_t)
                    if use_r:
                        m_t = wpool.tile([128, G], mdt, name=f"m{b}{h}",
                                         tag="m", bufs=4)
                        nc.vector.tensor_copy(m_t[:], mf_t[:])
                    else:
                        m_t = mf_t
                    ms.append(m_t)
                ms_all.append(ms)

                pcnt = spool.tile([G, 1], f32, name=f"pcnt{b}", tag="pcnt")
                nc.tensor.matmul(pcnt[:], ms_f[0][:], ones_col[:],
                                 start=True, stop=False)
                nc.tensor.matmul(pcnt[:], ms_f[1][:], ones_col[:],
                                 start=False, stop=True)
                cnt_sb = wpool.tile([G, 1], f32, name=f"cnt{b}", tag="cnt")
                nc.vector.tensor_copy(cnt_sb[:], pcnt[:])
                pbc = spool.tile([D, 1], f32, name=f"pbc{b}", tag="pbc")
                nc.tensor.matmul(pbc[:], brow[:], cnt_sb[0:1, 0:1],
                                 start=True, stop=True)
                bcnt = wpool.tile([D, 1], f32, name=f"bcnt{b}", tag="bcnt")
                nc.vector.tensor_copy(bcnt[:], pbc[:])
                bcnt_all.append(bcnt)

            s_all = []
            for b in range(BPC):
                xp = xps[b]
                s_sb = wpool.tile([G, T], cdt, name=f"s{b}", tag="s")
                for c in range(4):
                    ps = ppool.tile([G, 512], f32, name=f"ps{b}{c}", tag="ps")
                    off = 512 * c
                    p, o = off // 1024, off % 1024
                    nc.tensor.matmul(ps[:], ms_all[b][0][:],
                                     xp[0][p][:, o:o + 512],
                                     start=True, stop=False)
                    nc.tensor.matmul(ps[:], ms_all[b][1][:],
                                     xp[1][p][:, o:o + 512],
                                     start=False, stop=True)
                    nc.vector.tensor_copy(s_sb[:, off:off + 512], ps[:])
                    if swin:
                        nc.sync.dma_start(s_dram[b, :, off:off + 512],
                                          s_sb[:, off:off + 512])
                s_all.append(s_sb)

            swin_all = []
            if swin:
                for b in range(BPC):
                    swin_sb = wpool.tile([K * G, T_OUT], cdt,
                                         name=f"swin{b}", tag="swin")
                    half = 1024
                    for lo, hi in ((0, half), (half, T_OUT)):
                        for k in range(K):
                            nc.sync.dma_start(
                                swin_sb[G * k:G * (k + 1), lo:hi],
                                s_dram[b, :, k + lo:k + hi])
                    swin_all.append(swin_sb)

            for b in range(BPC):
                for c in range(4):
                    c0 = 512 * c
                    L = min(512, T_OUT - c0)
                    po = opool.tile([D, 512], f32, name=f"po{b}{c}", tag="po")
                    if swin:
                        nc.tensor.matmul(po[:, :L], ws_r[:],
                                         swin_all[b][:, c0:c0 + L],
                                         start=True, stop=True)
                    else:
                        for k in range(K):
                            nc.tensor.matmul(po[:, :L],
                                             wt_r[:, D * k:D * (k + 1)],
                                             s_all[b][:, c0 + k:c0 + k + L],
                                             start=(k == 0), stop=(k == K - 1))
                    osb = wpool.tile([D, 512], f32, name=f"osb{b}{c}",
                                     tag="osb", bufs=4)
                    nc.scalar.activation(osb[:, :L], po[:, :L],
                                         mybir.ActivationFunctionType.Identity,
                                         bias=bcnt_all[b][:, 0:1])
                    nc.sync.dma_start(out_ext[b, :, c0:c0 + L], osb[:, :L])

    nc.compile()
    return nc


def _build(cfg: str):
    if cfg.startswith("v2") or cfg.startswith("v3"):
        return _build_v2(cfg)
    return _build_v1(cfg)


def _get_nc(mm_dtype: str):
    if mm_dtype not in _COMPILED:
        _COMPILED[mm_dtype] = _build(mm_dtype)
    return _COMPILED[mm_dtype]


def _run(x, group_idxs, W, bias, mm_dtype=None, trace=False, tmpdir=None):
    from concourse.bass_utils import run_bass_kernel_spmd

    cfg = mm_dtype or MM_DTYPE
    x = np.ascontiguousarray(np.asarray(x, dtype=np.float32))
    gi_f = np.asarray(group_idxs).astype(np.float32)  # [BS, CH]
    W = np.asarray(W, dtype=np.float32)
    bias = np.asarray(bias, dtype=np.float32)
    # ws[k*16+g, d] = W[g,d,k]
    ws = np.ascontiguousarray(W.transpose(2, 0, 1).reshape(K * G, D))
    brow = np.ascontiguousarray(bias.reshape(1, D))
    nc = _get_nc(cfg)

    in_maps = []
    if cfg.startswith("v2") or cfg.startswith("v3"):
        if cfg.startswith("v3"):
            import ml_dtypes
            x_send = x.astype(ml_dtypes.bfloat16)
            ws_send = ws.astype(ml_dtypes.bfloat16)
        else:
            x_send, ws_send = x, ws
        for i in range(N_CORES):
            sl = slice(i * BPC, (i + 1) * BPC)
            gi_c = gi_f[sl]  # [BPC, CH]
            # git[c, 2b+h] = gi[b, h*128+c]
            git = np.ascontiguousarray(
                gi_c.reshape(BPC * 2, 128).T)  # [128, 2*BPC]
            gir = np.ascontiguousarray(gi_c.reshape(1, BPC * CH))
            in_maps.append({
                "x": np.ascontiguousarray(x_send[sl]),
                "git": git,
                "gir": gir,
                "ws": ws_send,
                "bias": brow,
            })
    else:
        gi = gi_f.reshape(BS, 2, 128, 1)
        wt = np.ascontiguousarray(W.transpose(0, 2, 1).reshape(G, K * D))
        iota = np.ascontiguousarray(
            np.broadcast_to(np.arange(G, dtype=np.float32), (128, G)))
        for i in range(N_CORES):
            sl = slice(i * BPC, (i + 1) * BPC)
            in_maps.append({
                "x": np.ascontiguousarray(x[sl]),
                "gi": np.ascontiguousarray(gi[sl]),
                "wt": wt,
                "ws": ws,
                "bias": brow,
                "iota": iota,
            })
    res = run_bass_kernel_spmd(nc, in_maps, core_ids=list(range(N_CORES)),
                               trace=trace, tmpdir=tmpdir)
    out = np.concatenate([r["out"] for r in res.results], axis=0)
    assert out.shape == (BS, D, T_OUT)
    return out.astype(np.float32), res


def kernel(x, group_idxs, W, bias):
    out, _ = _run(x, group_idxs, W, bias)
    return out


# revision 12
# speedup vs baseline: 1.8993x; 1.1470x over previous
"""Grouped-index Conv1D (moe_routing) on 8 TRN2 NeuronCores.

Math:  out[b,d,t] = sum_c sum_k x[b,c,t+k] * W[gi[b,c],d,k] + count0[b]*bias[d]

v2 device algorithm (per core, 2 batches, data-parallel over batch):
  1. one-hot M[c,g] = (gi[b,c]==g) built on-chip (gpsimd iota + DVE is_equal)
  2. S[g,t] = sum_c M[c,g]*x[c,t]       (PE one-hot matmul, f32r via bitcast,
                                         written into swin block 0)
  3. swin[16r+g, t] = S[g, t+r]         (SBUF->SBUF DMA replication for
                                         r=1..R-1; no DRAM bounce)
  4. out[d,t] = sum_j Wj^T swin[...]    (ceil(7/R) PSUM-accumulated matmuls
                                         per 512-chunk)
  5. bias: count0 via DVE is_equal+reduce on a [1,512] gi row; one tiny
     matmul broadcasts count0*bias to [64,2]; fused add on DVE/ACT during
     PSUM->SBUF evacuation.
DMA queues: x split across qSP/qAct, control+weights on qPool (SWDGE),
replication on qSP, outputs on qAct.  (v1 used one queue: 59 serialized
DMAs ~ 35us; v2 has ~20 spread over 3 queues.)
"""

import sys
import numpy as np

sys.path.insert(0, "/opt/trn_rl_repo")

BS, CH, T = 16, 256, 2048
G, D, K = 16, 64, 7
T_OUT = T - K + 1  # 2042
N_CORES = 8
BPC = BS // N_CORES  # batches per core = 2

# default config: "v2/<R>" with R = partition replicas for the conv
# (R=7: 1 conv matmul/chunk, 6 repl DMAs/batch; R=4: 2 mm, 3 DMAs)
MM_DTYPE = "v2/4"

_COMPILED = {}

NCHUNK = 4  # T / 512


def _build_v2(cfg: str):
    from concourse import bacc, tile
    import concourse.mybir as mybir

    parts = cfg.split("/")
    R = int(parts[1]) if len(parts) > 1 else 4
    assert 1 <= R <= 7
    f32 = mybir.dt.float32
    f32r = mybir.dt.float32r
    # v2: f32r (raw f32 bits, 2e-2 budget); v3: bf16 (half the HBM/SBUF/DVE
    # bytes for x, swin, weights); v4: v3 + fp8e4m3 DoubleRow S stage
    # (one matmul per chunk, 2 channels contracted per cycle)
    ver = parts[0]
    s8 = ver == "v4"
    f8 = mybir.dt.float8e4
    bf16 = mybir.dt.bfloat16
    mdt = bf16 if ver in ("v3", "v4") else f32r
    eq = mybir.AluOpType.is_equal
    add = mybir.AluOpType.add
    NMM = (K + R - 1) // R  # conv matmuls per chunk
    NP = 16 * R             # swin partitions

    # block b of swin is read by conv matmul j (with blocks 0..len_j-1,
    # column window up to 1536 + j*R + 505); width needed per block:
    blk_w = [0] * R
    for j in range(NMM):
        jR = j * R
        lenj = min(R, K - jR)
        maxcol = 1536 + jR + (T_OUT - 1536) - 1  # 2041 + jR
        for b in range(lenj):
            blk_w[b] = max(blk_w[b], maxcol + 1)
    for b in range(1, R):
        assert blk_w[b] + b <= T, (b, blk_w[b])

    nc = bacc.Bacc("TRN2", target_bir_lowering=False, debug=False,
                   num_devices=N_CORES)
    if s8:
        # x8[b, c, j, t] = x[b, c + 128*j, t]  (fp8, DoubleRow k-tile layout)
        x_ext = nc.dram_tensor("x", [BPC, 128, 2 * T], f8,
                               kind="ExternalInput").ap()
    else:
        x_ext = nc.dram_tensor("x", [BPC, CH, T], mdt,
                               kind="ExternalInput").ap()
    git_ext = nc.dram_tensor("git", [128, 2 * BPC], f32, kind="ExternalInput").ap()
    gir_ext = nc.dram_tensor("gir", [1, CH * BPC], f32, kind="ExternalInput").ap()
    ws_ext = nc.dram_tensor("ws", [K * G, D], mdt, kind="ExternalInput").ap()
    b_ext = nc.dram_tensor("bias", [1, D], f32, kind="ExternalInput").ap()
    out_ext = nc.dram_tensor("out", [BPC, D, T_OUT], f32, kind="ExternalOutput").ap()

    with tile.TileContext(nc) as tc:
        with (
            tc.tile_pool(name="const", bufs=1) as cpool,
            tc.tile_pool(name="work", bufs=2) as wpool,
            tc.tile_pool(name="ps_pool", bufs=4, space="PSUM") as ppool,
            tc.tile_pool(name="psmall", bufs=1, space="PSUM") as spool,
            tc.tile_pool(name="po_pool", bufs=3, space="PSUM") as opool,
        ):
            # --- on-chip constants (Pool engine + SWDGE queue) ---
            iota_f = cpool.tile([128, G], f32, name="iota_f")
            nc.gpsimd.iota(iota_f[:], [[1, G]], channel_multiplier=0,
                           allow_small_or_imprecise_dtypes=True)
            # --- qSP: gi rows (tiny, feed one-hot + count) then batch-1 x ---
            # (SWDGE/gpsimd DMAs have ~5us end-to-end latency; keep every
            # DMA on the two HWDGE queues)
            git_sb = cpool.tile([128, 2 * BPC], f32, name="git_sb")
            nc.sync.dma_start(git_sb[:], git_ext[:])
            gir_sb = cpool.tile([1, CH * BPC], f32, name="gir_sb")
            nc.sync.dma_start(gir_sb[:], gir_ext[:])
            if s8:
                # one DMA per batch: [128, 2*T] fp8 (4KB/partition)
                x8t = []
                for b in range(BPC):
                    t_ = wpool.tile([128, 2 * T], f8, name=f"x8{b}",
                                    tag="xp", bufs=2)
                    (nc.scalar if b == 0 else nc.sync).dma_start(
                        t_[:], x_ext[b])
                    x8t.append(t_)
            else:
                xps = [[[None, None], [None, None]] for _ in range(BPC)]
                for p in range(2):
                    for h in range(2):
                        t_ = wpool.tile([128, 1024], mdt, name=f"xp1{h}{p}",
                                        tag="xp", bufs=8)
                        nc.sync.dma_start(
                            t_[:], x_ext[1, 128 * h:128 * (h + 1),
                                         1024 * p:1024 * (p + 1)])
                        xps[1][h][p] = t_
                # --- qAct: batch-0 x tiles ---
                for p in range(2):
                    for h in range(2):
                        t_ = wpool.tile([128, 1024], mdt, name=f"xp0{h}{p}",
                                        tag="xp", bufs=8)
                        nc.scalar.dma_start(
                            t_[:], x_ext[0, 128 * h:128 * (h + 1),
                                         1024 * p:1024 * (p + 1)])
                        xps[0][h][p] = t_
            # conv stationaries + bias row: tiny, on qSP so the scheduler's
            # hoist-small-DMAs-first policy cannot delay batch-0 x on qAct;
            # one tile per matmul (matmul requires equal base partitions
            # for stationary and moving operands)
            wsj_all = []
            for j in range(NMM):
                jR = j * R
                lenj = min(R, K - jR)
                wsj = cpool.tile([G * lenj, D], mdt, name=f"wsj{j}")
                nc.sync.dma_start(wsj[:], ws_ext[G * jR:G * (jR + lenj), :])
                wsj_all.append(wsj)
            brow = cpool.tile([1, D], f32, name="brow")
            nc.sync.dma_start(brow[:], b_ext[:])

            # --- DVE: one-hot + count0 ---
            ms_all = []
            if s8:
                # M2[c, 16j+g] = (gi[b, c+128j] == g), fp8 (0/1 exact)
                for b in range(BPC):
                    m2 = wpool.tile([128, 2 * G], f8, name=f"m2{b}",
                                    tag="m", bufs=2)
                    for h in range(2):
                        nc.vector.tensor_scalar(
                            out=m2[:, G * h:G * (h + 1)], in0=iota_f[:],
                            scalar1=git_sb[:, 2 * b + h:2 * b + h + 1],
                            scalar2=None, op0=eq)
                    ms_all.append(m2)
            else:
                for b in range(BPC):
                    ms = []
                    for h in range(2):
                        m_t = wpool.tile([128, G], mdt, name=f"m{b}{h}",
                                         tag="m", bufs=4)
                        nc.vector.tensor_scalar(
                            out=m_t[:], in0=iota_f[:],
                            scalar1=git_sb[:, 2 * b + h:2 * b + h + 1],
                            scalar2=None, op0=eq)
                        ms.append(m_t)
                    ms_all.append(ms)
            # --- PE: S = M^T @ x, evacuated into swin block 0 (DVE);
            #     then SBUF->SBUF replication DMAs (qSP) ---
            swin_all = []
            for b in range(BPC):
                swin = wpool.tile([NP, T], mdt, name=f"swin{b}", tag="swin")
                if s8:
                    lhs3 = ms_all[b][:, :].rearrange("p (j g) -> p j g", j=2)
                    rhs_full = x8t[b][:, :].rearrange("p (j t) -> p j t", j=2)
                for c in range(NCHUNK):
                    off = 512 * c
                    ps = ppool.tile([G, 512], f32, name=f"ps{b}{c}", tag="ps")
                    if s8:
                        nc.tensor.matmul(
                            ps[:], lhs3, rhs_full[:, :, off:off + 512],
                            start=True, stop=True,
                            perf_mode=mybir.MatmulPerfMode.DoubleRow)
                    else:
                        p, o = off // 1024, off % 1024
                        nc.tensor.matmul(ps[:], ms_all[b][0][:],
                                         xps[b][0][p][:, o:o + 512],
                                         start=True, stop=False)
                        nc.tensor.matmul(ps[:], ms_all[b][1][:],
                                         xps[b][1][p][:, o:o + 512],
                                         start=False, stop=True)
                    nc.vector.tensor_copy(swin[0:G, off:off + 512], ps[:])
                eng = nc.scalar if b == 0 else nc.sync
                for r in range(1, R):
                    w = blk_w[r]
                    eng.dma_start(swin[G * r:G * (r + 1), 0:w],
                                  swin[0:G, r:r + w])
                swin_all.append(swin)

            # --- count0 (DVE, after S evacs so S never waits on it)
            #     then count0*bias broadcast to [64, BPC] on PE ---
            eq0 = wpool.tile([1, CH * BPC], f32, name="eq0", tag="eq0", bufs=1)
            nc.vector.tensor_scalar(out=eq0[:], in0=gir_sb[:], scalar1=0.0,
                                    scalar2=None, op0=eq)
            cnt2 = wpool.tile([1, BPC], f32, name="cnt2", tag="cnt2", bufs=1)
            for b in range(BPC):
                nc.vector.tensor_reduce(
                    out=cnt2[:, b:b + 1], in_=eq0[:, CH * b:CH * (b + 1)],
                    axis=mybir.AxisListType.X, op=add)
            pb = spool.tile([D, BPC], f32, name="pb", tag="pb")
            nc.tensor.matmul(pb[:], brow[:], cnt2[:],
                             start=True, stop=True)
            bcnt = wpool.tile([D, BPC], f32, name="bcnt", tag="bcnt", bufs=1)
            nc.vector.tensor_copy(bcnt[:], pb[:])

            # --- conv + fused bias-add + store ---
            for b in range(BPC):
                osb = wpool.tile([D, T_OUT], f32, name=f"osb{b}", tag="osb")
                for c in range(NCHUNK):
                    c0 = 512 * c
                    L = min(512, T_OUT - c0)
                    po = opool.tile([D, 512], f32, name=f"po{b}{c}", tag="po")
                    for j in range(NMM):
                        jR = j * R
                        lenj = min(R, K - jR)
                        nc.tensor.matmul(
                            po[:, :L],
                            wsj_all[j][:],
                            swin_all[b][0:G * lenj,
                                        c0 + jR:c0 + jR + L],
                            start=(j == 0), stop=(j == NMM - 1))
                    # fused bias add during evacuation; alternate DVE/ACT
                    if c % 2 == 0:
                        nc.vector.tensor_scalar(
                            out=osb[:, c0:c0 + L], in0=po[:, :L],
                            scalar1=bcnt[:, b:b + 1], scalar2=None, op0=add)
                    else:
                        nc.scalar.activation(
                            osb[:, c0:c0 + L], po[:, :L],
                            mybir.ActivationFunctionType.Identity,
                            bias=bcnt[:, b:b + 1])
                    if c == 1:
                        nc.scalar.dma_start(out_ext[b, :, 0:1024],
                                            osb[:, 0:1024])
                    elif c == NCHUNK - 1:
                        nc.scalar.dma_start(out_ext[b, :, 1024:T_OUT],
                                            osb[:, 1024:T_OUT])

    nc.compile()
    return nc


# ----------------------------------------------------------------------
# v1 baseline (kept for A/B comparison): config "<s_dtype>/<conv_mode>/<conv_dtype>"
def _build_v1(cfg: str):
    from concourse import bacc, tile
    import concourse.mybir as mybir

    s_dt, conv_mode, conv_dt = (cfg.split("/") + ["kaccum", "f32"])[:3] \
        if "/" in cfg else (cfg, "kaccum", cfg)
    f32 = mybir.dt.float32
    f32r = mybir.dt.float32r
    eq = mybir.AluOpType.is_equal
    mmdt = f32r if s_dt == "f32r" else f32
    use_r = s_dt == "f32r"
    cdt = f32r if conv_dt == "f32r" else f32

    nc = bacc.Bacc("TRN2", target_bir_lowering=False, debug=False,
                   num_devices=N_CORES)
    x_ext = nc.dram_tensor("x", [BPC, CH, T], f32, kind="ExternalInput").ap()
    gi_ext = nc.dram_tensor("gi", [BPC, 2, 128, 1], f32, kind="ExternalInput").ap()
    wt_ext = nc.dram_tensor("wt", [G, K * D], f32, kind="ExternalInput").ap()
    ws_ext = nc.dram_tensor("ws", [K * G, D], f32, kind="ExternalInput").ap()
    b_ext = nc.dram_tensor("bias", [1, D], f32, kind="ExternalInput").ap()
    io_ext = nc.dram_tensor("iota", [128, G], f32, kind="ExternalInput").ap()
    out_ext = nc.dram_tensor("out", [BPC, D, T_OUT], f32, kind="ExternalOutput").ap()
    swin = conv_mode == "swin"
    if swin:
        s_dram = nc.dram_tensor("s_dram", [BPC, G, T], cdt).ap()

    with tile.TileContext(nc) as tc:
        with (
            tc.tile_pool(name="const", bufs=1) as cpool,
            tc.tile_pool(name="work", bufs=2) as wpool,
            tc.tile_pool(name="ps_pool", bufs=4, space="PSUM") as ppool,
            tc.tile_pool(name="psmall", bufs=1, space="PSUM") as spool,
            tc.tile_pool(name="po_pool", bufs=3, space="PSUM") as opool,
        ):
            iota_f = cpool.tile([128, G], f32, name="iota_f")
            nc.sync.dma_start(iota_f[:], io_ext[:])
            if swin:
                ws_sb = cpool.tile([K * G, D], f32, name="ws_sb")
                nc.sync.dma_start(ws_sb[:], ws_ext[:])
                if cdt == f32r:
                    ws_r = cpool.tile([K * G, D], f32r, name="ws_r")
                    nc.vector.tensor_copy(ws_r[:], ws_sb[:])
                else:
                    ws_r = ws_sb
            else:
                wt_sb = cpool.tile([G, K * D], f32, name="wt_sb")
                nc.sync.dma_start(wt_sb[:], wt_ext[:])
                if cdt == f32r:
                    wt_r = cpool.tile([G, K * D], f32r, name="wt_r")
                    nc.vector.tensor_copy(wt_r[:], wt_sb[:])
                else:
                    wt_r = wt_sb
            brow = cpool.tile([1, D], f32, name="brow")
            nc.sync.dma_start(brow[:], b_ext[:])
            ones_col = cpool.tile([128, 1], f32, name="ones_col")
            nc.vector.memset(ones_col[:], 1.0)

            gi_all = []
            for b in range(BPC):
                gis = []
                for h in range(2):
                    gi_t = wpool.tile([128, 1], f32, name=f"gi{b}{h}",
                                      tag="gi", bufs=4)
                    nc.sync.dma_start(gi_t[:], gi_ext[b, h])
                    gis.append(gi_t)
                gi_all.append(gis)

            xps = [[[None, None], [None, None]] for _ in range(BPC)]
            for b in range(BPC):
                for h in range(2):
                    for p in range(2):
                        t_ = wpool.tile([128, 1024], f32, name=f"xp{b}{h}{p}",
                                        tag="xp", bufs=8)
                        nc.sync.dma_start(
                            t_[:],
                            x_ext[b, 128 * h:128 * (h + 1),
                                  1024 * p:1024 * (p + 1)])
                        if use_r:
                            xr = wpool.tile([128, 1024], f32r,
                                            name=f"xr{b}{h}{p}", tag="xr",
                                            bufs=8)
                            if (h + p) % 2 == 0:
                                nc.vector.tensor_copy(xr[:], t_[:])
                            else:
                                nc.scalar.activation(
                                    xr[:], t_[:],
                                    mybir.ActivationFunctionType.Copy)
                            t_ = xr
                        xps[b][h][p] = t_

            ms_all, bcnt_all = [], []
            for b in range(BPC):
                ms, ms_f = [], []
                for h in range(2):
                    gi_t = gi_all[b][h]
                    mf_t = wpool.tile([128, G], f32, name=f"mf{b}{h}",
                                      tag="mf", bufs=4)
                    nc.vector.tensor_scalar(out=mf_t[:], in0=iota_f[:],
                                            scalar1=gi_t[:, 0:1], scalar2=None,
                                            op0=eq)
                    ms_f.append(mf_t)
                    if use_r:
                        m_t = wpool.tile([128, G], mdt, name=f"m{b}{h}",
                                         tag="m", bufs=4)
                        nc.vector.tensor_copy(m_t[:], mf_t[:])
                    else:
                        m_t = mf_t
                    ms.append(m_t)
                ms_all.append(ms)

                pcnt = spool.tile([G, 1], f32, name=f"pcnt{b}", tag="pcnt")
                nc.tensor.matmul(pcnt[:], ms_f[0][:], ones_col[:],
                                 start=True, stop=False)
                nc.tensor.matmul(pcnt[:], ms_f[1][:], ones_col[:],
                                 start=False, stop=True)
                cnt_sb = wpool.tile([G, 1], f32, name=f"cnt{b}", tag="cnt")
                nc.vector.tensor_copy(cnt_sb[:], pcnt[:])
                pbc = spool.tile([D, 1], f32, name=f"pbc{b}", tag="pbc")
                nc.tensor.matmul(pbc[:], brow[:], cnt_sb[0:1, 0:1],
                                 start=True, stop=True)
                bcnt = wpool.tile([D, 1], f32, name=f"bcnt{b}", tag="bcnt")
                nc.vector.tensor_copy(bcnt[:], pbc[:])
                bcnt_all.append(bcnt)

            s_all = []
            for b in range(BPC):
                xp = xps[b]
                s_sb = wpool.tile([G, T], cdt, name=f"s{b}", tag="s")
                for c in range(4):
                    ps = ppool.tile([G, 512], f32, name=f"ps{b}{c}", tag="ps")
                    off = 512 * c
                    p, o = off // 1024, off % 1024
                    nc.tensor.matmul(ps[:], ms_all[b][0][:],
                                     xp[0][p][:, o:o + 512],
                                     start=True, stop=False)
                    nc.tensor.matmul(ps[:], ms_all[b][1][:],
                                     xp[1][p][:, o:o + 512],
                                     start=False, stop=True)
                    nc.vector.tensor_copy(s_sb[:, off:off + 512], ps[:])
                    if swin:
                        nc.sync.dma_start(s_dram[b, :, off:off + 512],
                                          s_sb[:, off:off + 512])
                s_all.append(s_sb)

            swin_all = []
            if swin:
                for b in range(BPC):
                    swin_sb = wpool.tile([K * G, T_OUT], cdt,
                                         name=f"swin{b}", tag="swin")
                    half = 1024
                    for lo, hi in ((0, half), (half, T_OUT)):
                        for k in range(K):
                            nc.sync.dma_start(
                                swin_sb[G * k:G * (k + 1), lo:hi],
                                s_dram[b, :, k + lo:k + hi])
                    swin_all.append(swin_sb)

            for b in range(BPC):
                for c in range(4):
                    c0 = 512 * c
                    L = min(512, T_OUT - c0)
                    po = opool.tile([D, 512], f32, name=f"po{b}{c}", tag="po")
                    if swin:
                        nc.tensor.matmul(po[:, :L], ws_r[:],
                                         swin_all[b][:, c0:c0 + L],
                                         start=True, stop=True)
                    else:
                        for k in range(K):
                            nc.tensor.matmul(po[:, :L],
                                             wt_r[:, D * k:D * (k + 1)],
                                             s_all[b][:, c0 + k:c0 + k + L],
                                             start=(k == 0), stop=(k == K - 1))
                    osb = wpool.tile([D, 512], f32, name=f"osb{b}{c}",
                                     tag="osb", bufs=4)
                    nc.scalar.activation(osb[:, :L], po[:, :L],
                                         mybir.ActivationFunctionType.Identity,
                                         bias=bcnt_all[b][:, 0:1])
                    nc.sync.dma_start(out_ext[b, :, c0:c0 + L], osb[:, :L])

    nc.compile()
    return nc


def _build(cfg: str):
    if cfg.startswith(("v2", "v3", "v4")):
        return _build_v2(cfg)
    return _build_v1(cfg)


def _get_nc(mm_dtype: str):
    if mm_dtype not in _COMPILED:
        _COMPILED[mm_dtype] = _build(mm_dtype)
    return _COMPILED[mm_dtype]


def _run(x, group_idxs, W, bias, mm_dtype=None, trace=False, tmpdir=None):
    from concourse.bass_utils import run_bass_kernel_spmd

    cfg = mm_dtype or MM_DTYPE
    x = np.ascontiguousarray(np.asarray(x, dtype=np.float32))
    gi_f = np.asarray(group_idxs).astype(np.float32)  # [BS, CH]
    W = np.asarray(W, dtype=np.float32)
    bias = np.asarray(bias, dtype=np.float32)
    # ws[k*16+g, d] = W[g,d,k]
    ws = np.ascontiguousarray(W.transpose(2, 0, 1).reshape(K * G, D))
    brow = np.ascontiguousarray(bias.reshape(1, D))
    nc = _get_nc(cfg)

    in_maps = []
    if cfg.startswith(("v2", "v3", "v4")):
        import ml_dtypes
        if cfg.startswith("v4"):
            # [BS, 128, 2*T] fp8: x8[b, c, j*T+t] = x[b, c+128j, t]
            x_send = np.ascontiguousarray(
                x.reshape(BS, 2, 128, T).transpose(0, 2, 1, 3)
                 .reshape(BS, 128, 2 * T)).astype(ml_dtypes.float8_e4m3)
            ws_send = ws.astype(ml_dtypes.bfloat16)
        elif cfg.startswith("v3"):
            x_send = x.astype(ml_dtypes.bfloat16)
            ws_send = ws.astype(ml_dtypes.bfloat16)
        else:
            x_send, ws_send = x, ws
        for i in range(N_CORES):
            sl = slice(i * BPC, (i + 1) * BPC)
            gi_c = gi_f[sl]  # [BPC, CH]
            # git[c, 2b+h] = gi[b, h*128+c]
            git = np.ascontiguousarray(
                gi_c.reshape(BPC * 2, 128).T)  # [128, 2*BPC]
            gir = np.ascontiguousarray(gi_c.reshape(1, BPC * CH))
            in_maps.append({
                "x": np.ascontiguousarray(x_send[sl]),
                "git": git,
                "gir": gir,
                "ws": ws_send,
                "bias": brow,
            })
    else:
        gi = gi_f.reshape(BS, 2, 128, 1)
        wt = np.ascontiguousarray(W.transpose(0, 2, 1).reshape(G, K * D))
        iota = np.ascontiguousarray(
            np.broadcast_to(np.arange(G, dtype=np.float32), (128, G)))
        for i in range(N_CORES):
            sl = slice(i * BPC, (i + 1) * BPC)
            in_maps.append({
                "x": np.ascontiguousarray(x[sl]),
                "gi": np.ascontiguousarray(gi[sl]),
                "wt": wt,
                "ws": ws,
                "bias": brow,
                "iota": iota,
            })
    res = run_bass_kernel_spmd(nc, in_maps, core_ids=list(range(N_CORES)),
                               trace=trace, tmpdir=tmpdir)
    out = np.concatenate([r["out"] for r in res.results], axis=0)
    assert out.shape == (BS, D, T_OUT)
    return out.astype(np.float32), res


def kernel(x, group_idxs, W, bias):
    out, _ = _run(x, group_idxs, W, bias)
    return out


# revision 13
# speedup vs baseline: 1.9140x; 1.0077x over previous
"""Grouped-index Conv1D (moe_routing) on 8 TRN2 NeuronCores.

Math:  out[b,d,t] = sum_c sum_k x[b,c,t+k] * W[gi[b,c],d,k] + count0[b]*bias[d]

v2 device algorithm (per core, 2 batches, data-parallel over batch):
  1. one-hot M[c,g] = (gi[b,c]==g) built on-chip (gpsimd iota + DVE is_equal)
  2. S[g,t] = sum_c M[c,g]*x[c,t]       (PE one-hot matmul, f32r via bitcast,
                                         written into swin block 0)
  3. swin[16r+g, t] = S[g, t+r]         (SBUF->SBUF DMA replication for
                                         r=1..R-1; no DRAM bounce)
  4. out[d,t] = sum_j Wj^T swin[...]    (ceil(7/R) PSUM-accumulated matmuls
                                         per 512-chunk)
  5. bias: count0 via DVE is_equal+reduce on a [1,512] gi row; one tiny
     matmul broadcasts count0*bias to [64,2]; fused add on DVE/ACT during
     PSUM->SBUF evacuation.
DMA queues: x split across qSP/qAct, control+weights on qPool (SWDGE),
replication on qSP, outputs on qAct.  (v1 used one queue: 59 serialized
DMAs ~ 35us; v2 has ~20 spread over 3 queues.)
"""

import sys
import numpy as np

sys.path.insert(0, "/opt/trn_rl_repo")

BS, CH, T = 16, 256, 2048
G, D, K = 16, 64, 7
T_OUT = T - K + 1  # 2042
N_CORES = 8
BPC = BS // N_CORES  # batches per core = 2

# default config: "v2/<R>" with R = partition replicas for the conv
# (R=7: 1 conv matmul/chunk, 6 repl DMAs/batch; R=4: 2 mm, 3 DMAs)
MM_DTYPE = "v2/4"

_COMPILED = {}

NCHUNK = 4  # T / 512


def _build_v2(cfg: str):
    from concourse import bacc, tile
    import concourse.mybir as mybir

    parts = cfg.split("/")
    R = int(parts[1]) if len(parts) > 1 else 4
    assert 1 <= R <= 7
    f32 = mybir.dt.float32
    f32r = mybir.dt.float32r
    # v2: f32r (raw f32 bits, 2e-2 budget); v3: bf16 (half the HBM/SBUF/DVE
    # bytes for x, swin, weights); v4: v3 + fp8e4m3 DoubleRow S stage
    # (one matmul per chunk, 2 channels contracted per cycle)
    ver = parts[0]
    s8 = ver == "v4"
    f8 = mybir.dt.float8e4
    bf16 = mybir.dt.bfloat16
    mdt = bf16 if ver in ("v3", "v4") else f32r
    eq = mybir.AluOpType.is_equal
    add = mybir.AluOpType.add
    NMM = (K + R - 1) // R  # conv matmuls per chunk
    NP = 16 * R             # swin partitions

    # block b of swin is read by conv matmul j (with blocks 0..len_j-1,
    # column window up to 1536 + j*R + 505); width needed per block:
    blk_w = [0] * R
    for j in range(NMM):
        jR = j * R
        lenj = min(R, K - jR)
        maxcol = 1536 + jR + (T_OUT - 1536) - 1  # 2041 + jR
        for b in range(lenj):
            blk_w[b] = max(blk_w[b], maxcol + 1)
    for b in range(1, R):
        assert blk_w[b] + b <= T, (b, blk_w[b])

    nc = bacc.Bacc("TRN2", target_bir_lowering=False, debug=False,
                   num_devices=N_CORES)
    if s8:
        # x8[b, c, j, t] = x[b, c + 128*j, t]  (fp8, DoubleRow k-tile layout)
        x_ext = nc.dram_tensor("x", [BPC, 128, 2 * T], f8,
                               kind="ExternalInput").ap()
    else:
        x_ext = nc.dram_tensor("x", [BPC, CH, T], mdt,
                               kind="ExternalInput").ap()
    git_ext = nc.dram_tensor("git", [128, 2 * BPC], f32, kind="ExternalInput").ap()
    gir_ext = nc.dram_tensor("gir", [1, CH * BPC], f32, kind="ExternalInput").ap()
    ws_ext = nc.dram_tensor("ws", [K * G, D], mdt, kind="ExternalInput").ap()
    b_ext = nc.dram_tensor("bias", [1, D], f32, kind="ExternalInput").ap()
    out_ext = nc.dram_tensor("out", [BPC, D, T_OUT], f32, kind="ExternalOutput").ap()

    with tile.TileContext(nc) as tc:
        with (
            tc.tile_pool(name="const", bufs=1) as cpool,
            tc.tile_pool(name="work", bufs=2) as wpool,
            tc.tile_pool(name="ps_pool", bufs=4, space="PSUM") as ppool,
            tc.tile_pool(name="psmall", bufs=1, space="PSUM") as spool,
            tc.tile_pool(name="po_pool", bufs=3, space="PSUM") as opool,
        ):
            # --- on-chip constants (Pool engine + SWDGE queue) ---
            iota_f = cpool.tile([128, G], f32, name="iota_f")
            nc.gpsimd.iota(iota_f[:], [[1, G]], channel_multiplier=0,
                           allow_small_or_imprecise_dtypes=True)
            # --- qSP: gi rows (tiny, feed one-hot + count) then batch-1 x ---
            # (SWDGE/gpsimd DMAs have ~5us end-to-end latency; keep every
            # DMA on the two HWDGE queues)
            git_sb = cpool.tile([128, 2 * BPC], f32, name="git_sb")
            nc.sync.dma_start(git_sb[:], git_ext[:])
            gir_sb = cpool.tile([1, CH * BPC], f32, name="gir_sb")
            nc.sync.dma_start(gir_sb[:], gir_ext[:])
            if s8:
                # one DMA per batch: [128, 2*T] fp8 (4KB/partition)
                x8t = []
                for b in range(BPC):
                    t_ = wpool.tile([128, 2 * T], f8, name=f"x8{b}",
                                    tag="xp", bufs=2)
                    (nc.scalar if b == 0 else nc.sync).dma_start(
                        t_[:], x_ext[b])
                    x8t.append(t_)
            else:
                xps = [[[None, None], [None, None]] for _ in range(BPC)]
                for p in range(2):
                    for h in range(2):
                        t_ = wpool.tile([128, 1024], mdt, name=f"xp1{h}{p}",
                                        tag="xp", bufs=8)
                        nc.sync.dma_start(
                            t_[:], x_ext[1, 128 * h:128 * (h + 1),
                                         1024 * p:1024 * (p + 1)])
                        xps[1][h][p] = t_
                # --- qAct: batch-0 x tiles ---
                for p in range(2):
                    for h in range(2):
                        t_ = wpool.tile([128, 1024], mdt, name=f"xp0{h}{p}",
                                        tag="xp", bufs=8)
                        nc.scalar.dma_start(
                            t_[:], x_ext[0, 128 * h:128 * (h + 1),
                                         1024 * p:1024 * (p + 1)])
                        xps[0][h][p] = t_
            # conv stationaries + bias row: tiny, on qSP so the scheduler's
            # hoist-small-DMAs-first policy cannot delay batch-0 x on qAct;
            # one tile per matmul (matmul requires equal base partitions
            # for stationary and moving operands)
            wsj_all = []
            for j in range(NMM):
                jR = j * R
                lenj = min(R, K - jR)
                wsj = cpool.tile([G * lenj, D], mdt, name=f"wsj{j}")
                nc.sync.dma_start(wsj[:], ws_ext[G * jR:G * (jR + lenj), :])
                wsj_all.append(wsj)
            brow = cpool.tile([1, D], f32, name="brow")
            nc.sync.dma_start(brow[:], b_ext[:])

            # --- DVE: one-hot + count0 ---
            ms_all = []
            if s8:
                # M2[c, 16j+g] = (gi[b, c+128j] == g), fp8 (0/1 exact)
                for b in range(BPC):
                    m2 = wpool.tile([128, 2 * G], f8, name=f"m2{b}",
                                    tag="m", bufs=2)
                    for h in range(2):
                        nc.vector.tensor_scalar(
                            out=m2[:, G * h:G * (h + 1)], in0=iota_f[:],
                            scalar1=git_sb[:, 2 * b + h:2 * b + h + 1],
                            scalar2=None, op0=eq)
                    ms_all.append(m2)
            else:
                for b in range(BPC):
                    ms = []
                    for h in range(2):
                        m_t = wpool.tile([128, G], mdt, name=f"m{b}{h}",
                                         tag="m", bufs=4)
                        nc.vector.tensor_scalar(
                            out=m_t[:], in0=iota_f[:],
                            scalar1=git_sb[:, 2 * b + h:2 * b + h + 1],
                            scalar2=None, op0=eq)
                        ms.append(m_t)
                    ms_all.append(ms)
            # --- PE: S = M^T @ x, evacuated into swin block 0 (DVE);
            #     then SBUF->SBUF replication DMAs (qSP) ---
            swin_all = []
            for b in range(BPC):
                swin = wpool.tile([NP, T], mdt, name=f"swin{b}", tag="swin")
                if s8:
                    lhs3 = ms_all[b][:, :].rearrange("p (j g) -> p j g", j=2)
                    rhs_full = x8t[b][:, :].rearrange("p (j t) -> p j t", j=2)
                for c in range(NCHUNK):
                    off = 512 * c
                    ps = ppool.tile([G, 512], f32, name=f"ps{b}{c}", tag="ps")
                    if s8:
                        nc.tensor.matmul(
                            ps[:], lhs3, rhs_full[:, :, off:off + 512],
                            start=True, stop=True,
                            perf_mode=mybir.MatmulPerfMode.DoubleRow)
                    else:
                        p, o = off // 1024, off % 1024
                        nc.tensor.matmul(ps[:], ms_all[b][0][:],
                                         xps[b][0][p][:, o:o + 512],
                                         start=True, stop=False)
                        nc.tensor.matmul(ps[:], ms_all[b][1][:],
                                         xps[b][1][p][:, o:o + 512],
                                         start=False, stop=True)
                    nc.vector.tensor_copy(swin[0:G, off:off + 512], ps[:])
                # replicate in column halves, split across both queues:
                # cols [0,1021) need only S chunks 0-1 (1021+r <= 1024 for
                # r<=3... for r>3 cut lower), so conv chunk 0 can start
                # while chunks 2-3 still evacuate
                engs = [nc.scalar, nc.sync] if b == 0 else [nc.sync, nc.scalar]
                cut = 1024 - max(r for r in range(1, R))
                for half in range(2):
                    for ri, r in enumerate(range(1, R)):
                        w = blk_w[r]
                        lo, hi = (0, cut) if half == 0 else (cut, w)
                        engs[(ri + half) % 2].dma_start(
                            swin[G * r:G * (r + 1), lo:hi],
                            swin[0:G, r + lo:r + hi])
                swin_all.append(swin)

            # --- count0 (DVE, after S evacs so S never waits on it)
            #     then count0*bias broadcast to [64, BPC] on PE ---
            eq0 = wpool.tile([1, CH * BPC], f32, name="eq0", tag="eq0", bufs=1)
            nc.vector.tensor_scalar(out=eq0[:], in0=gir_sb[:], scalar1=0.0,
                                    scalar2=None, op0=eq)
            cnt2 = wpool.tile([1, BPC], f32, name="cnt2", tag="cnt2", bufs=1)
            for b in range(BPC):
                nc.vector.tensor_reduce(
                    out=cnt2[:, b:b + 1], in_=eq0[:, CH * b:CH * (b + 1)],
                    axis=mybir.AxisListType.X, op=add)
            pb = spool.tile([D, BPC], f32, name="pb", tag="pb")
            nc.tensor.matmul(pb[:], brow[:], cnt2[:],
                             start=True, stop=True)
            bcnt = wpool.tile([D, BPC], f32, name="bcnt", tag="bcnt", bufs=1)
            nc.vector.tensor_copy(bcnt[:], pb[:])

            # --- conv + fused bias-add + store ---
            for b in range(BPC):
                osb = wpool.tile([D, T_OUT], f32, name=f"osb{b}", tag="osb")
                for c in range(NCHUNK):
                    c0 = 512 * c
                    L = min(512, T_OUT - c0)
                    po = opool.tile([D, 512], f32, name=f"po{b}{c}", tag="po")
                    for j in range(NMM):
                        jR = j * R
                        lenj = min(R, K - jR)
                        nc.tensor.matmul(
                            po[:, :L],
                            wsj_all[j][:],
                            swin_all[b][0:G * lenj,
                                        c0 + jR:c0 + jR + L],
                            start=(j == 0), stop=(j == NMM - 1))
                    # fused bias add during evacuation; alternate DVE/ACT
                    if c % 2 == 0:
                        nc.vector.tensor_scalar(
                            out=osb[:, c0:c0 + L], in0=po[:, :L],
                            scalar1=bcnt[:, b:b + 1], scalar2=None, op0=add)
                    else:
                        nc.scalar.activation(
                            osb[:, c0:c0 + L], po[:, :L],
                            mybir.ActivationFunctionType.Identity,
                            bias=bcnt[:, b:b + 1])
                    if c == 1:
                        nc.scalar.dma_start(out_ext[b, :, 0:1024],
                                            osb[:, 0:1024])
                    elif c == NCHUNK - 1:
                        nc.scalar.dma_start(out_ext[b, :, 1024:T_OUT],
                                            osb[:, 1024:T_OUT])

    nc.compile()
    return nc


# ----------------------------------------------------------------------
# v1 baseline (kept for A/B comparison): config "<s_dtype>/<conv_mode>/<conv_dtype>"
def _build_v1(cfg: str):
    from concourse import bacc, tile
    import concourse.mybir as mybir

    s_dt, conv_mode, conv_dt = (cfg.split("/") + ["kaccum", "f32"])[:3] \
        if "/" in cfg else (cfg, "kaccum", cfg)
    f32 = mybir.dt.float32
    f32r = mybir.dt.float32r
    eq = mybir.AluOpType.is_equal
    mmdt = f32r if s_dt == "f32r" else f32
    use_r = s_dt == "f32r"
    cdt = f32r if conv_dt == "f32r" else f32

    nc = bacc.Bacc("TRN2", target_bir_lowering=False, debug=False,
                   num_devices=N_CORES)
    x_ext = nc.dram_tensor("x", [BPC, CH, T], f32, kind="ExternalInput").ap()
    gi_ext = nc.dram_tensor("gi", [BPC, 2, 128, 1], f32, kind="ExternalInput").ap()
    wt_ext = nc.dram_tensor("wt", [G, K * D], f32, kind="ExternalInput").ap()
    ws_ext = nc.dram_tensor("ws", [K * G, D], f32, kind="ExternalInput").ap()
    b_ext = nc.dram_tensor("bias", [1, D], f32, kind="ExternalInput").ap()
    io_ext = nc.dram_tensor("iota", [128, G], f32, kind="ExternalInput").ap()
    out_ext = nc.dram_tensor("out", [BPC, D, T_OUT], f32, kind="ExternalOutput").ap()
    swin = conv_mode == "swin"
    if swin:
        s_dram = nc.dram_tensor("s_dram", [BPC, G, T], cdt).ap()

    with tile.TileContext(nc) as tc:
        with (
            tc.tile_pool(name="const", bufs=1) as cpool,
            tc.tile_pool(name="work", bufs=2) as wpool,
            tc.tile_pool(name="ps_pool", bufs=4, space="PSUM") as ppool,
            tc.tile_pool(name="psmall", bufs=1, space="PSUM") as spool,
            tc.tile_pool(name="po_pool", bufs=3, space="PSUM") as opool,
        ):
            iota_f = cpool.tile([128, G], f32, name="iota_f")
            nc.sync.dma_start(iota_f[:], io_ext[:])
            if swin:
                ws_sb = cpool.tile([K * G, D], f32, name="ws_sb")
                nc.sync.dma_start(ws_sb[:], ws_ext[:])
                if cdt == f32r:
                    ws_r = cpool.tile([K * G, D], f32r, name="ws_r")
                    nc.vector.tensor_copy(ws_r[:], ws_sb[:])
                else:
                    ws_r = ws_sb
            else:
                wt_sb = cpool.tile([G, K * D], f32, name="wt_sb")
                nc.sync.dma_start(wt_sb[:], wt_ext[:])
                if cdt == f32r:
                    wt_r = cpool.tile([G, K * D], f32r, name="wt_r")
                    nc.vector.tensor_copy(wt_r[:], wt_sb[:])
                else:
                    wt_r = wt_sb
            brow = cpool.tile([1, D], f32, name="brow")
            nc.sync.dma_start(brow[:], b_ext[:])
            ones_col = cpool.tile([128, 1], f32, name="ones_col")
            nc.vector.memset(ones_col[:], 1.0)

            gi_all = []
            for b in range(BPC):
                gis = []
                for h in range(2):
                    gi_t = wpool.tile([128, 1], f32, name=f"gi{b}{h}",
                                      tag="gi", bufs=4)
                    nc.sync.dma_start(gi_t[:], gi_ext[b, h])
                    gis.append(gi_t)
                gi_all.append(gis)

            xps = [[[None, None], [None, None]] for _ in range(BPC)]
            for b in range(BPC):
                for h in range(2):
                    for p in range(2):
                        t_ = wpool.tile([128, 1024], f32, name=f"xp{b}{h}{p}",
                                        tag="xp", bufs=8)
                        nc.sync.dma_start(
                            t_[:],
                            x_ext[b, 128 * h:128 * (h + 1),
                                  1024 * p:1024 * (p + 1)])
                        if use_r:
                            xr = wpool.tile([128, 1024], f32r,
                                            name=f"xr{b}{h}{p}", tag="xr",
                                            bufs=8)
                            if (h + p) % 2 == 0:
                                nc.vector.tensor_copy(xr[:], t_[:])
                            else:
                                nc.scalar.activation(
                                    xr[:], t_[:],
                                    mybir.ActivationFunctionType.Copy)
                            t_ = xr
                        xps[b][h][p] = t_

            ms_all, bcnt_all = [], []
            for b in range(BPC):
                ms, ms_f = [], []
                for h in range(2):
                    gi_t = gi_all[b][h]
                    mf_t = wpool.tile([128, G], f32, name=f"mf{b}{h}",
                                      tag="mf", bufs=4)
                    nc.vector.tensor_scalar(out=mf_t[:], in0=iota_f[:],
                                            scalar1=gi_t[:, 0:1], scalar2=None,
                                            op0=eq)
                    ms_f.append(mf_t)
                    if use_r:
                        m_t = wpool.tile([128, G], mdt, name=f"m{b}{h}",
                                         tag="m", bufs=4)
                        nc.vector.tensor_copy(m_t[:], mf_t[:])
                    else:
                        m_t = mf_t
                    ms.append(m_t)
                ms_all.append(ms)

                pcnt = spool.tile([G, 1], f32, name=f"pcnt{b}", tag="pcnt")
                nc.tensor.matmul(pcnt[:], ms_f[0][:], ones_col[:],
                                 start=True, stop=False)
                nc.tensor.matmul(pcnt[:], ms_f[1][:], ones_col[:],
                                 start=False, stop=True)
                cnt_sb = wpool.tile([G, 1], f32, name=f"cnt{b}", tag="cnt")
                nc.vector.tensor_copy(cnt_sb[:], pcnt[:])
                pbc = spool.tile([D, 1], f32, name=f"pbc{b}", tag="pbc")
                nc.tensor.matmul(pbc[:], brow[:], cnt_sb[0:1, 0:1],
                                 start=True, stop=True)
                bcnt = wpool.tile([D, 1], f32, name=f"bcnt{b}", tag="bcnt")
                nc.vector.tensor_copy(bcnt[:], pbc[:])
                bcnt_all.append(bcnt)

            s_all = []
            for b in range(BPC):
                xp = xps[b]
                s_sb = wpool.tile([G, T], cdt, name=f"s{b}", tag="s")
                for c in range(4):
                    ps = ppool.tile([G, 512], f32, name=f"ps{b}{c}", tag="ps")
                    off = 512 * c
                    p, o = off // 1024, off % 1024
                    nc.tensor.matmul(ps[:], ms_all[b][0][:],
                                     xp[0][p][:, o:o + 512],
                                     start=True, stop=False)
                    nc.tensor.matmul(ps[:], ms_all[b][1][:],
                                     xp[1][p][:, o:o + 512],
                                     start=False, stop=True)
                    nc.vector.tensor_copy(s_sb[:, off:off + 512], ps[:])
                    if swin:
                        nc.sync.dma_start(s_dram[b, :, off:off + 512],
                                          s_sb[:, off:off + 512])
                s_all.append(s_sb)

            swin_all = []
            if swin:
                for b in range(BPC):
                    swin_sb = wpool.tile([K * G, T_OUT], cdt,
                                         name=f"swin{b}", tag="swin")
                    half = 1024
                    for lo, hi in ((0, half), (half, T_OUT)):
                        for k in range(K):
                            nc.sync.dma_start(
                                swin_sb[G * k:G * (k + 1), lo:hi],
                                s_dram[b, :, k + lo:k + hi])
                    swin_all.append(swin_sb)

            for b in range(BPC):
                for c in range(4):
                    c0 = 512 * c
                    L = min(512, T_OUT - c0)
                    po = opool.tile([D, 512], f32, name=f"po{b}{c}", tag="po")
                    if swin:
                        nc.tensor.matmul(po[:, :L], ws_r[:],
                                         swin_all[b][:, c0:c0 + L],
                                         start=True, stop=True)
                    else:
                        for k in range(K):
                            nc.tensor.matmul(po[:, :L],
                                             wt_r[:, D * k:D * (k + 1)],
                                             s_all[b][:, c0 + k:c0 + k + L],
                                             start=(k == 0), stop=(k == K - 1))
                    osb = wpool.tile([D, 512], f32, name=f"osb{b}{c}",
                                     tag="osb", bufs=4)
                    nc.scalar.activation(osb[:, :L], po[:, :L],
                                         mybir.ActivationFunctionType.Identity,
                                         bias=bcnt_all[b][:, 0:1])
                    nc.sync.dma_start(out_ext[b, :, c0:c0 + L], osb[:, :L])

    nc.compile()
    return nc


def _build(cfg: str):
    if cfg.startswith(("v2", "v3", "v4")):
        return _build_v2(cfg)
    return _build_v1(cfg)


def _get_nc(mm_dtype: str):
    if mm_dtype not in _COMPILED:
        _COMPILED[mm_dtype] = _build(mm_dtype)
    return _COMPILED[mm_dtype]


def _run(x, group_idxs, W, bias, mm_dtype=None, trace=False, tmpdir=None):
    from concourse.bass_utils import run_bass_kernel_spmd

    cfg = mm_dtype or MM_DTYPE
    x = np.ascontiguousarray(np.asarray(x, dtype=np.float32))
    gi_f = np.asarray(group_idxs).astype(np.float32)  # [BS, CH]
    W = np.asarray(W, dtype=np.float32)
    bias = np.asarray(bias, dtype=np.float32)
    # ws[k*16+g, d] = W[g,d,k]
    ws = np.ascontiguousarray(W.transpose(2, 0, 1).reshape(K * G, D))
    brow = np.ascontiguousarray(bias.reshape(1, D))
    nc = _get_nc(cfg)

    in_maps = []
    if cfg.startswith(("v2", "v3", "v4")):
        import ml_dtypes
        if cfg.startswith("v4"):
            # [BS, 128, 2*T] fp8: x8[b, c, j*T+t] = x[b, c+128j, t]
            x_send = np.ascontiguousarray(
                x.reshape(BS, 2, 128, T).transpose(0, 2, 1, 3)
                 .reshape(BS, 128, 2 * T)).astype(ml_dtypes.float8_e4m3)
            ws_send = ws.astype(ml_dtypes.bfloat16)
        elif cfg.startswith("v3"):
            x_send = x.astype(ml_dtypes.bfloat16)
            ws_send = ws.astype(ml_dtypes.bfloat16)
        else:
            x_send, ws_send = x, ws
        for i in range(N_CORES):
            sl = slice(i * BPC, (i + 1) * BPC)
            gi_c = gi_f[sl]  # [BPC, CH]
            # git[c, 2b+h] = gi[b, h*128+c]
            git = np.ascontiguousarray(
                gi_c.reshape(BPC * 2, 128).T)  # [128, 2*BPC]
            gir = np.ascontiguousarray(gi_c.reshape(1, BPC * CH))
            in_maps.append({
                "x": np.ascontiguousarray(x_send[sl]),
                "git": git,
                "gir": gir,
                "ws": ws_send,
                "bias": brow,
            })
    else:
        gi = gi_f.reshape(BS, 2, 128, 1)
        wt = np.ascontiguousarray(W.transpose(0, 2, 1).reshape(G, K * D))
        iota = np.ascontiguousarray(
            np.broadcast_to(np.arange(G, dtype=np.float32), (128, G)))
        for i in range(N_CORES):
            sl = slice(i * BPC, (i + 1) * BPC)
            in_maps.append({
                "x": np.ascontiguousarray(x[sl]),
                "gi": np.ascontiguousarray(gi[sl]),
                "wt": wt,
                "ws": ws,
                "bias": brow,
                "iota": iota,
            })
    res = run_bass_kernel_spmd(nc, in_maps, core_ids=list(range(N_CORES)),
                               trace=trace, tmpdir=tmpdir)
    out = np.concatenate([r["out"] for r in res.results], axis=0)
    assert out.shape == (BS, D, T_OUT)
    return out.astype(np.float32), res


def kernel(x, group_idxs, W, bias):
    out, _ = _run(x, group_idxs, W, bias)
    return out


# revision 14
# speedup vs baseline: 1.9339x; 1.0104x over previous
"""Grouped-index Conv1D (moe_routing) on 8 TRN2 NeuronCores.

Math:  out[b,d,t] = sum_c sum_k x[b,c,t+k] * W[gi[b,c],d,k] + count0[b]*bias[d]

v2 device algorithm (per core, 2 batches, data-parallel over batch):
  1. one-hot M[c,g] = (gi[b,c]==g) built on-chip (gpsimd iota + DVE is_equal)
  2. S[g,t] = sum_c M[c,g]*x[c,t]       (PE one-hot matmul, f32r via bitcast,
                                         written into swin block 0)
  3. swin[16r+g, t] = S[g, t+r]         (SBUF->SBUF DMA replication for
                                         r=1..R-1; no DRAM bounce)
  4. out[d,t] = sum_j Wj^T swin[...]    (ceil(7/R) PSUM-accumulated matmuls
                                         per 512-chunk)
  5. bias: count0 via DVE is_equal+reduce on a [1,512] gi row; one tiny
     matmul broadcasts count0*bias to [64,2]; fused add on DVE/ACT during
     PSUM->SBUF evacuation.
DMA queues: x split across qSP/qAct, control+weights on qPool (SWDGE),
replication on qSP, outputs on qAct.  (v1 used one queue: 59 serialized
DMAs ~ 35us; v2 has ~20 spread over 3 queues.)
"""

import sys
import numpy as np

sys.path.insert(0, "/opt/trn_rl_repo")

BS, CH, T = 16, 256, 2048
G, D, K = 16, 64, 7
T_OUT = T - K + 1  # 2042
N_CORES = 8
BPC = BS // N_CORES  # batches per core = 2

# default config: "v2/<R>" with R = partition replicas for the conv
# (R=7: 1 conv matmul/chunk, 6 repl DMAs/batch; R=4: 2 mm, 3 DMAs)
MM_DTYPE = "v2/4"

_COMPILED = {}

NCHUNK = 4  # T / 512


def _build_v2(cfg: str):
    from concourse import bacc, tile
    import concourse.mybir as mybir

    parts = cfg.split("/")
    R = int(parts[1]) if len(parts) > 1 else 4
    assert 1 <= R <= 7
    f32 = mybir.dt.float32
    f32r = mybir.dt.float32r
    # v2: f32r (raw f32 bits, 2e-2 budget); v3: bf16 (half the HBM/SBUF/DVE
    # bytes for x, swin, weights); v4: v3 + fp8e4m3 DoubleRow S stage
    # (one matmul per chunk, 2 channels contracted per cycle)
    ver = parts[0]
    s8 = ver == "v4"
    f8 = mybir.dt.float8e4
    bf16 = mybir.dt.bfloat16
    mdt = bf16 if ver in ("v3", "v4") else f32r
    eq = mybir.AluOpType.is_equal
    add = mybir.AluOpType.add
    NMM = (K + R - 1) // R  # conv matmuls per chunk
    NP = 16 * R             # swin partitions

    # block b of swin is read by conv matmul j (with blocks 0..len_j-1,
    # column window up to 1536 + j*R + 505); width needed per block:
    blk_w = [0] * R
    for j in range(NMM):
        jR = j * R
        lenj = min(R, K - jR)
        maxcol = 1536 + jR + (T_OUT - 1536) - 1  # 2041 + jR
        for b in range(lenj):
            blk_w[b] = max(blk_w[b], maxcol + 1)
    for b in range(1, R):
        assert blk_w[b] + b <= T, (b, blk_w[b])

    nc = bacc.Bacc("TRN2", target_bir_lowering=False, debug=False,
                   num_devices=N_CORES)
    if s8:
        # x8[b, c, j, t] = x[b, c + 128*j, t]  (fp8, DoubleRow k-tile layout)
        x_ext = nc.dram_tensor("x", [BPC, 128, 2 * T], f8,
                               kind="ExternalInput").ap()
    else:
        x_ext = nc.dram_tensor("x", [BPC, CH, T], mdt,
                               kind="ExternalInput").ap()
    git_ext = nc.dram_tensor("git", [128, 2 * BPC], f32, kind="ExternalInput").ap()
    gir_ext = nc.dram_tensor("gir", [1, CH * BPC], f32, kind="ExternalInput").ap()
    ws_ext = nc.dram_tensor("ws", [K * G, D], mdt, kind="ExternalInput").ap()
    b_ext = nc.dram_tensor("bias", [1, D], f32, kind="ExternalInput").ap()
    out_ext = nc.dram_tensor("out", [BPC, D, T_OUT], f32, kind="ExternalOutput").ap()

    with tile.TileContext(nc) as tc:
        with (
            tc.tile_pool(name="const", bufs=1) as cpool,
            tc.tile_pool(name="work", bufs=2) as wpool,
            tc.tile_pool(name="ps_pool", bufs=4, space="PSUM") as ppool,
            tc.tile_pool(name="psmall", bufs=1, space="PSUM") as spool,
            tc.tile_pool(name="po_pool", bufs=3, space="PSUM") as opool,
        ):
            # --- on-chip constants (Pool engine + SWDGE queue) ---
            iota_f = cpool.tile([128, G], f32, name="iota_f")
            nc.gpsimd.iota(iota_f[:], [[1, G]], channel_multiplier=0,
                           allow_small_or_imprecise_dtypes=True)
            # --- qSP: gi rows (tiny, feed one-hot + count) then batch-1 x ---
            # (SWDGE/gpsimd DMAs have ~5us end-to-end latency; keep every
            # DMA on the two HWDGE queues)
            git_sb = cpool.tile([128, 2 * BPC], f32, name="git_sb")
            nc.sync.dma_start(git_sb[:], git_ext[:])
            gir_sb = cpool.tile([1, CH * BPC], f32, name="gir_sb")
            nc.sync.dma_start(gir_sb[:], gir_ext[:])
            if s8:
                # one DMA per batch: [128, 2*T] fp8 (4KB/partition)
                x8t = []
                for b in range(BPC):
                    t_ = wpool.tile([128, 2 * T], f8, name=f"x8{b}",
                                    tag="xp", bufs=2)
                    (nc.scalar if b == 0 else nc.sync).dma_start(
                        t_[:], x_ext[b])
                    x8t.append(t_)
            else:
                xps = [[[None, None], [None, None]] for _ in range(BPC)]
                for p in range(2):
                    for h in range(2):
                        t_ = wpool.tile([128, 1024], mdt, name=f"xp1{h}{p}",
                                        tag="xp", bufs=8)
                        nc.sync.dma_start(
                            t_[:], x_ext[1, 128 * h:128 * (h + 1),
                                         1024 * p:1024 * (p + 1)])
                        xps[1][h][p] = t_
                # --- qAct: batch-0 x tiles ---
                for p in range(2):
                    for h in range(2):
                        t_ = wpool.tile([128, 1024], mdt, name=f"xp0{h}{p}",
                                        tag="xp", bufs=8)
                        nc.scalar.dma_start(
                            t_[:], x_ext[0, 128 * h:128 * (h + 1),
                                         1024 * p:1024 * (p + 1)])
                        xps[0][h][p] = t_
            # conv stationaries + bias row: tiny, on qSP so the scheduler's
            # hoist-small-DMAs-first policy cannot delay batch-0 x on qAct;
            # one tile per matmul (matmul requires equal base partitions
            # for stationary and moving operands)
            wsj_all = []
            for j in range(NMM):
                jR = j * R
                lenj = min(R, K - jR)
                wsj = cpool.tile([G * lenj, D], mdt, name=f"wsj{j}")
                nc.sync.dma_start(wsj[:], ws_ext[G * jR:G * (jR + lenj), :])
                wsj_all.append(wsj)
            brow = cpool.tile([1, D], f32, name="brow")
            nc.sync.dma_start(brow[:], b_ext[:])

            # --- DVE: one-hot + count0 ---
            ms_all = []
            if s8:
                # M2[c, 16j+g] = (gi[b, c+128j] == g), fp8 (0/1 exact)
                for b in range(BPC):
                    m2 = wpool.tile([128, 2 * G], f8, name=f"m2{b}",
                                    tag="m", bufs=2)
                    for h in range(2):
                        nc.vector.tensor_scalar(
                            out=m2[:, G * h:G * (h + 1)], in0=iota_f[:],
                            scalar1=git_sb[:, 2 * b + h:2 * b + h + 1],
                            scalar2=None, op0=eq)
                    ms_all.append(m2)
            else:
                for b in range(BPC):
                    ms = []
                    for h in range(2):
                        m_t = wpool.tile([128, G], mdt, name=f"m{b}{h}",
                                         tag="m", bufs=4)
                        nc.vector.tensor_scalar(
                            out=m_t[:], in0=iota_f[:],
                            scalar1=git_sb[:, 2 * b + h:2 * b + h + 1],
                            scalar2=None, op0=eq)
                        ms.append(m_t)
                    ms_all.append(ms)
            # --- PE: S = M^T @ x, evacuated into swin block 0 (DVE);
            #     then SBUF->SBUF replication DMAs (qSP) ---
            swin_all = []
            for b in range(BPC):
                swin = wpool.tile([NP, T], mdt, name=f"swin{b}", tag="swin")
                if s8:
                    lhs3 = ms_all[b][:, :].rearrange("p (j g) -> p j g", j=2)
                    rhs_full = x8t[b][:, :].rearrange("p (j t) -> p j t", j=2)
                for c in range(NCHUNK):
                    off = 512 * c
                    ps = ppool.tile([G, 512], f32, name=f"ps{b}{c}", tag="ps")
                    if s8:
                        nc.tensor.matmul(
                            ps[:], lhs3, rhs_full[:, :, off:off + 512],
                            start=True, stop=True,
                            perf_mode=mybir.MatmulPerfMode.DoubleRow)
                    else:
                        p, o = off // 1024, off % 1024
                        nc.tensor.matmul(ps[:], ms_all[b][0][:],
                                         xps[b][0][p][:, o:o + 512],
                                         start=True, stop=False)
                        nc.tensor.matmul(ps[:], ms_all[b][1][:],
                                         xps[b][1][p][:, o:o + 512],
                                         start=False, stop=True)
                    nc.vector.tensor_copy(swin[0:G, off:off + 512], ps[:])
                # replicate in three column slices, split across both
                # queues; slice s only needs S chunks <= s+1 evacuated
                # ([0,520) <- evacs c0-c1, [520,1530) <- c2, rest <- c3),
                # so conv chunk 0 unlocks while later chunks still evacuate
                # and the PE pipeline never drains (a PE gap resets the
                # 2.4GHz clock ramp)
                engs = [nc.scalar, nc.sync] if b == 0 else [nc.sync, nc.scalar]
                cuts = (0, 520, 1530)
                for s in range(3):
                    for ri, r in enumerate(range(1, R)):
                        w = blk_w[r]
                        lo = cuts[s]
                        hi = cuts[s + 1] if s < 2 else w
                        assert r + hi <= T
                        engs[(ri + s) % 2].dma_start(
                            swin[G * r:G * (r + 1), lo:hi],
                            swin[0:G, r + lo:r + hi])
                swin_all.append(swin)

            # --- count0 (DVE, after S evacs so S never waits on it)
            #     then count0*bias broadcast to [64, BPC] on PE ---
            eq0 = wpool.tile([1, CH * BPC], f32, name="eq0", tag="eq0", bufs=1)
            nc.vector.tensor_scalar(out=eq0[:], in0=gir_sb[:], scalar1=0.0,
                                    scalar2=None, op0=eq)
            cnt2 = wpool.tile([1, BPC], f32, name="cnt2", tag="cnt2", bufs=1)
            for b in range(BPC):
                nc.vector.tensor_reduce(
                    out=cnt2[:, b:b + 1], in_=eq0[:, CH * b:CH * (b + 1)],
                    axis=mybir.AxisListType.X, op=add)
            pb = spool.tile([D, BPC], f32, name="pb", tag="pb")
            nc.tensor.matmul(pb[:], brow[:], cnt2[:],
                             start=True, stop=True)
            bcnt = wpool.tile([D, BPC], f32, name="bcnt", tag="bcnt", bufs=1)
            nc.vector.tensor_copy(bcnt[:], pb[:])

            # --- conv + fused bias-add + store ---
            for b in range(BPC):
                osb = wpool.tile([D, T_OUT], f32, name=f"osb{b}", tag="osb")
                for c in range(NCHUNK):
                    c0 = 512 * c
                    L = min(512, T_OUT - c0)
                    po = opool.tile([D, 512], f32, name=f"po{b}{c}", tag="po")
                    for j in range(NMM):
                        jR = j * R
                        lenj = min(R, K - jR)
                        nc.tensor.matmul(
                            po[:, :L],
                            wsj_all[j][:],
                            swin_all[b][0:G * lenj,
                                        c0 + jR:c0 + jR + L],
                            start=(j == 0), stop=(j == NMM - 1))
                    # fused bias add during evacuation; alternate DVE/ACT
                    if c % 2 == 0:
                        nc.vector.tensor_scalar(
                            out=osb[:, c0:c0 + L], in0=po[:, :L],
                            scalar1=bcnt[:, b:b + 1], scalar2=None, op0=add)
                    else:
                        nc.scalar.activation(
                            osb[:, c0:c0 + L], po[:, :L],
                            mybir.ActivationFunctionType.Identity,
                            bias=bcnt[:, b:b + 1])
                    if c == 1:
                        nc.scalar.dma_start(out_ext[b, :, 0:1024],
                                            osb[:, 0:1024])
                    elif c == NCHUNK - 1:
                        nc.scalar.dma_start(out_ext[b, :, 1024:T_OUT],
                                            osb[:, 1024:T_OUT])

    nc.compile()
    return nc


# ----------------------------------------------------------------------
# v1 baseline (kept for A/B comparison): config "<s_dtype>/<conv_mode>/<conv_dtype>"
def _build_v1(cfg: str):
    from concourse import bacc, tile
    import concourse.mybir as mybir

    s_dt, conv_mode, conv_dt = (cfg.split("/") + ["kaccum", "f32"])[:3] \
        if "/" in cfg else (cfg, "kaccum", cfg)
    f32 = mybir.dt.float32
    f32r = mybir.dt.float32r
    eq = mybir.AluOpType.is_equal
    mmdt = f32r if s_dt == "f32r" else f32
    use_r = s_dt == "f32r"
    cdt = f32r if conv_dt == "f32r" else f32

    nc = bacc.Bacc("TRN2", target_bir_lowering=False, debug=False,
                   num_devices=N_CORES)
    x_ext = nc.dram_tensor("x", [BPC, CH, T], f32, kind="ExternalInput").ap()
    gi_ext = nc.dram_tensor("gi", [BPC, 2, 128, 1], f32, kind="ExternalInput").ap()
    wt_ext = nc.dram_tensor("wt", [G, K * D], f32, kind="ExternalInput").ap()
    ws_ext = nc.dram_tensor("ws", [K * G, D], f32, kind="ExternalInput").ap()
    b_ext = nc.dram_tensor("bias", [1, D], f32, kind="ExternalInput").ap()
    io_ext = nc.dram_tensor("iota", [128, G], f32, kind="ExternalInput").ap()
    out_ext = nc.dram_tensor("out", [BPC, D, T_OUT], f32, kind="ExternalOutput").ap()
    swin = conv_mode == "swin"
    if swin:
        s_dram = nc.dram_tensor("s_dram", [BPC, G, T], cdt).ap()

    with tile.TileContext(nc) as tc:
        with (
            tc.tile_pool(name="const", bufs=1) as cpool,
            tc.tile_pool(name="work", bufs=2) as wpool,
            tc.tile_pool(name="ps_pool", bufs=4, space="PSUM") as ppool,
            tc.tile_pool(name="psmall", bufs=1, space="PSUM") as spool,
            tc.tile_pool(name="po_pool", bufs=3, space="PSUM") as opool,
        ):
            iota_f = cpool.tile([128, G], f32, name="iota_f")
            nc.sync.dma_start(iota_f[:], io_ext[:])
            if swin:
                ws_sb = cpool.tile([K * G, D], f32, name="ws_sb")
                nc.sync.dma_start(ws_sb[:], ws_ext[:])
                if cdt == f32r:
                    ws_r = cpool.tile([K * G, D], f32r, name="ws_r")
                    nc.vector.tensor_copy(ws_r[:], ws_sb[:])
                else:
                    ws_r = ws_sb
            else:
                wt_sb = cpool.tile([G, K * D], f32, name="wt_sb")
                nc.sync.dma_start(wt_sb[:], wt_ext[:])
                if cdt == f32r:
                    wt_r = cpool.tile([G, K * D], f32r, name="wt_r")
                    nc.vector.tensor_copy(wt_r[:], wt_sb[:])
                else:
                    wt_r = wt_sb
            brow = cpool.tile([1, D], f32, name="brow")
            nc.sync.dma_start(brow[:], b_ext[:])
            ones_col = cpool.tile([128, 1], f32, name="ones_col")
            nc.vector.memset(ones_col[:], 1.0)

            gi_all = []
            for b in range(BPC):
                gis = []
                for h in range(2):
                    gi_t = wpool.tile([128, 1], f32, name=f"gi{b}{h}",
                                      tag="gi", bufs=4)
                    nc.sync.dma_start(gi_t[:], gi_ext[b, h])
                    gis.append(gi_t)
                gi_all.append(gis)

            xps = [[[None, None], [None, None]] for _ in range(BPC)]
            for b in range(BPC):
                for h in range(2):
                    for p in range(2):
                        t_ = wpool.tile([128, 1024], f32, name=f"xp{b}{h}{p}",
                                        tag="xp", bufs=8)
                        nc.sync.dma_start(
                            t_[:],
                            x_ext[b, 128 * h:128 * (h + 1),
                                  1024 * p:1024 * (p + 1)])
                        if use_r:
                            xr = wpool.tile([128, 1024], f32r,
                                            name=f"xr{b}{h}{p}", tag="xr",
                                            bufs=8)
                            if (h + p) % 2 == 0:
                                nc.vector.tensor_copy(xr[:], t_[:])
                            else:
                                nc.scalar.activation(
                                    xr[:], t_[:],
                                    mybir.ActivationFunctionType.Copy)
                            t_ = xr
                        xps[b][h][p] = t_

            ms_all, bcnt_all = [], []
            for b in range(BPC):
                ms, ms_f = [], []
                for h in range(2):
                    gi_t = gi_all[b][h]
                    mf_t = wpool.tile([128, G], f32, name=f"mf{b}{h}",
                                      tag="mf", bufs=4)
                    nc.vector.tensor_scalar(out=mf_t[:], in0=iota_f[:],
                                            scalar1=gi_t[:, 0:1], scalar2=None,
                                            op0=eq)
                    ms_f.append(mf_t)
                    if use_r:
                        m_t = wpool.tile([128, G], mdt, name=f"m{b}{h}",
                                         tag="m", bufs=4)
                        nc.vector.tensor_copy(m_t[:], mf_t[:])
                    else:
                        m_t = mf_t
                    ms.append(m_t)
                ms_all.append(ms)

                pcnt = spool.tile([G, 1], f32, name=f"pcnt{b}", tag="pcnt")
                nc.tensor.matmul(pcnt[:], ms_f[0][:], ones_col[:],
                                 start=True, stop=False)
                nc.tensor.matmul(pcnt[:], ms_f[1][:], ones_col[:],
                                 start=False, stop=True)
                cnt_sb = wpool.tile([G, 1], f32, name=f"cnt{b}", tag="cnt")
                nc.vector.tensor_copy(cnt_sb[:], pcnt[:])
                pbc = spool.tile([D, 1], f32, name=f"pbc{b}", tag="pbc")
                nc.tensor.matmul(pbc[:], brow[:], cnt_sb[0:1, 0:1],
                                 start=True, stop=True)
                bcnt = wpool.tile([D, 1], f32, name=f"bcnt{b}", tag="bcnt")
                nc.vector.tensor_copy(bcnt[:], pbc[:])
                bcnt_all.append(bcnt)

            s_all = []
            for b in range(BPC):
                xp = xps[b]
                s_sb = wpool.tile([G, T], cdt, name=f"s{b}", tag="s")
                for c in range(4):
                    ps = ppool.tile([G, 512], f32, name=f"ps{b}{c}", tag="ps")
                    off = 512 * c
                    p, o = off // 1024, off % 1024
                    nc.tensor.matmul(ps[:], ms_all[b][0][:],
                                     xp[0][p][:, o:o + 512],
                                     start=True, stop=False)
                    nc.tensor.matmul(ps[:], ms_all[b][1][:],
                                     xp[1][p][:, o:o + 512],
                                     start=False, stop=True)
                    nc.vector.tensor_copy(s_sb[:, off:off + 512], ps[:])
                    if swin:
                        nc.sync.dma_start(s_dram[b, :, off:off + 512],
                                          s_sb[:, off:off + 512])
                s_all.append(s_sb)

            swin_all = []
            if swin:
                for b in range(BPC):
                    swin_sb = wpool.tile([K * G, T_OUT], cdt,
                                         name=f"swin{b}", tag="swin")
                    half = 1024
                    for lo, hi in ((0, half), (half, T_OUT)):
                        for k in range(K):
                            nc.sync.dma_start(
                                swin_sb[G * k:G * (k + 1), lo:hi],
                                s_dram[b, :, k + lo:k + hi])
                    swin_all.append(swin_sb)

            for b in range(BPC):
                for c in range(4):
                    c0 = 512 * c
                    L = min(512, T_OUT - c0)
                    po = opool.tile([D, 512], f32, name=f"po{b}{c}", tag="po")
                    if swin:
                        nc.tensor.matmul(po[:, :L], ws_r[:],
                                         swin_all[b][:, c0:c0 + L],
                                         start=True, stop=True)
                    else:
                        for k in range(K):
                            nc.tensor.matmul(po[:, :L],
                                             wt_r[:, D * k:D * (k + 1)],
                                             s_all[b][:, c0 + k:c0 + k + L],
                                             start=(k == 0), stop=(k == K - 1))
                    osb = wpool.tile([D, 512], f32, name=f"osb{b}{c}",
                                     tag="osb", bufs=4)
                    nc.scalar.activation(osb[:, :L], po[:, :L],
                                         mybir.ActivationFunctionType.Identity,
                                         bias=bcnt_all[b][:, 0:1])
                    nc.sync.dma_start(out_ext[b, :, c0:c0 + L], osb[:, :L])

    nc.compile()
    return nc


def _build(cfg: str):
    if cfg.startswith(("v2", "v3", "v4")):
        return _build_v2(cfg)
    return _build_v1(cfg)


def _get_nc(mm_dtype: str):
    if mm_dtype not in _COMPILED:
        _COMPILED[mm_dtype] = _build(mm_dtype)
    return _COMPILED[mm_dtype]


def _run(x, group_idxs, W, bias, mm_dtype=None, trace=False, tmpdir=None):
    from concourse.bass_utils import run_bass_kernel_spmd

    cfg = mm_dtype or MM_DTYPE
    x = np.ascontiguousarray(np.asarray(x, dtype=np.float32))
    gi_f = np.asarray(group_idxs).astype(np.float32)  # [BS, CH]
    W = np.asarray(W, dtype=np.float32)
    bias = np.asarray(bias, dtype=np.float32)
    # ws[k*16+g, d] = W[g,d,k]
    ws = np.ascontiguousarray(W.transpose(2, 0, 1).reshape(K * G, D))
    brow = np.ascontiguousarray(bias.reshape(1, D))
    nc = _get_nc(cfg)

    in_maps = []
    if cfg.startswith(("v2", "v3", "v4")):
        import ml_dtypes
        if cfg.startswith("v4"):
            # [BS, 128, 2*T] fp8: x8[b, c, j*T+t] = x[b, c+128j, t]
            x_send = np.ascontiguousarray(
                x.reshape(BS, 2, 128, T).transpose(0, 2, 1, 3)
                 .reshape(BS, 128, 2 * T)).astype(ml_dtypes.float8_e4m3)
            ws_send = ws.astype(ml_dtypes.bfloat16)
        elif cfg.startswith("v3"):
            x_send = x.astype(ml_dtypes.bfloat16)
            ws_send = ws.astype(ml_dtypes.bfloat16)
        else:
            x_send, ws_send = x, ws
        for i in range(N_CORES):
            sl = slice(i * BPC, (i + 1) * BPC)
            gi_c = gi_f[sl]  # [BPC, CH]
            # git[c, 2b+h] = gi[b, h*128+c]
            git = np.ascontiguousarray(
                gi_c.reshape(BPC * 2, 128).T)  # [128, 2*BPC]
            gir = np.ascontiguousarray(gi_c.reshape(1, BPC * CH))
            in_maps.append({
                "x": np.ascontiguousarray(x_send[sl]),
                "git": git,
                "gir": gir,
                "ws": ws_send,
                "bias": brow,
            })
    else:
        gi = gi_f.reshape(BS, 2, 128, 1)
        wt = np.ascontiguousarray(W.transpose(0, 2, 1).reshape(G, K * D))
        iota = np.ascontiguousarray(
            np.broadcast_to(np.arange(G, dtype=np.float32), (128, G)))
        for i in range(N_CORES):
            sl = slice(i * BPC, (i + 1) * BPC)
            in_maps.append({
                "x": np.ascontiguousarray(x[sl]),
                "gi": np.ascontiguousarray(gi[sl]),
                "wt": wt,
                "ws": ws,
                "bias": brow,
                "iota": iota,
            })
    res = run_bass_kernel_spmd(nc, in_maps, core_ids=list(range(N_CORES)),
                               trace=trace, tmpdir=tmpdir)
    out = np.concatenate([r["out"] for r in res.results], axis=0)
    assert out.shape == (BS, D, T_OUT)
    return out.astype(np.float32), res


def kernel(x, group_idxs, W, bias):
    out, _ = _run(x, group_idxs, W, bias)
    return out


# revision 15
# speedup vs baseline: 2.0184x; 1.0437x over previous
"""Grouped-index Conv1D (moe_routing) on 8 TRN2 NeuronCores.

Math:  out[b,d,t] = sum_c sum_k x[b,c,t+k] * W[gi[b,c],d,k] + count0[b]*bias[d]

v2 device algorithm (per core, 2 batches, data-parallel over batch):
  1. one-hot M[c,g] = (gi[b,c]==g) built on-chip (gpsimd iota + DVE is_equal)
  2. S[g,t] = sum_c M[c,g]*x[c,t]       (PE one-hot matmul, f32r via bitcast,
                                         written into swin block 0)
  3. swin[16r+g, t] = S[g, t+r]         (SBUF->SBUF DMA replication for
                                         r=1..R-1; no DRAM bounce)
  4. out[d,t] = sum_j Wj^T swin[...]    (ceil(7/R) PSUM-accumulated matmuls
                                         per 512-chunk)
  5. bias: count0 via DVE is_equal+reduce on a [1,512] gi row; one tiny
     matmul broadcasts count0*bias to [64,2]; fused add on DVE/ACT during
     PSUM->SBUF evacuation.
DMA queues: x split across qSP/qAct, control+weights on qPool (SWDGE),
replication on qSP, outputs on qAct.  (v1 used one queue: 59 serialized
DMAs ~ 35us; v2 has ~20 spread over 3 queues.)
"""

import sys
import numpy as np

sys.path.insert(0, "/opt/trn_rl_repo")

BS, CH, T = 16, 256, 2048
G, D, K = 16, 64, 7
T_OUT = T - K + 1  # 2042
N_CORES = 8
BPC = BS // N_CORES  # batches per core = 2

# default config: "v2/<R>" with R = partition replicas for the conv
# (R=7: 1 conv matmul/chunk, 6 repl DMAs/batch; R=4: 2 mm, 3 DMAs)
MM_DTYPE = "v2/4"

_COMPILED = {}

NCHUNK = 4  # T / 512


def _build_v2(cfg: str):
    from concourse import bacc, tile
    import concourse.mybir as mybir

    parts = cfg.split("/")
    R = int(parts[1]) if len(parts) > 1 else 4
    assert 1 <= R <= 7
    f32 = mybir.dt.float32
    f32r = mybir.dt.float32r
    # v2: f32r (raw f32 bits, 2e-2 budget); v3: bf16 (half the HBM/SBUF/DVE
    # bytes for x, swin, weights); v4: v3 + fp8e4m3 DoubleRow S stage
    # (one matmul per chunk, 2 channels contracted per cycle)
    ver = parts[0]
    s8 = ver == "v4"
    f8 = mybir.dt.float8e4
    bf16 = mybir.dt.bfloat16
    mdt = bf16 if ver in ("v3", "v4") else f32r
    eq = mybir.AluOpType.is_equal
    add = mybir.AluOpType.add
    NMM = (K + R - 1) // R  # conv matmuls per chunk
    NP = 16 * R             # swin partitions

    # block b of swin is read by conv matmul j (with blocks 0..len_j-1,
    # column window up to 1536 + j*R + 505); width needed per block:
    blk_w = [0] * R
    for j in range(NMM):
        jR = j * R
        lenj = min(R, K - jR)
        maxcol = 1536 + jR + (T_OUT - 1536) - 1  # 2041 + jR
        for b in range(lenj):
            blk_w[b] = max(blk_w[b], maxcol + 1)
    for b in range(1, R):
        assert blk_w[b] + b <= T, (b, blk_w[b])

    nc = bacc.Bacc("TRN2", target_bir_lowering=False, debug=False,
                   num_devices=N_CORES)
    if s8:
        # x8[b, c, j, t] = x[b, c + 128*j, t]  (fp8, DoubleRow k-tile layout)
        x_ext = nc.dram_tensor("x", [BPC, 128, 2 * T], f8,
                               kind="ExternalInput").ap()
    else:
        x_ext = nc.dram_tensor("x", [BPC, CH, T], mdt,
                               kind="ExternalInput").ap()
    git_ext = nc.dram_tensor("git", [128, 2 * BPC], f32, kind="ExternalInput").ap()
    gir_ext = nc.dram_tensor("gir", [1, CH * BPC], f32, kind="ExternalInput").ap()
    ws_ext = nc.dram_tensor("ws", [K * G, D], mdt, kind="ExternalInput").ap()
    b_ext = nc.dram_tensor("bias", [1, D], f32, kind="ExternalInput").ap()
    out_ext = nc.dram_tensor("out", [BPC, D, T_OUT], f32, kind="ExternalOutput").ap()

    with tile.TileContext(nc) as tc:
        with (
            tc.tile_pool(name="const", bufs=1) as cpool,
            tc.tile_pool(name="work", bufs=2) as wpool,
            tc.tile_pool(name="ps_pool", bufs=4, space="PSUM") as ppool,
            tc.tile_pool(name="psmall", bufs=1, space="PSUM") as spool,
            tc.tile_pool(name="po_pool", bufs=3, space="PSUM") as opool,
        ):
            # --- on-chip constants (Pool engine + SWDGE queue) ---
            iota_f = cpool.tile([128, G], f32, name="iota_f")
            nc.gpsimd.iota(iota_f[:], [[1, G]], channel_multiplier=0,
                           allow_small_or_imprecise_dtypes=True)
            # --- qSP: gi rows (tiny, feed one-hot + count) then batch-1 x ---
            # (SWDGE/gpsimd DMAs have ~5us end-to-end latency; keep every
            # DMA on the two HWDGE queues)
            git_sb = cpool.tile([128, 2 * BPC], f32, name="git_sb")
            nc.sync.dma_start(git_sb[:], git_ext[:])
            gir_sb = cpool.tile([1, CH * BPC], f32, name="gir_sb")
            nc.sync.dma_start(gir_sb[:], gir_ext[:])
            if s8:
                # one DMA per batch: [128, 2*T] fp8 (4KB/partition)
                x8t = []
                for b in range(BPC):
                    t_ = wpool.tile([128, 2 * T], f8, name=f"x8{b}",
                                    tag="xp", bufs=2)
                    (nc.scalar if b == 0 else nc.sync).dma_start(
                        t_[:], x_ext[b])
                    x8t.append(t_)
            else:
                xps = [[[None, None], [None, None]] for _ in range(BPC)]
                for p in range(2):
                    for h in range(2):
                        t_ = wpool.tile([128, 1024], mdt, name=f"xp1{h}{p}",
                                        tag="xp", bufs=8)
                        nc.sync.dma_start(
                            t_[:], x_ext[1, 128 * h:128 * (h + 1),
                                         1024 * p:1024 * (p + 1)])
                        xps[1][h][p] = t_
                # --- qAct: batch-0 x tiles ---
                for p in range(2):
                    for h in range(2):
                        t_ = wpool.tile([128, 1024], mdt, name=f"xp0{h}{p}",
                                        tag="xp", bufs=8)
                        nc.scalar.dma_start(
                            t_[:], x_ext[0, 128 * h:128 * (h + 1),
                                         1024 * p:1024 * (p + 1)])
                        xps[0][h][p] = t_
            # conv stationaries + bias row: tiny, on qSP so the scheduler's
            # hoist-small-DMAs-first policy cannot delay batch-0 x on qAct;
            # one tile per matmul (matmul requires equal base partitions
            # for stationary and moving operands)
            wsj_all = []
            for j in range(NMM):
                jR = j * R
                lenj = min(R, K - jR)
                wsj = cpool.tile([G * lenj, D], mdt, name=f"wsj{j}")
                nc.sync.dma_start(wsj[:], ws_ext[G * jR:G * (jR + lenj), :])
                wsj_all.append(wsj)
            brow = cpool.tile([1, D], f32, name="brow")
            nc.sync.dma_start(brow[:], b_ext[:])

            # --- DVE: one-hot + count0 ---
            ms_all = []
            if s8:
                # M2[c, 16j+g] = (gi[b, c+128j] == g), fp8 (0/1 exact)
                for b in range(BPC):
                    m2 = wpool.tile([128, 2 * G], f8, name=f"m2{b}",
                                    tag="m", bufs=2)
                    for h in range(2):
                        nc.vector.tensor_scalar(
                            out=m2[:, G * h:G * (h + 1)], in0=iota_f[:],
                            scalar1=git_sb[:, 2 * b + h:2 * b + h + 1],
                            scalar2=None, op0=eq)
                    ms_all.append(m2)
            else:
                for b in range(BPC):
                    ms = []
                    for h in range(2):
                        m_t = wpool.tile([128, G], mdt, name=f"m{b}{h}",
                                         tag="m", bufs=4)
                        nc.vector.tensor_scalar(
                            out=m_t[:], in0=iota_f[:],
                            scalar1=git_sb[:, 2 * b + h:2 * b + h + 1],
                            scalar2=None, op0=eq)
                        ms.append(m_t)
                    ms_all.append(ms)
            # --- PE: S = M^T @ x, evacuated into swin block 0 (DVE);
            #     then SBUF->SBUF replication DMAs (qSP) ---
            swin_all = []
            for b in range(BPC):
                swin = wpool.tile([NP, T], mdt, name=f"swin{b}", tag="swin")
                if s8:
                    lhs3 = ms_all[b][:, :].rearrange("p (j g) -> p j g", j=2)
                    rhs_full = x8t[b][:, :].rearrange("p (j t) -> p j t", j=2)
                for c in range(NCHUNK):
                    off = 512 * c
                    ps = ppool.tile([G, 512], f32, name=f"ps{b}{c}", tag="ps")
                    if s8:
                        nc.tensor.matmul(
                            ps[:], lhs3, rhs_full[:, :, off:off + 512],
                            start=True, stop=True,
                            perf_mode=mybir.MatmulPerfMode.DoubleRow)
                    else:
                        p, o = off // 1024, off % 1024
                        nc.tensor.matmul(ps[:], ms_all[b][0][:],
                                         xps[b][0][p][:, o:o + 512],
                                         start=True, stop=False)
                        nc.tensor.matmul(ps[:], ms_all[b][1][:],
                                         xps[b][1][p][:, o:o + 512],
                                         start=False, stop=True)
                    if c % 2 == 0:
                        nc.vector.tensor_copy(swin[0:G, off:off + 512], ps[:])
                    else:
                        nc.scalar.copy(swin[0:G, off:off + 512], ps[:])
                # whole-row replication, the (R-1) copies of each batch
                # split across both queues to halve the serial issue cost
                engs = [nc.scalar, nc.sync] if b == 0 else [nc.sync, nc.scalar]
                for ri, r in enumerate(range(1, R)):
                    w = blk_w[r]
                    engs[ri % 2].dma_start(swin[G * r:G * (r + 1), 0:w],
                                           swin[0:G, r:r + w])
                swin_all.append(swin)

            # --- count0 (DVE, after S evacs so S never waits on it)
            #     then count0*bias broadcast to [64, BPC] on PE ---
            eq0 = wpool.tile([1, CH * BPC], f32, name="eq0", tag="eq0", bufs=1)
            nc.vector.tensor_scalar(out=eq0[:], in0=gir_sb[:], scalar1=0.0,
                                    scalar2=None, op0=eq)
            cnt2 = wpool.tile([1, BPC], f32, name="cnt2", tag="cnt2", bufs=1)
            for b in range(BPC):
                nc.vector.tensor_reduce(
                    out=cnt2[:, b:b + 1], in_=eq0[:, CH * b:CH * (b + 1)],
                    axis=mybir.AxisListType.X, op=add)
            pb = spool.tile([D, BPC], f32, name="pb", tag="pb")
            nc.tensor.matmul(pb[:], brow[:], cnt2[:],
                             start=True, stop=True)
            bcnt = wpool.tile([D, BPC], f32, name="bcnt", tag="bcnt", bufs=1)
            nc.vector.tensor_copy(bcnt[:], pb[:])

            # --- conv + fused bias-add + store ---
            for b in range(BPC):
                osb = wpool.tile([D, T_OUT], f32, name=f"osb{b}", tag="osb")
                for c in range(NCHUNK):
                    c0 = 512 * c
                    L = min(512, T_OUT - c0)
                    po = opool.tile([D, 512], f32, name=f"po{b}{c}", tag="po")
                    for j in range(NMM):
                        jR = j * R
                        lenj = min(R, K - jR)
                        nc.tensor.matmul(
                            po[:, :L],
                            wsj_all[j][:],
                            swin_all[b][0:G * lenj,
                                        c0 + jR:c0 + jR + L],
                            start=(j == 0), stop=(j == NMM - 1))
                    # fused bias add during evacuation; alternate DVE/ACT
                    if c % 2 == 0:
                        nc.vector.tensor_scalar(
                            out=osb[:, c0:c0 + L], in0=po[:, :L],
                            scalar1=bcnt[:, b:b + 1], scalar2=None, op0=add)
                    else:
                        nc.scalar.activation(
                            osb[:, c0:c0 + L], po[:, :L],
                            mybir.ActivationFunctionType.Identity,
                            bias=bcnt[:, b:b + 1])
                    if c == 1:
                        nc.scalar.dma_start(out_ext[b, :, 0:1024],
                                            osb[:, 0:1024])
                    elif c == NCHUNK - 1:
                        nc.scalar.dma_start(out_ext[b, :, 1024:T_OUT],
                                            osb[:, 1024:T_OUT])

    nc.compile()
    return nc


# ----------------------------------------------------------------------
# v1 baseline (kept for A/B comparison): config "<s_dtype>/<conv_mode>/<conv_dtype>"
def _build_v1(cfg: str):
    from concourse import bacc, tile
    import concourse.mybir as mybir

    s_dt, conv_mode, conv_dt = (cfg.split("/") + ["kaccum", "f32"])[:3] \
        if "/" in cfg else (cfg, "kaccum", cfg)
    f32 = mybir.dt.float32
    f32r = mybir.dt.float32r
    eq = mybir.AluOpType.is_equal
    mmdt = f32r if s_dt == "f32r" else f32
    use_r = s_dt == "f32r"
    cdt = f32r if conv_dt == "f32r" else f32

    nc = bacc.Bacc("TRN2", target_bir_lowering=False, debug=False,
                   num_devices=N_CORES)
    x_ext = nc.dram_tensor("x", [BPC, CH, T], f32, kind="ExternalInput").ap()
    gi_ext = nc.dram_tensor("gi", [BPC, 2, 128, 1], f32, kind="ExternalInput").ap()
    wt_ext = nc.dram_tensor("wt", [G, K * D], f32, kind="ExternalInput").ap()
    ws_ext = nc.dram_tensor("ws", [K * G, D], f32, kind="ExternalInput").ap()
    b_ext = nc.dram_tensor("bias", [1, D], f32, kind="ExternalInput").ap()
    io_ext = nc.dram_tensor("iota", [128, G], f32, kind="ExternalInput").ap()
    out_ext = nc.dram_tensor("out", [BPC, D, T_OUT], f32, kind="ExternalOutput").ap()
    swin = conv_mode == "swin"
    if swin:
        s_dram = nc.dram_tensor("s_dram", [BPC, G, T], cdt).ap()

    with tile.TileContext(nc) as tc:
        with (
            tc.tile_pool(name="const", bufs=1) as cpool,
            tc.tile_pool(name="work", bufs=2) as wpool,
            tc.tile_pool(name="ps_pool", bufs=4, space="PSUM") as ppool,
            tc.tile_pool(name="psmall", bufs=1, space="PSUM") as spool,
            tc.tile_pool(name="po_pool", bufs=3, space="PSUM") as opool,
        ):
            iota_f = cpool.tile([128, G], f32, name="iota_f")
            nc.sync.dma_start(iota_f[:], io_ext[:])
            if swin:
                ws_sb = cpool.tile([K * G, D], f32, name="ws_sb")
                nc.sync.dma_start(ws_sb[:], ws_ext[:])
                if cdt == f32r:
                    ws_r = cpool.tile([K * G, D], f32r, name="ws_r")
                    nc.vector.tensor_copy(ws_r[:], ws_sb[:])
                else:
                    ws_r = ws_sb
            else:
                wt_sb = cpool.tile([G, K * D], f32, name="wt_sb")
                nc.sync.dma_start(wt_sb[:], wt_ext[:])
                if cdt == f32r:
                    wt_r = cpool.tile([G, K * D], f32r, name="wt_r")
                    nc.vector.tensor_copy(wt_r[:], wt_sb[:])
                else:
                    wt_r = wt_sb
            brow = cpool.tile([1, D], f32, name="brow")
            nc.sync.dma_start(brow[:], b_ext[:])
            ones_col = cpool.tile([128, 1], f32, name="ones_col")
            nc.vector.memset(ones_col[:], 1.0)

            gi_all = []
            for b in range(BPC):
                gis = []
                for h in range(2):
                    gi_t = wpool.tile([128, 1], f32, name=f"gi{b}{h}",
                                      tag="gi", bufs=4)
                    nc.sync.dma_start(gi_t[:], gi_ext[b, h])
                    gis.append(gi_t)
                gi_all.append(gis)

            xps = [[[None, None], [None, None]] for _ in range(BPC)]
            for b in range(BPC):
                for h in range(2):
                    for p in range(2):
                        t_ = wpool.tile([128, 1024], f32, name=f"xp{b}{h}{p}",
                                        tag="xp", bufs=8)
                        nc.sync.dma_start(
                            t_[:],
                            x_ext[b, 128 * h:128 * (h + 1),
                                  1024 * p:1024 * (p + 1)])
                        if use_r:
                            xr = wpool.tile([128, 1024], f32r,
                                            name=f"xr{b}{h}{p}", tag="xr",
                                            bufs=8)
                            if (h + p) % 2 == 0:
                                nc.vector.tensor_copy(xr[:], t_[:])
                            else:
                                nc.scalar.activation(
                                    xr[:], t_[:],
                                    mybir.ActivationFunctionType.Copy)
                            t_ = xr
                        xps[b][h][p] = t_

            ms_all, bcnt_all = [], []
            for b in range(BPC):
                ms, ms_f = [], []
                for h in range(2):
                    gi_t = gi_all[b][h]
                    mf_t = wpool.tile([128, G], f32, name=f"mf{b}{h}",
                                      tag="mf", bufs=4)
                    nc.vector.tensor_scalar(out=mf_t[:], in0=iota_f[:],
                                            scalar1=gi_t[:, 0:1], scalar2=None,
                                            op0=eq)
                    ms_f.append(mf_t)
                    if use_r:
                        m_t = wpool.tile([128, G], mdt, name=f"m{b}{h}",
                                         tag="m", bufs=4)
                        nc.vector.tensor_copy(m_t[:], mf_t[:])
                    else:
                        m_t = mf_t
                    ms.append(m_t)
                ms_all.append(ms)

                pcnt = spool.tile([G, 1], f32, name=f"pcnt{b}", tag="pcnt")
                nc.tensor.matmul(pcnt[:], ms_f[0][:], ones_col[:],
                                 start=True, stop=False)
                nc.tensor.matmul(pcnt[:], ms_f[1][:], ones_col[:],
                                 start=False, stop=True)
                cnt_sb = wpool.tile([G, 1], f32, name=f"cnt{b}", tag="cnt")
                nc.vector.tensor_copy(cnt_sb[:], pcnt[:])
                pbc = spool.tile([D, 1], f32, name=f"pbc{b}", tag="pbc")
                nc.tensor.matmul(pbc[:], brow[:], cnt_sb[0:1, 0:1],
                                 start=True, stop=True)
                bcnt = wpool.tile([D, 1], f32, name=f"bcnt{b}", tag="bcnt")
                nc.vector.tensor_copy(bcnt[:], pbc[:])
                bcnt_all.append(bcnt)

            s_all = []
            for b in range(BPC):
                xp = xps[b]
                s_sb = wpool.tile([G, T], cdt, name=f"s{b}", tag="s")
                for c in range(4):
                    ps = ppool.tile([G, 512], f32, name=f"ps{b}{c}", tag="ps")
                    off = 512 * c
                    p, o = off // 1024, off % 1024
                    nc.tensor.matmul(ps[:], ms_all[b][0][:],
                                     xp[0][p][:, o:o + 512],
                                     start=True, stop=False)
                    nc.tensor.matmul(ps[:], ms_all[b][1][:],
                                     xp[1][p][:, o:o + 512],
                                     start=False, stop=True)
                    nc.vector.tensor_copy(s_sb[:, off:off + 512], ps[:])
                    if swin:
                        nc.sync.dma_start(s_dram[b, :, off:off + 512],
                                          s_sb[:, off:off + 512])
                s_all.append(s_sb)

            swin_all = []
            if swin:
                for b in range(BPC):
                    swin_sb = wpool.tile([K * G, T_OUT], cdt,
                                         name=f"swin{b}", tag="swin")
                    half = 1024
                    for lo, hi in ((0, half), (half, T_OUT)):
                        for k in range(K):
                            nc.sync.dma_start(
                                swin_sb[G * k:G * (k + 1), lo:hi],
                                s_dram[b, :, k + lo:k + hi])
                    swin_all.append(swin_sb)

            for b in range(BPC):
                for c in range(4):
                    c0 = 512 * c
                    L = min(512, T_OUT - c0)
                    po = opool.tile([D, 512], f32, name=f"po{b}{c}", tag="po")
                    if swin:
                        nc.tensor.matmul(po[:, :L], ws_r[:],
                                         swin_all[b][:, c0:c0 + L],
                                         start=True, stop=True)
                    else:
                        for k in range(K):
                            nc.tensor.matmul(po[:, :L],
                                             wt_r[:, D * k:D * (k + 1)],
                                             s_all[b][:, c0 + k:c0 + k + L],
                                             start=(k == 0), stop=(k == K - 1))
                    osb = wpool.tile([D, 512], f32, name=f"osb{b}{c}",
                                     tag="osb", bufs=4)
                    nc.scalar.activation(osb[:, :L], po[:, :L],
                                         mybir.ActivationFunctionType.Identity,
                                         bias=bcnt_all[b][:, 0:1])
                    nc.sync.dma_start(out_ext[b, :, c0:c0 + L], osb[:, :L])

    nc.compile()
    return nc


def _build(cfg: str):
    if cfg.startswith(("v2", "v3", "v4")):
        return _build_v2(cfg)
    return _build_v1(cfg)


def _get_nc(mm_dtype: str):
    if mm_dtype not in _COMPILED:
        _COMPILED[mm_dtype] = _build(mm_dtype)
    return _COMPILED[mm_dtype]


def _run(x, group_idxs, W, bias, mm_dtype=None, trace=False, tmpdir=None):
    from concourse.bass_utils import run_bass_kernel_spmd

    cfg = mm_dtype or MM_DTYPE
    x = np.ascontiguousarray(np.asarray(x, dtype=np.float32))
    gi_f = np.asarray(group_idxs).astype(np.float32)  # [BS, CH]
    W = np.asarray(W, dtype=np.float32)
    bias = np.asarray(bias, dtype=np.float32)
    # ws[k*16+g, d] = W[g,d,k]
    ws = np.ascontiguousarray(W.transpose(2, 0, 1).reshape(K * G, D))
    brow = np.ascontiguousarray(bias.reshape(1, D))
    nc = _get_nc(cfg)

    in_maps = []
    if cfg.startswith(("v2", "v3", "v4")):
        import ml_dtypes
        if cfg.startswith("v4"):
            # [BS, 128, 2*T] fp8: x8[b, c, j*T+t] = x[b, c+128j, t]
            x_send = np.ascontiguousarray(
                x.reshape(BS, 2, 128, T).transpose(0, 2, 1, 3)
                 .reshape(BS, 128, 2 * T)).astype(ml_dtypes.float8_e4m3)
            ws_send = ws.astype(ml_dtypes.bfloat16)
        elif cfg.startswith("v3"):
            x_send = x.astype(ml_dtypes.bfloat16)
            ws_send = ws.astype(ml_dtypes.bfloat16)
        else:
            x_send, ws_send = x, ws
        for i in range(N_CORES):
            sl = slice(i * BPC, (i + 1) * BPC)
            gi_c = gi_f[sl]  # [BPC, CH]
            # git[c, 2b+h] = gi[b, h*128+c]
            git = np.ascontiguousarray(
                gi_c.reshape(BPC * 2, 128).T)  # [128, 2*BPC]
            gir = np.ascontiguousarray(gi_c.reshape(1, BPC * CH))
            in_maps.append({
                "x": np.ascontiguousarray(x_send[sl]),
                "git": git,
                "gir": gir,
                "ws": ws_send,
                "bias": brow,
            })
    else:
        gi = gi_f.reshape(BS, 2, 128, 1)
        wt = np.ascontiguousarray(W.transpose(0, 2, 1).reshape(G, K * D))
        iota = np.ascontiguousarray(
            np.broadcast_to(np.arange(G, dtype=np.float32), (128, G)))
        for i in range(N_CORES):
            sl = slice(i * BPC, (i + 1) * BPC)
            in_maps.append({
                "x": np.ascontiguousarray(x[sl]),
                "gi": np.ascontiguousarray(gi[sl]),
                "wt": wt,
                "ws": ws,
                "bias": brow,
                "iota": iota,
            })
    res = run_bass_kernel_spmd(nc, in_maps, core_ids=list(range(N_CORES)),
                               trace=trace, tmpdir=tmpdir)
    out = np.concatenate([r["out"] for r in res.results], axis=0)
    assert out.shape == (BS, D, T_OUT)
    return out.astype(np.float32), res


def kernel(x, group_idxs, W, bias):
    out, _ = _run(x, group_idxs, W, bias)
    return out


# revision 17
# speedup vs baseline: 2.0257x; 1.0036x over previous
"""Grouped-index Conv1D (moe_routing) on 8 TRN2 NeuronCores.

Math:  out[b,d,t] = sum_c sum_k x[b,c,t+k] * W[gi[b,c],d,k] + count0[b]*bias[d]

v2 device algorithm (per core, 2 batches, data-parallel over batch):
  1. one-hot M[c,g] = (gi[b,c]==g) built on-chip (gpsimd iota + DVE is_equal)
  2. S[g,t] = sum_c M[c,g]*x[c,t]       (PE one-hot matmul, f32r via bitcast,
                                         written into swin block 0)
  3. swin[16r+g, t] = S[g, t+r]         (SBUF->SBUF DMA replication for
                                         r=1..R-1; no DRAM bounce)
  4. out[d,t] = sum_j Wj^T swin[...]    (ceil(7/R) PSUM-accumulated matmuls
                                         per 512-chunk)
  5. bias: count0 via DVE is_equal+reduce on a [1,512] gi row; one tiny
     matmul broadcasts count0*bias to [64,2]; fused add on DVE/ACT during
     PSUM->SBUF evacuation.
DMA queues: x split across qSP/qAct, control+weights on qPool (SWDGE),
replication on qSP, outputs on qAct.  (v1 used one queue: 59 serialized
DMAs ~ 35us; v2 has ~20 spread over 3 queues.)
"""

import sys
import numpy as np

sys.path.insert(0, "/opt/trn_rl_repo")

BS, CH, T = 16, 256, 2048
G, D, K = 16, 64, 7
T_OUT = T - K + 1  # 2042
N_CORES = 8
BPC = BS // N_CORES  # batches per core = 2

# default config: "v4/<R>" = fp8e4m3 DoubleRow S-stage + bf16 conv, R =
# partition replicas for the conv (R=4: 2 conv matmuls/chunk, 3 repl
# copies/batch).  "v3/<R>" = all-bf16, "v2/<R>" = f32r.
MM_DTYPE = "v4/4"

_COMPILED = {}

NCHUNK = 4  # T / 512


def _build_v2(cfg: str):
    from concourse import bacc, tile
    import concourse.mybir as mybir

    parts = cfg.split("/")
    R = int(parts[1]) if len(parts) > 1 else 4
    assert 1 <= R <= 7
    f32 = mybir.dt.float32
    f32r = mybir.dt.float32r
    # v2: f32r (raw f32 bits, 2e-2 budget); v3: bf16 (half the HBM/SBUF/DVE
    # bytes for x, swin, weights); v4: v3 + fp8e4m3 DoubleRow S stage
    # (one matmul per chunk, 2 channels contracted per cycle)
    ver = parts[0]
    s8 = ver == "v4"
    f8 = mybir.dt.float8e4
    bf16 = mybir.dt.bfloat16
    mdt = bf16 if ver in ("v3", "v4") else f32r
    eq = mybir.AluOpType.is_equal
    add = mybir.AluOpType.add
    NMM = (K + R - 1) // R  # conv matmuls per chunk
    NP = 16 * R             # swin partitions

    # block b of swin is read by conv matmul j (with blocks 0..len_j-1,
    # column window up to 1536 + j*R + 505); width needed per block:
    blk_w = [0] * R
    for j in range(NMM):
        jR = j * R
        lenj = min(R, K - jR)
        maxcol = 1536 + jR + (T_OUT - 1536) - 1  # 2041 + jR
        for b in range(lenj):
            blk_w[b] = max(blk_w[b], maxcol + 1)
    for b in range(1, R):
        assert blk_w[b] + b <= T, (b, blk_w[b])

    nc = bacc.Bacc("TRN2", target_bir_lowering=False, debug=False,
                   num_devices=N_CORES)
    if s8:
        # x8[b, c, j, t] = x[b, c + 128*j, t]  (fp8, DoubleRow k-tile layout)
        x_ext = nc.dram_tensor("x", [BPC, 128, 2 * T], f8,
                               kind="ExternalInput").ap()
    else:
        x_ext = nc.dram_tensor("x", [BPC, CH, T], mdt,
                               kind="ExternalInput").ap()
    git_ext = nc.dram_tensor("git", [128, 2 * BPC], f32, kind="ExternalInput").ap()
    gir_ext = nc.dram_tensor("gir", [1, CH * BPC], f32, kind="ExternalInput").ap()
    ws_ext = nc.dram_tensor("ws", [K * G, D], mdt, kind="ExternalInput").ap()
    b_ext = nc.dram_tensor("bias", [1, D], f32, kind="ExternalInput").ap()
    out_ext = nc.dram_tensor("out", [BPC, D, T_OUT], f32, kind="ExternalOutput").ap()

    with tile.TileContext(nc) as tc:
        with (
            tc.tile_pool(name="const", bufs=1) as cpool,
            tc.tile_pool(name="work", bufs=2) as wpool,
            tc.tile_pool(name="ps_pool", bufs=4, space="PSUM") as ppool,
            tc.tile_pool(name="psmall", bufs=1, space="PSUM") as spool,
            tc.tile_pool(name="po_pool", bufs=3, space="PSUM") as opool,
        ):
            # --- on-chip constants (Pool engine + SWDGE queue) ---
            iota_f = cpool.tile([128, G], f32, name="iota_f")
            nc.gpsimd.iota(iota_f[:], [[1, G]], channel_multiplier=0,
                           allow_small_or_imprecise_dtypes=True)
            # --- qSP: gi rows (tiny, feed one-hot + count) then batch-1 x ---
            # (SWDGE/gpsimd DMAs have ~5us end-to-end latency; keep every
            # DMA on the two HWDGE queues)
            git_sb = cpool.tile([128, 2 * BPC], f32, name="git_sb")
            nc.sync.dma_start(git_sb[:], git_ext[:])
            gir_sb = cpool.tile([1, CH * BPC], f32, name="gir_sb")
            nc.sync.dma_start(gir_sb[:], gir_ext[:])
            if s8:
                # one DMA per batch: [128, 2*T] fp8 (4KB/partition)
                x8t = []
                for b in range(BPC):
                    t_ = wpool.tile([128, 2 * T], f8, name=f"x8{b}",
                                    tag="xp", bufs=2)
                    (nc.scalar if b == 0 else nc.sync).dma_start(
                        t_[:], x_ext[b])
                    x8t.append(t_)
            else:
                xps = [[[None, None], [None, None]] for _ in range(BPC)]
                for p in range(2):
                    for h in range(2):
                        t_ = wpool.tile([128, 1024], mdt, name=f"xp1{h}{p}",
                                        tag="xp", bufs=8)
                        nc.sync.dma_start(
                            t_[:], x_ext[1, 128 * h:128 * (h + 1),
                                         1024 * p:1024 * (p + 1)])
                        xps[1][h][p] = t_
                # --- qAct: batch-0 x tiles ---
                for p in range(2):
                    for h in range(2):
                        t_ = wpool.tile([128, 1024], mdt, name=f"xp0{h}{p}",
                                        tag="xp", bufs=8)
                        nc.scalar.dma_start(
                            t_[:], x_ext[0, 128 * h:128 * (h + 1),
                                         1024 * p:1024 * (p + 1)])
                        xps[0][h][p] = t_
            # conv stationaries + bias row: tiny, on qSP so the scheduler's
            # hoist-small-DMAs-first policy cannot delay batch-0 x on qAct;
            # one tile per matmul (matmul requires equal base partitions
            # for stationary and moving operands)
            wsj_all = []
            for j in range(NMM):
                jR = j * R
                lenj = min(R, K - jR)
                wsj = cpool.tile([G * lenj, D], mdt, name=f"wsj{j}")
                nc.sync.dma_start(wsj[:], ws_ext[G * jR:G * (jR + lenj), :])
                wsj_all.append(wsj)
            brow = cpool.tile([1, D], f32, name="brow")
            nc.sync.dma_start(brow[:], b_ext[:])

            # --- DVE: one-hot + count0 ---
            ms_all = []
            if s8:
                # M2[c, 16j+g] = (gi[b, c+128j] == g), fp8 (0/1 exact)
                for b in range(BPC):
                    m2 = wpool.tile([128, 2 * G], f8, name=f"m2{b}",
                                    tag="m", bufs=2)
                    for h in range(2):
                        nc.vector.tensor_scalar(
                            out=m2[:, G * h:G * (h + 1)], in0=iota_f[:],
                            scalar1=git_sb[:, 2 * b + h:2 * b + h + 1],
                            scalar2=None, op0=eq)
                    ms_all.append(m2)
            else:
                for b in range(BPC):
                    ms = []
                    for h in range(2):
                        m_t = wpool.tile([128, G], mdt, name=f"m{b}{h}",
                                         tag="m", bufs=4)
                        nc.vector.tensor_scalar(
                            out=m_t[:], in0=iota_f[:],
                            scalar1=git_sb[:, 2 * b + h:2 * b + h + 1],
                            scalar2=None, op0=eq)
                        ms.append(m_t)
                    ms_all.append(ms)
            # --- PE: S = M^T @ x, evacuated into swin block 0 (DVE);
            #     then SBUF->SBUF replication DMAs (qSP) ---
            swin_all = []
            for b in range(BPC):
                swin = wpool.tile([NP, T], mdt, name=f"swin{b}", tag="swin")
                if s8:
                    lhs3 = ms_all[b][:, :].rearrange("p (j g) -> p j g", j=2)
                    rhs_full = x8t[b][:, :].rearrange("p (j t) -> p j t", j=2)
                for c in range(NCHUNK):
                    off = 512 * c
                    ps = ppool.tile([G, 512], f32, name=f"ps{b}{c}", tag="ps")
                    if s8:
                        nc.tensor.matmul(
                            ps[:], lhs3, rhs_full[:, :, off:off + 512],
                            start=True, stop=True,
                            perf_mode=mybir.MatmulPerfMode.DoubleRow)
                    else:
                        p, o = off // 1024, off % 1024
                        nc.tensor.matmul(ps[:], ms_all[b][0][:],
                                         xps[b][0][p][:, o:o + 512],
                                         start=True, stop=False)
                        nc.tensor.matmul(ps[:], ms_all[b][1][:],
                                         xps[b][1][p][:, o:o + 512],
                                         start=False, stop=True)
                    if c % 2 == 0:
                        nc.vector.tensor_copy(swin[0:G, off:off + 512], ps[:])
                    else:
                        nc.scalar.copy(swin[0:G, off:off + 512], ps[:])
                # two-slice replication split across both queues: the head
                # slice [0,520) only needs S chunks 0-1 evacuated and covers
                # everything conv chunk 0 reads, so conv starts ~2us earlier;
                # the rest fires once the batch is fully evacuated
                engs = [nc.scalar, nc.sync] if b == 0 else [nc.sync, nc.scalar]
                CUT = 520
                for s in range(2):
                    for ri, r in enumerate(range(1, R)):
                        w = blk_w[r]
                        lo, hi = (0, CUT) if s == 0 else (CUT, w)
                        assert r + hi <= T
                        engs[(ri + s) % 2].dma_start(
                            swin[G * r:G * (r + 1), lo:hi],
                            swin[0:G, r + lo:r + hi])
                swin_all.append(swin)

            # --- count0 (DVE, after S evacs so S never waits on it)
            #     then count0*bias broadcast to [64, BPC] on PE ---
            eq0 = wpool.tile([1, CH * BPC], f32, name="eq0", tag="eq0", bufs=1)
            nc.vector.tensor_scalar(out=eq0[:], in0=gir_sb[:], scalar1=0.0,
                                    scalar2=None, op0=eq)
            cnt2 = wpool.tile([1, BPC], f32, name="cnt2", tag="cnt2", bufs=1)
            for b in range(BPC):
                nc.vector.tensor_reduce(
                    out=cnt2[:, b:b + 1], in_=eq0[:, CH * b:CH * (b + 1)],
                    axis=mybir.AxisListType.X, op=add)
            pb = spool.tile([D, BPC], f32, name="pb", tag="pb")
            nc.tensor.matmul(pb[:], brow[:], cnt2[:],
                             start=True, stop=True)
            bcnt = wpool.tile([D, BPC], f32, name="bcnt", tag="bcnt", bufs=1)
            nc.vector.tensor_copy(bcnt[:], pb[:])

            # --- conv + fused bias-add + store ---
            for b in range(BPC):
                osb = wpool.tile([D, T_OUT], f32, name=f"osb{b}", tag="osb")
                for c in range(NCHUNK):
                    c0 = 512 * c
                    L = min(512, T_OUT - c0)
                    po = opool.tile([D, 512], f32, name=f"po{b}{c}", tag="po")
                    for j in range(NMM):
                        jR = j * R
                        lenj = min(R, K - jR)
                        nc.tensor.matmul(
                            po[:, :L],
                            wsj_all[j][:],
                            swin_all[b][0:G * lenj,
                                        c0 + jR:c0 + jR + L],
                            start=(j == 0), stop=(j == NMM - 1))
                    # fused bias add during evacuation; alternate DVE/ACT
                    if c % 2 == 0:
                        nc.vector.tensor_scalar(
                            out=osb[:, c0:c0 + L], in0=po[:, :L],
                            scalar1=bcnt[:, b:b + 1], scalar2=None, op0=add)
                    else:
                        nc.scalar.activation(
                            osb[:, c0:c0 + L], po[:, :L],
                            mybir.ActivationFunctionType.Identity,
                            bias=bcnt[:, b:b + 1])
                    if c == 1:
                        nc.scalar.dma_start(out_ext[b, :, 0:1024],
                                            osb[:, 0:1024])
                    elif c == NCHUNK - 1:
                        nc.scalar.dma_start(out_ext[b, :, 1024:T_OUT],
                                            osb[:, 1024:T_OUT])

    nc.compile()
    return nc


# ----------------------------------------------------------------------
# v1 baseline (kept for A/B comparison): config "<s_dtype>/<conv_mode>/<conv_dtype>"
def _build_v1(cfg: str):
    from concourse import bacc, tile
    import concourse.mybir as mybir

    s_dt, conv_mode, conv_dt = (cfg.split("/") + ["kaccum", "f32"])[:3] \
        if "/" in cfg else (cfg, "kaccum", cfg)
    f32 = mybir.dt.float32
    f32r = mybir.dt.float32r
    eq = mybir.AluOpType.is_equal
    mmdt = f32r if s_dt == "f32r" else f32
    use_r = s_dt == "f32r"
    cdt = f32r if conv_dt == "f32r" else f32

    nc = bacc.Bacc("TRN2", target_bir_lowering=False, debug=False,
                   num_devices=N_CORES)
    x_ext = nc.dram_tensor("x", [BPC, CH, T], f32, kind="ExternalInput").ap()
    gi_ext = nc.dram_tensor("gi", [BPC, 2, 128, 1], f32, kind="ExternalInput").ap()
    wt_ext = nc.dram_tensor("wt", [G, K * D], f32, kind="ExternalInput").ap()
    ws_ext = nc.dram_tensor("ws", [K * G, D], f32, kind="ExternalInput").ap()
    b_ext = nc.dram_tensor("bias", [1, D], f32, kind="ExternalInput").ap()
    io_ext = nc.dram_tensor("iota", [128, G], f32, kind="ExternalInput").ap()
    out_ext = nc.dram_tensor("out", [BPC, D, T_OUT], f32, kind="ExternalOutput").ap()
    swin = conv_mode == "swin"
    if swin:
        s_dram = nc.dram_tensor("s_dram", [BPC, G, T], cdt).ap()

    with tile.TileContext(nc) as tc:
        with (
            tc.tile_pool(name="const", bufs=1) as cpool,
            tc.tile_pool(name="work", bufs=2) as wpool,
            tc.tile_pool(name="ps_pool", bufs=4, space="PSUM") as ppool,
            tc.tile_pool(name="psmall", bufs=1, space="PSUM") as spool,
            tc.tile_pool(name="po_pool", bufs=3, space="PSUM") as opool,
        ):
            iota_f = cpool.tile([128, G], f32, name="iota_f")
            nc.sync.dma_start(iota_f[:], io_ext[:])
            if swin:
                ws_sb = cpool.tile([K * G, D], f32, name="ws_sb")
                nc.sync.dma_start(ws_sb[:], ws_ext[:])
                if cdt == f32r:
                    ws_r = cpool.tile([K * G, D], f32r, name="ws_r")
                    nc.vector.tensor_copy(ws_r[:], ws_sb[:])
                else:
                    ws_r = ws_sb
            else:
                wt_sb = cpool.tile([G, K * D], f32, name="wt_sb")
                nc.sync.dma_start(wt_sb[:], wt_ext[:])
                if cdt == f32r:
                    wt_r = cpool.tile([G, K * D], f32r, name="wt_r")
                    nc.vector.tensor_copy(wt_r[:], wt_sb[:])
                else:
                    wt_r = wt_sb
            brow = cpool.tile([1, D], f32, name="brow")
            nc.sync.dma_start(brow[:], b_ext[:])
            ones_col = cpool.tile([128, 1], f32, name="ones_col")
            nc.vector.memset(ones_col[:], 1.0)

            gi_all = []
            for b in range(BPC):
                gis = []
                for h in range(2):
                    gi_t = wpool.tile([128, 1], f32, name=f"gi{b}{h}",
                                      tag="gi", bufs=4)
                    nc.sync.dma_start(gi_t[:], gi_ext[b, h])
                    gis.append(gi_t)
                gi_all.append(gis)

            xps = [[[None, None], [None, None]] for _ in range(BPC)]
            for b in range(BPC):
                for h in range(2):
                    for p in range(2):
                        t_ = wpool.tile([128, 1024], f32, name=f"xp{b}{h}{p}",
                                        tag="xp", bufs=8)
                        nc.sync.dma_start(
                            t_[:],
                            x_ext[b, 128 * h:128 * (h + 1),
                                  1024 * p:1024 * (p + 1)])
                        if use_r:
                            xr = wpool.tile([128, 1024], f32r,
                                            name=f"xr{b}{h}{p}", tag="xr",
                                            bufs=8)
                            if (h + p) % 2 == 0:
                                nc.vector.tensor_copy(xr[:], t_[:])
                            else:
                                nc.scalar.activation(
                                    xr[:], t_[:],
                                    mybir.ActivationFunctionType.Copy)
                            t_ = xr
                        xps[b][h][p] = t_

            ms_all, bcnt_all = [], []
            for b in range(BPC):
                ms, ms_f = [], []
                for h in range(2):
                    gi_t = gi_all[b][h]
                    mf_t = wpool.tile([128, G], f32, name=f"mf{b}{h}",
                                      tag="mf", bufs=4)
                    nc.vector.tensor_scalar(out=mf_t[:], in0=iota_f[:],
                                            scalar1=gi_t[:, 0:1], scalar2=None,
                                            op0=eq)
                    ms_f.append(mf_t)
                    if use_r:
                        m_t = wpool.tile([128, G], mdt, name=f"m{b}{h}",
                                         tag="m", bufs=4)
                        nc.vector.tensor_copy(m_t[:], mf_t[:])
                    else:
                        m_t = mf_t
                    ms.append(m_t)
                ms_all.append(ms)

                pcnt = spool.tile([G, 1], f32, name=f"pcnt{b}", tag="pcnt")
                nc.tensor.matmul(pcnt[:], ms_f[0][:], ones_col[:],
                                 start=True, stop=False)
                nc.tensor.matmul(pcnt[:], ms_f[1][:], ones_col[:],
                                 start=False, stop=True)
                cnt_sb = wpool.tile([G, 1], f32, name=f"cnt{b}", tag="cnt")
                nc.vector.tensor_copy(cnt_sb[:], pcnt[:])
                pbc = spool.tile([D, 1], f32, name=f"pbc{b}", tag="pbc")
                nc.tensor.matmul(pbc[:], brow[:], cnt_sb[0:1, 0:1],
                                 start=True, stop=True)
                bcnt = wpool.tile([D, 1], f32, name=f"bcnt{b}", tag="bcnt")
                nc.vector.tensor_copy(bcnt[:], pbc[:])
                bcnt_all.append(bcnt)

            s_all = []
            for b in range(BPC):
                xp = xps[b]
                s_sb = wpool.tile([G, T], cdt, name=f"s{b}", tag="s")
                for c in range(4):
                    ps = ppool.tile([G, 512], f32, name=f"ps{b}{c}", tag="ps")
                    off = 512 * c
                    p, o = off // 1024, off % 1024
                    nc.tensor.matmul(ps[:], ms_all[b][0][:],
                                     xp[0][p][:, o:o + 512],
                                     start=True, stop=False)
                    nc.tensor.matmul(ps[:], ms_all[b][1][:],
                                     xp[1][p][:, o:o + 512],
                                     start=False, stop=True)
                    nc.vector.tensor_copy(s_sb[:, off:off + 512], ps[:])
                    if swin:
                        nc.sync.dma_start(s_dram[b, :, off:off + 512],
                                          s_sb[:, off:off + 512])
                s_all.append(s_sb)

            swin_all = []
            if swin:
                for b in range(BPC):
                    swin_sb = wpool.tile([K * G, T_OUT], cdt,
                                         name=f"swin{b}", tag="swin")
                    half = 1024
                    for lo, hi in ((0, half), (half, T_OUT)):
                        for k in range(K):
                            nc.sync.dma_start(
                                swin_sb[G * k:G * (k + 1), lo:hi],
                                s_dram[b, :, k + lo:k + hi])
                    swin_all.append(swin_sb)

            for b in range(BPC):
                for c in range(4):
                    c0 = 512 * c
                    L = min(512, T_OUT - c0)
                    po = opool.tile([D, 512], f32, name=f"po{b}{c}", tag="po")
                    if swin:
                        nc.tensor.matmul(po[:, :L], ws_r[:],
                                         swin_all[b][:, c0:c0 + L],
                                         start=True, stop=True)
                    else:
                        for k in range(K):
                            nc.tensor.matmul(po[:, :L],
                                             wt_r[:, D * k:D * (k + 1)],
                                             s_all[b][:, c0 + k:c0 + k + L],
                                             start=(k == 0), stop=(k == K - 1))
                    osb = wpool.tile([D, 512], f32, name=f"osb{b}{c}",
                                     tag="osb", bufs=4)
                    nc.scalar.activation(osb[:, :L], po[:, :L],
                                         mybir.ActivationFunctionType.Identity,
                                         bias=bcnt_all[b][:, 0:1])
                    nc.sync.dma_start(out_ext[b, :, c0:c0 + L], osb[:, :L])

    nc.compile()
    return nc


def _build(cfg: str):
    if cfg.startswith(("v2", "v3", "v4")):
        return _build_v2(cfg)
    return _build_v1(cfg)


def _get_nc(mm_dtype: str):
    if mm_dtype not in _COMPILED:
        _COMPILED[mm_dtype] = _build(mm_dtype)
    return _COMPILED[mm_dtype]


def _run(x, group_idxs, W, bias, mm_dtype=None, trace=False, tmpdir=None):
    from concourse.bass_utils import run_bass_kernel_spmd

    cfg = mm_dtype or MM_DTYPE
    x = np.ascontiguousarray(np.asarray(x, dtype=np.float32))
    gi_f = np.asarray(group_idxs).astype(np.float32)  # [BS, CH]
    W = np.asarray(W, dtype=np.float32)
    bias = np.asarray(bias, dtype=np.float32)
    # ws[k*16+g, d] = W[g,d,k]
    ws = np.ascontiguousarray(W.transpose(2, 0, 1).reshape(K * G, D))
    brow = np.ascontiguousarray(bias.reshape(1, D))
    nc = _get_nc(cfg)

    in_maps = []
    if cfg.startswith(("v2", "v3", "v4")):
        import ml_dtypes
        if cfg.startswith("v4"):
            # [BS, 128, 2*T] fp8: x8[b, c, j*T+t] = x[b, c+128j, t]
            x_send = np.ascontiguousarray(
                x.reshape(BS, 2, 128, T).transpose(0, 2, 1, 3)
                 .reshape(BS, 128, 2 * T)).astype(ml_dtypes.float8_e4m3)
            ws_send = ws.astype(ml_dtypes.bfloat16)
        elif cfg.startswith("v3"):
            x_send = x.astype(ml_dtypes.bfloat16)
            ws_send = ws.astype(ml_dtypes.bfloat16)
        else:
            x_send, ws_send = x, ws
        for i in range(N_CORES):
            sl = slice(i * BPC, (i + 1) * BPC)
            gi_c = gi_f[sl]  # [BPC, CH]
            # git[c, 2b+h] = gi[b, h*128+c]
            git = np.ascontiguousarray(
                gi_c.reshape(BPC * 2, 128).T)  # [128, 2*BPC]
            gir = np.ascontiguousarray(gi_c.reshape(1, BPC * CH))
            in_maps.append({
                "x": np.ascontiguousarray(x_send[sl]),
                "git": git,
                "gir": gir,
                "ws": ws_send,
                "bias": brow,
            })
    else:
        gi = gi_f.reshape(BS, 2, 128, 1)
        wt = np.ascontiguousarray(W.transpose(0, 2, 1).reshape(G, K * D))
        iota = np.ascontiguousarray(
            np.broadcast_to(np.arange(G, dtype=np.float32), (128, G)))
        for i in range(N_CORES):
            sl = slice(i * BPC, (i + 1) * BPC)
            in_maps.append({
                "x": np.ascontiguousarray(x[sl]),
                "gi": np.ascontiguousarray(gi[sl]),
                "wt": wt,
                "ws": ws,
                "bias": brow,
                "iota": iota,
            })
    res = run_bass_kernel_spmd(nc, in_maps, core_ids=list(range(N_CORES)),
                               trace=trace, tmpdir=tmpdir)
    out = np.concatenate([r["out"] for r in res.results], axis=0)
    assert out.shape == (BS, D, T_OUT)
    return out.astype(np.float32), res


def kernel(x, group_idxs, W, bias):
    out, _ = _run(x, group_idxs, W, bias)
    return out
